# revision 51
# baseline (speedup 1.0000x reference)
"""Trainium2 Bass kernel for a 4-layer hierarchical-attention encoder.

Sharding: 8 cores = 2 batch groups x 4 sequence chunks of 512 query tokens.
Each core runs the full layer stack for its 512 tokens; the hidden state is
all-gathered (per batch group) at each layer boundary so every core can
compute full-sequence self-attention K/V locally.

Fast path (inputs with zero biases, unit LN gamma, zero LN beta — which is
what setup_inputs() produces):
 - no bias matmul rows; K-bias is dropped (exactly free under softmax)
 - residual adds ride on the PE via identity matmuls into the out-proj PSUM
 - LN sqrt computed as exp(0.5*ln(v)) so the Act engine never leaves the
   exp table on the critical path
 - exp/gelu processed on 2-bank (1024-wide) PSUM regions
 - all TM->FM transposes on the DMA transpose engine (PE/DVE freed)
 - attention denominator via a ones-mask added during the V' PSUM drain
 - batched weight DMA layouts ([128, E]-row tiles)
A general fallback (the previous kernel) handles arbitrary bias/gamma.
"""
import os
import sys

for _p in ("/root/.axon_site/_ro/trn_rl_repo", "/opt/trn_rl_repo", "/opt/pypackages",
           "/root/.axon_site/_ro/pypackages"):
    if os.path.isdir(_p) and _p not in sys.path:
        sys.path.append(_p)

import numpy as np

import concourse.bass as bass
import concourse.mybir as mybir
import concourse.tile as tile
from concourse import bacc
from concourse.bass_utils import run_bass_kernel_spmd

L, E, H, D, F = 4, 512, 8, 64, 2048
B, S, SK = 2, 2048, 1024
NCORES = 8
GROUPS = [[0, 1, 2, 3], [4, 5, 6, 7]]
CH = 512          # tokens per core
ET = E // 128     # 4 feature tiles
TT = CH // 128    # 4 token tiles in own chunk
FT = F // 128     # 16 ffn tiles
KT_SA = S // 128  # 16 key tiles (self)
KT_CA = SK // 128  # 8 key tiles (cross)
HW = 65           # head width incl. denominator column
HWP = 80          # fp8 DoubleRow head stride (M%16==0 requirement)

FP32 = mybir.dt.float32
FP16 = mybir.dt.float16
FP8 = mybir.dt.float8e4
AF = mybir.ActivationFunctionType
OP = mybir.AluOpType
PM = mybir.MatmulPerfMode
W2_SCALE = 32.0  # fp8 w2 pre-scale; exact under LN's scale invariance

_CACHE = {}

# Feature toggles for HW bring-up bisection.
# EXP_2BANK stays off: a single Act instruction reading a PSUM access
# pattern that crosses a bank boundary wedges the exec unit on HW.
USE_DMA_TRANSPOSE = os.environ.get("K_DMA_T", "1") == "1"
EXP_2BANK = os.environ.get("K_EXP2", "0") == "1"
DEBUG_DUMPS = os.environ.get("K_DEBUG", "0") == "1"
FP8_AV = os.environ.get("K_FP8AV", "1") == "1"
FP8_H2 = os.environ.get("K_FP8H2", "1") == "1"


def _patch_act_tables():
    """Steer the act-table-load pass away from the exp-less `natural_log`
    table so Ln resolves to `natural_log_exp_and_others` and the LN
    ln/exp pair never swaps tables against the attention exps.

    Only the bass-side selector sees the emptied entry; table ids and the
    hardware table contents (walrus reads act_info.json directly) are
    unchanged, so every emitted load remains valid.
    """
    import concourse.bacc as bacc_mod
    if getattr(bacc_mod, "_ln_exp_patched", False):
        return
    orig = bacc_mod.get_activation_tables

    def patched(arch):
        tables = dict(orig(arch))
        shared = "natural_log_exp_and_others"
        if shared in tables and {mybir.ActivationFunctionType.Exp,
                                 mybir.ActivationFunctionType.Ln} <= tables[shared]:
            for name, fns in tables.items():
                if name != shared:
                    tables[name] = fns - {mybir.ActivationFunctionType.Exp,
                                          mybir.ActivationFunctionType.Ln}
        return tables

    bacc_mod.get_activation_tables = patched
    bacc_mod._ln_exp_patched = True


def _build_fast():
    _patch_act_tables()
    nc = bacc.Bacc("TRN2", target_bir_lowering=False, debug=False, num_devices=NCORES)

    def din(name, shape, dt=FP16):
        return nc.dram_tensor(name, shape, dt, kind="ExternalInput").ap()

    sen_fm = din("sen_fm", [E, S])            # full batch sequence, feature-major
    own_fm0 = din("own_fm0", [E, CH])         # own chunk, feature-major
    own_tm0 = din("own_tm0", [CH, E])         # own chunk, token-major
    know_fm_d = din("know_fm", [E, SK])
    ident_d = din("ident", [128, 128])
    ident32_d = din("ident32", [128, 128])

    wq_sa = din("wq_sa", [L, ET, 128, E])
    wk_sa = din("wk_sa", [L, ET, 128, E])
    wv_sa = din("wv_sa", [L, ET, 128, H * HW])
    wo_sa = din("wo_sa", [L, ET, 128, E])
    wq_ca = din("wq_ca", [L, ET, 128, E])
    wk_ca = din("wk_ca", [L, ET, 128, E])
    wv_ca = din("wv_ca", [L, ET, 128, H * HW])
    wo_ca = din("wo_ca", [L, ET, 128, E])
    w1_d = din("w1", [L, ET, 128, F])
    w2_d = din("w2", [L, FT, 128, E], FP8 if FP8_H2 else FP16)

    out_d = nc.dram_tensor("out_tm", [CH, E], FP32, kind="ExternalOutput").ap()
    dbg = {}
    if DEBUG_DUMPS:
        for nm, shape in [("dbg_q", [128, 512]), ("dbg_k", [128, S]),
                          ("dbg_v", [128, H * HW]), ("dbg_attn", [128, 512]),
                          ("dbg_inter", [128, E]), ("dbg_co", [128, E]),
                          ("dbg_hid1", [128, E])]:
            dbg[nm] = nc.dram_tensor(nm, shape, FP16,
                                     kind="ExternalOutput").ap()

    HH = H * HW // 2  # 260, half of the padded V width

    with tile.TileContext(nc) as tc:
        from contextlib import ExitStack
        with ExitStack() as ctx:
            ep = ctx.enter_context
            const_p = ep(tc.tile_pool(name="const", bufs=1))
            know_p = ep(tc.tile_pool(name="know", bufs=1))    # [128,4096] know FM
            kfm_p = ep(tc.tile_pool(name="kfm", bufs=4))      # [128,2048] SA K fp16
            kca_p = ep(tc.tile_pool(name="kca", bufs=4))      # [128,1024] CA K fp16
            vp_p = ep(tc.tile_pool(name="vp", bufs=12))       # resident V' pairs
            hch_p = ep(tc.tile_pool(name="hch", bufs=2))      # [128,2048] H_fm chunk
            qfm_p = ep(tc.tile_pool(name="qfm", bufs=8))
            attn_p = ep(tc.tile_pool(name="attn", bufs=8))
            pt_p = ep(tc.tile_pool(name="pt", bufs=4))        # exp out [128,1024] fp16
            gel_p = ep(tc.tile_pool(name="gel", bufs=3))      # [128,1024] fp16
            stm_p = ep(tc.tile_pool(name="stm", bufs=12))     # hid/inter/co TM fp16
            xfm_p = ep(tc.tile_pool(name="xfm", bufs=2))      # inter_fm / co_fm
            ofm_p = ep(tc.tile_pool(name="ofm", bufs=2))      # own_fm
            out32_p = ep(tc.tile_pool(name="out32", bufs=2))  # final layer fp32 out
            wbig_p = ep(tc.tile_pool(name="wbig", bufs=1))    # [128,2048] weights
            wsm_p = ep(tc.tile_pool(name="wsm", bufs=1))      # [128,520] wv weights
            st_p = ep(tc.tile_pool(name="st", bufs=8))        # small stats
            rr_p = ep(tc.tile_pool(name="rr", bufs=4))        # recip rows / bcast
            ps_p = ep(tc.tile_pool(name="ps", bufs=8, space="PSUM"))
            dram_p = ep(tc.tile_pool(name="dram", bufs=2, space="DRAM"))

            def big_ps():
                return ps_p.tile([128, 1024], FP32, tag="big", name="big",
                                 bufs=2 if USE_DMA_TRANSPOSE else 1)

            def small_ps():
                return ps_p.tile([128, 512], FP32, tag="small", name="small", bufs=4)

            def fm_from_tm(out_fm, tm_tile, t):
                """FM[:, e*512 + t*128 + b] = TM[b, e*128 + p]: one batched
                DMA transpose per TM tile (out is a 3D strided AP whose
                (partition, e) dims cover the 512 transposed rows)."""
                if USE_DMA_TRANSPOSE:
                    out3 = out_fm[:].rearrange("p (e c) -> p e c", e=ET)
                    nc.sync.dma_start_transpose(
                        out3[:, :, t * 128:(t + 1) * 128], tm_tile[:])
                else:
                    for e in range(ET):
                        tp = ps_p.tile([128, 128], FP16, tag="tp", name="tp",
                                       bufs=2)
                        nc.tensor.transpose(tp[:], tm_tile[:, e * 128:(e + 1) * 128],
                                            identt[:])
                        nc.vector.tensor_copy(
                            out_fm[:, e * 512 + t * 128:e * 512 + (t + 1) * 128],
                            tp[:])

            identt = const_p.tile([128, 128], FP16, tag="ident", name="ident")
            nc.sync.dma_start(identt[:], ident_d[:])
            ident32t = const_p.tile([128, 128], FP16, tag="ident32",
                                    name="ident32")
            nc.sync.dma_start(ident32t[:], ident32_d[:])
            # ones-mask for the V' drain: 1.0 at each head's denominator
            # column (rel. cols 64,129,194,259 in each 260-wide half)
            vmask = const_p.tile([128, HH], FP16, tag="vmask", name="vmask")
            nc.vector.memset(vmask[:], 0.0)
            for hh in range(4):
                nc.vector.memset(vmask[:, hh * HW + D:hh * HW + D + 1], 1.0)

            knowfm = know_p.tile([128, ET * SK], FP16, tag="know", name="know")
            for e in range(ET):
                nc.sync.dma_start(knowfm[:, e * SK:(e + 1) * SK],
                                  know_fm_d[e * 128:(e + 1) * 128, :])

            hid = []
            for t in range(TT):
                h = stm_p.tile([128, E], FP16, tag="stm", name="stm")
                nc.sync.dma_start(h[:], own_tm0[t * 128:(t + 1) * 128, :])
                hid.append(h)
            ownfm = ofm_p.tile([128, ET * CH], FP16, tag="ofm", name="ofm")
            for e in range(ET):
                nc.sync.dma_start(ownfm[:, e * CH:(e + 1) * CH],
                                  own_fm0[e * 128:(e + 1) * 128, :])

            def load_w(dram, l, cols, tag, bufs=1):
                """One [128, ET*cols] tile; slice (ei, c) = [:, ei*cols+c]."""
                wt = wbig_p.tile([128, ET * cols], FP16, tag=tag, name=tag, bufs=bufs)
                for ei in range(ET):
                    nc.sync.dma_start(wt[:, ei * cols:(ei + 1) * cols], dram[l, ei])
                return wt

            def load_wv(dram, l, tag):
                """Four [128, H*HW] tiles, one per input-feature block ei."""
                wts = []
                for ei in range(ET):
                    wt = wsm_p.tile([128, H * HW], FP16, tag=tag, name=tag, bufs=4)
                    nc.sync.dma_start(wt[:], dram[l, ei])
                    wts.append(wt)
                return wts

            def q_proj(wq_t, src_fm):
                """Q_fm tiles [128, 512] from a single [128, ET*512] FM tile."""
                qs = []
                for e in range(ET):
                    ps = small_ps()
                    for ei in range(ET):
                        nc.tensor.matmul(
                            ps[:],
                            wq_t[:, ei * E + e * 128:ei * E + (e + 1) * 128],
                            src_fm[:, ei * 512:(ei + 1) * 512],
                            start=(ei == 0), stop=(ei == ET - 1))
                    qt = qfm_p.tile([128, 512], FP16, tag="qfm", name="qfm")
                    nc.vector.tensor_copy(qt[:], ps[:])
                    qs.append(qt)
                return qs

            def kv_chunk(kdst, col0, hch, wk_t):
                """K_fm columns [col0:col0+512) from one FM chunk tile."""
                bps = [big_ps(), big_ps()]
                for e in range(ET):
                    ps = bps[e // 2][:, (e % 2) * 512:(e % 2 + 1) * 512]
                    for ei in range(ET):
                        nc.tensor.matmul(
                            ps, wk_t[:, ei * E + e * 128:ei * E + (e + 1) * 128],
                            hch[:, ei * 512:(ei + 1) * 512],
                            start=(ei == 0), stop=(ei == ET - 1))
                    nc.vector.tensor_copy(kdst[e][:, col0:col0 + 512], ps)

            def v_chunk(vdst, kt0, hch, wv_ts):
                """V' token tiles kt0..kt0+3 into kt-pair tiles [*, 2*H*HW]."""
                for ktl in range(4):
                    psA = small_ps()
                    psB = small_ps()
                    for ei in range(ET):
                        lhs = hch[:, ei * 512 + ktl * 128:ei * 512 + (ktl + 1) * 128]
                        nc.tensor.matmul(psA[:, 0:HH], lhs, wv_ts[ei][:, 0:HH],
                                         start=(ei == 0), stop=(ei == ET - 1))
                        nc.tensor.matmul(psB[:, 0:HH], lhs, wv_ts[ei][:, HH:2 * HH],
                                         start=(ei == 0), stop=(ei == ET - 1))
                    kt = kt0 + ktl
                    vt = vdst[kt // 2]
                    j = kt % 2
                    vt4 = vt[:].rearrange("p (h two c) -> p h two c", h=H, two=2)
                    if j == 0:
                        nc.vector.memset(vt4[:, :, :, HW:HWP], 0.0)
                    m3 = vmask[:].rearrange("p (h c) -> p h c", h=4)
                    for half, psX in ((0, psA), (1, psB)):
                        p3 = psX[:, 0:HH].rearrange("p (h c) -> p h c", h=4)
                        nc.vector.tensor_add(
                            vt4[:, half * 4:half * 4 + 4, j, 0:HW], p3, m3)

            def attention(qfm, kfm, vp, nkt, attn_tiles):
                """vp: with FP8_AV, kt-PAIR tiles [128, 2*H*HW] fp8 (one per
                2 key tiles); AV runs one fp8 DoubleRow matmul per pair.
                Without FP8_AV, per-kt fp16 tiles as before."""
                nkp = nkt // 2
                for hs in range(2):
                    attps = [small_ps() for _ in range(4)]
                    pts = {}
                    for kp in range(nkp + 1):
                        for h4 in range(4):
                            h = hs * 4 + h4
                            e, r = h // 2, (h % 2) * 64
                            if kp < nkp:
                                sp2 = big_ps()
                                for j in range(2):
                                    kt = kp * 2 + j
                                    nc.tensor.matmul(
                                        sp2[:, j * 512:(j + 1) * 512],
                                        kfm[e][r:r + 64, kt * 128:(kt + 1) * 128],
                                        qfm[e][r:r + 64, :],
                                        start=True, stop=True)
                                pt = pt_p.tile([128, 1024],
                                               FP8 if FP8_AV else FP16,
                                               tag="pt", name="pt", bufs=8)
                                for j in range(2):
                                    nc.scalar.activation(
                                        pt[:, j * 512:(j + 1) * 512],
                                        sp2[:, j * 512:(j + 1) * 512],
                                        AF.Exp, scale=0.125)
                                pts[kp, h4] = pt
                            if kp >= 1:
                                pt = pts.pop((kp - 1, h4))
                                if FP8_AV:
                                    lhs3 = vp[kp - 1][:, h * 2 * HWP:
                                                      (h + 1) * 2 * HWP] \
                                        .rearrange("p (two c) -> p two c",
                                                   two=2)
                                    rhs3 = pt[:].rearrange(
                                        "p (two c) -> p two c", two=2)
                                    nc.tensor.matmul(
                                        attps[h4][0:HWP, :], lhs3, rhs3,
                                        start=(kp == 1), stop=(kp == nkp),
                                        perf_mode=PM.DoubleRow)
                                else:
                                    for j in range(2):
                                        c0 = h * 2 * HWP + j * HWP
                                        nc.tensor.matmul(
                                            attps[h4][0:HW, :],
                                            vp[kp - 1][:, c0:c0 + HW],
                                            pt[:, j * 512:(j + 1) * 512],
                                            start=(kp == 1 and j == 0),
                                            stop=(kp == nkp and j == 1))
                    for h4 in range(4):
                        h = hs * 4 + h4
                        e, r = h // 2, (h % 2) * 64
                        # den must be copied to a partition-0 SBUF tile first:
                        # custom-DVE ops mishandle partition-offset PSUM reads
                        den = rr_p.tile([1, 512], FP32, tag="den", name="den",
                                        bufs=2)
                        nc.vector.tensor_copy(den[:], attps[h4][D:D + 1, :])
                        rec = rr_p.tile([1, 512], FP32, tag="rec", name="rec", bufs=2)
                        nc.vector.reciprocal_approx_fast(rec[:], den[:])
                        rb = rr_p.tile([64, 512], FP32, tag="rb", name="rb", bufs=2)
                        nc.gpsimd.partition_broadcast(rb[:], rec[:])
                        nc.vector.tensor_mul(attn_tiles[e][r:r + 64, :],
                                             attps[h4][0:64, :], rb[:])

            def ln_tile(ps, out_t):
                """out = (ps - mean)/(bessel_std + eps), LN gamma=1 beta=0.

                sqrt runs as exp(0.5*ln(v)) so the Act engine stays in the
                ln+exp table; the final scale/shift rides on Act (Copy with
                per-partition scale/bias) to keep the serial DVE chain short.
                """
                stt = st_p.tile([128, 6], FP32, tag="bnst", name="bnst")
                nc.vector.bn_stats(out=stt[:], in_=ps)
                mv = st_p.tile([128, 2], FP32, tag="bnmv", name="bnmv")
                nc.vector.bn_aggr(out=mv[:], in_=stt[:])
                lnv = st_p.tile([128, 1], FP32, tag="lnv", name="lnv")
                nc.scalar.activation(lnv[:], mv[:, 1:2], AF.Ln,
                                     scale=float(E) / (E - 1))
                sd = st_p.tile([128, 1], FP32, tag="sd", name="sd")
                nc.scalar.activation(sd[:], lnv[:], AF.Exp, scale=0.5)
                nc.vector.tensor_scalar_add(sd[:], sd[:], 1e-6)
                inv = st_p.tile([128, 1], FP32, tag="inv", name="inv")
                nc.vector.reciprocal_approx_fast(inv[:], sd[:])
                negm = st_p.tile([128, 1], FP32, tag="negm", name="negm")
                nc.vector.tensor_scalar_mul(negm[:], mv[:, 0:1], -1.0)
                nm = st_p.tile([128, 1], FP32, tag="nm", name="nm")
                nc.vector.tensor_mul(nm[:], negm[:], inv[:])
                nc.scalar.activation(out_t, ps, AF.Identity, scale=inv[:],
                                     bias=nm[:])

            def out_ln(attn_tiles, wo_t, res_tiles, out_tm, out_fm):
                """out-proj + residual (ident matmul) + LN + DMA transpose."""
                bps = [big_ps(), big_ps()]
                pss = []
                for t in range(TT):
                    ps = bps[t // 2][:, (t % 2) * 512:(t % 2 + 1) * 512]
                    for ei in range(ET):
                        nc.tensor.matmul(
                            ps, attn_tiles[ei][:, t * 128:(t + 1) * 128],
                            wo_t[:, ei * E:(ei + 1) * E],
                            start=(ei == 0), stop=False)
                    nc.tensor.matmul(ps, identt[:], res_tiles[t][:],
                                     start=False, stop=True)
                    pss.append(ps)
                for t in range(TT):
                    ln_tile(pss[t], out_tm[t][:])
                    if out_fm is not None:
                        fm_from_tm(out_fm, out_tm[t], t)

            # --- CA K/V (uses knowfm, which is a 2-chunk FM source) ---
            def ca_kv(l, wk_t, wv_ts):
                kca = [kca_p.tile([128, SK], FP16, tag="kca", name="kca")
                       for _ in range(ET)]
                for c2 in range(2):
                    bps = [big_ps(), big_ps()]
                    for e in range(ET):
                        ps = bps[e // 2][:, (e % 2) * 512:(e % 2 + 1) * 512]
                        for ei in range(ET):
                            nc.tensor.matmul(
                                ps, wk_t[:, ei * E + e * 128:ei * E + (e + 1) * 128],
                                knowfm[:, ei * SK + c2 * 512:ei * SK + (c2 + 1) * 512],
                                start=(ei == 0), stop=(ei == ET - 1))
                        nc.vector.tensor_copy(kca[e][:, c2 * 512:(c2 + 1) * 512], ps)
                vp_ca = [vp_p.tile([128, 2 * H * HWP], FP8 if FP8_AV else FP16,
                                   tag="vp", name="vp")
                         for _ in range(KT_CA // 2)]
                for kt in range(KT_CA):
                    psA = small_ps()
                    psB = small_ps()
                    for ei in range(ET):
                        lhs = knowfm[:, ei * SK + kt * 128:ei * SK + (kt + 1) * 128]
                        nc.tensor.matmul(psA[:, 0:HH], lhs, wv_ts[ei][:, 0:HH],
                                         start=(ei == 0), stop=(ei == ET - 1))
                        nc.tensor.matmul(psB[:, 0:HH], lhs,
                                         wv_ts[ei][:, HH:2 * HH],
                                         start=(ei == 0), stop=(ei == ET - 1))
                    vt = vp_ca[kt // 2]
                    j = kt % 2
                    vt4 = vt[:].rearrange("p (h two c) -> p h two c", h=H, two=2)
                    if j == 0:
                        nc.vector.memset(vt4[:, :, :, HW:HWP], 0.0)
                    m3 = vmask[:].rearrange("p (h c) -> p h c", h=4)
                    for half, psX in ((0, psA), (1, psB)):
                        p3 = psX[:, 0:HH].rearrange("p (h c) -> p h c", h=4)
                        nc.vector.tensor_add(
                            vt4[:, half * 4:half * 4 + 4, j, 0:HW], p3, m3)
                return kca, vp_ca

            # --- layer 0 prologue: weights + CA KV + own Q ---
            wq_sa_t = load_w(wq_sa, 0, E, "wq_sa")
            wk_sa_t = load_w(wk_sa, 0, E, "wk_sa")
            wv_sa_t = load_wv(wv_sa, 0, "wv_sa")
            wo_sa_t = load_w(wo_sa, 0, E, "wo_sa")
            wq_ca_t = load_w(wq_ca, 0, E, "wq_ca")
            wk_ca_t = load_w(wk_ca, 0, E, "wk_ca")
            wv_ca_t = load_wv(wv_ca, 0, "wv_ca")
            wo_ca_t = load_w(wo_ca, 0, E, "wo_ca")

            ca_state = ca_kv(0, wk_ca_t, wv_ca_t)
            qsa = q_proj(wq_sa_t, ownfm)

            ag_out_prev = None
            for l in range(L):
                with nc.named_scope(f"L{l}"):
                    kca, vp_ca = ca_state
                    # ---- SA K/V from the gathered hidden state ----
                    ksa = [kfm_p.tile([128, S], FP16, tag="kfm", name="kfm")
                           for _ in range(ET)]
                    vp_sa = [vp_p.tile([128, 2 * H * HWP],
                                       FP8 if FP8_AV else FP16,
                                       tag="vp", name="vp")
                             for _ in range(KT_SA // 2)]
                    for ch in range(4):
                        hch = hch_p.tile([128, ET * 512], FP16, tag="hch",
                                         name="hch")
                        for ei in range(ET):
                            if l == 0:
                                nc.sync.dma_start(
                                    hch[:, ei * 512:(ei + 1) * 512],
                                    sen_fm[ei * 128:(ei + 1) * 128,
                                           ch * 512:(ch + 1) * 512])
                            else:
                                nc.sync.dma_start(
                                    hch[:, ei * 512:(ei + 1) * 512],
                                    ag_out_prev[ch * 512 + ei * 128:
                                                ch * 512 + (ei + 1) * 128, :])
                        kv_chunk(ksa, ch * 512, hch, wk_sa_t)
                        v_chunk(vp_sa, ch * 4, hch, wv_sa_t)

                    # prefetch next layer K/V/Q weights (rings just freed)
                    if l < L - 1:
                        wk_sa_t = load_w(wk_sa, l + 1, E, "wk_sa")
                        wv_sa_t = load_wv(wv_sa, l + 1, "wv_sa")
                        wq_sa_t = load_w(wq_sa, l + 1, E, "wq_sa")
                        wk_ca_t = load_w(wk_ca, l + 1, E, "wk_ca")
                        wv_ca_t = load_wv(wv_ca, l + 1, "wv_ca")

                    # ---- SA attention + out-proj + LN1 ----
                    attn = [attn_p.tile([128, 512], FP16, tag="attn", name="attn")
                            for _ in range(ET)]
                    attention(qsa, ksa, vp_sa, KT_SA, attn)
                    inter = [stm_p.tile([128, E], FP16, tag="stm", name="stm")
                             for _ in range(TT)]
                    interfm = xfm_p.tile([128, ET * CH], FP16, tag="xfm",
                                         name="xfm")
                    out_ln(attn, wo_sa_t, hid, inter, interfm)
                    if DEBUG_DUMPS and l == 0:
                        nc.sync.dma_start(dbg["dbg_q"][:], qsa[0][:])
                        nc.sync.dma_start(dbg["dbg_k"][:], ksa[0][:])
                        nc.sync.dma_start(dbg["dbg_v"][:], vp_sa[0][:])
                        nc.sync.dma_start(dbg["dbg_attn"][:], attn[0][:])
                        nc.sync.dma_start(dbg["dbg_inter"][:], inter[0][:])
                    if l < L - 1:
                        wo_sa_t = load_w(wo_sa, l + 1, E, "wo_sa")

                    # ---- CA Q + attention + out-proj + LN2 ----
                    qca = q_proj(wq_ca_t, interfm)
                    if l < L - 1:
                        wq_ca_t = load_w(wq_ca, l + 1, E, "wq_ca")
                    attn2 = [attn_p.tile([128, 512], FP16, tag="attn", name="attn")
                             for _ in range(ET)]
                    attention(qca, kca, vp_ca, KT_CA, attn2)
                    co = [stm_p.tile([128, E], FP16, tag="stm", name="stm")
                          for _ in range(TT)]
                    cofm = xfm_p.tile([128, ET * CH], FP16, tag="xfm",
                                      name="xfm")
                    out_ln(attn2, wo_ca_t, inter, co, cofm)
                    if DEBUG_DUMPS and l == 0:
                        nc.sync.dma_start(dbg["dbg_co"][:], co[0][:])
                    if l < L - 1:
                        wo_ca_t = load_w(wo_ca, l + 1, E, "wo_ca")

                    # ---- FFN: software-pipelined h1 -> gelu -> h2 ----
                    w1_ts = []
                    for ei in range(ET):
                        wt = wbig_p.tile([128, F], FP16, tag="w1", name="w1",
                                         bufs=4)
                        nc.sync.dma_start(wt[:], w1_d[l, ei])
                        w1_ts.append(wt)
                    w2_t = wbig_p.tile([128, FT * E], FP8 if FP8_H2 else FP16,
                                       tag="w2", name="w2", bufs=1)
                    for ft in range(FT):
                        nc.sync.dma_start(w2_t[:, ft * E:(ft + 1) * E],
                                          w2_d[l, ft])
                    h2ps = [small_ps() for _ in range(TT)]
                    gts = {}
                    for fp in range(9):
                        if fp < 8:
                            sp2 = big_ps()
                            for j in range(2):
                                ft = fp * 2 + j
                                for ei in range(ET):
                                    nc.tensor.matmul(
                                        sp2[:, j * 512:(j + 1) * 512],
                                        w1_ts[ei][:, ft * 128:(ft + 1) * 128],
                                        cofm[:, ei * 512:(ei + 1) * 512],
                                        start=(ei == 0), stop=(ei == ET - 1))
                            gt = gel_p.tile([128, 1024],
                                            FP8 if FP8_H2 else FP16,
                                            tag="gel", name="gel")
                            gt4 = gt[:].rearrange(
                                "p (t two c) -> p t two c", t=TT, two=2)
                            for j in range(2):
                                nc.scalar.activation(
                                    gt4[:, :, j, :],
                                    sp2[:, j * 512:(j + 1) * 512]
                                    .rearrange("p (t c) -> p t c", t=TT),
                                    AF.Gelu)
                            gts[fp] = gt
                        if fp >= 1:
                            gt = gts.pop(fp - 1)
                            if FP8_H2:
                                w23 = w2_t[:, (fp - 1) * 2 * E:fp * 2 * E] \
                                    .rearrange("p (two c) -> p two c", two=2)
                                for t in range(TT):
                                    nc.tensor.matmul(
                                        h2ps[t][:],
                                        gt[:, t * 256:(t + 1) * 256]
                                        .rearrange("p (two c) -> p two c",
                                                   two=2),
                                        w23, start=(fp == 1), stop=False,
                                        perf_mode=PM.DoubleRow)
                            else:
                                for j in range(2):
                                    ft = (fp - 1) * 2 + j
                                    for t in range(TT):
                                        nc.tensor.matmul(
                                            h2ps[t][:],
                                            gt[:, t * 256 + j * 128:
                                               t * 256 + (j + 1) * 128],
                                            w2_t[:, ft * E:(ft + 1) * E],
                                            start=(ft == 0), stop=False)
                    for t in range(TT):
                        nc.tensor.matmul(h2ps[t][:],
                                         ident32t[:] if FP8_H2 else identt[:],
                                         co[t][:], start=False, stop=True)
                    if l == L - 1:
                        for t in range(TT):
                            o32 = out32_p.tile([128, E], FP32, tag="out32",
                                               name="out32")
                            ln_tile(h2ps[t][:], o32[:])
                            nc.sync.dma_start(out_d[t * 128:(t + 1) * 128, :],
                                              o32[:])
                    else:
                        hidn = [stm_p.tile([128, E], FP16, tag="stm", name="stm")
                                for _ in range(TT)]
                        ownfm_n = ofm_p.tile([128, ET * CH], FP16, tag="ofm",
                                             name="ofm")
                        for t in range(TT):
                            ln_tile(h2ps[t][:], hidn[t][:])
                            fm_from_tm(ownfm_n, hidn[t], t)
                        if DEBUG_DUMPS and l == 0:
                            nc.sync.dma_start(dbg["dbg_hid1"][:], hidn[0][:])
                        ag_in = dram_p.tile([CH, E], FP16, tag="agin", name="agin")
                        for e in range(ET):
                            nc.sync.dma_start(ag_in[e * 128:(e + 1) * 128, :],
                                              ownfm_n[:, e * CH:(e + 1) * CH])
                        ag_out = dram_p.tile([S, E], FP16, tag="agout",
                                             name="agout")
                        nc.gpsimd.collective_compute(
                            "AllGather", OP.bypass, replica_groups=GROUPS,
                            ins=[ag_in.opt()], outs=[ag_out.opt()])
                        # AG-independent work fills the collective latency
                        ca_state = ca_kv(l + 1, wk_ca_t, wv_ca_t)
                        qsa = q_proj(wq_sa_t, ownfm_n)
                        ag_out_prev = ag_out
                        hid = hidn

    nc.compile()
    return nc


def _prep_inputs_fast(sen, know, sa_qkv_w, sa_qkv_b, sa_out_w, sa_out_b,
                      ca_qkv_w, ca_qkv_b, ca_out_w, ca_out_b,
                      ff_w1, ff_b1, ff_w2, ff_b2, ln_g, ln_b):
    f16 = np.float16

    def rowtile(w):  # [L,E,cols] -> [L,ET,128,cols]
        return np.ascontiguousarray(w.reshape(L, ET, 128, -1).astype(f16))

    def padv(w):  # [L,E,E] -> [L,ET,128,H*HW], no bias/ones (mask adds ones)
        wp = np.zeros((L, E, H, HW), np.float32)
        wp[:, :, :, :D] = w.reshape(L, E, H, D)
        return np.ascontiguousarray(wp.reshape(L, ET, 128, H * HW).astype(f16))

    f8 = mybir.dt.np(FP8)
    common = {
        "ident": np.eye(128, dtype=f16),
        "ident32": (np.eye(128) * (W2_SCALE if FP8_H2 else 1.0)).astype(f16),
        "wq_sa": rowtile(sa_qkv_w[:, 0]), "wk_sa": rowtile(sa_qkv_w[:, 1]),
        "wv_sa": padv(sa_qkv_w[:, 2]),
        "wo_sa": rowtile(sa_out_w),
        "wq_ca": rowtile(ca_qkv_w[:, 0]), "wk_ca": rowtile(ca_qkv_w[:, 1]),
        "wv_ca": padv(ca_qkv_w[:, 2]),
        "wo_ca": rowtile(ca_out_w),
        "w1": rowtile(ff_w1),
        "w2": np.ascontiguousarray(
            (ff_w2 * W2_SCALE).reshape(L, FT, 128, E).astype(f8))
        if FP8_H2 else
        np.ascontiguousarray(ff_w2.reshape(L, FT, 128, E).astype(f16)),
    }
    in_maps = []
    for core in range(NCORES):
        g, c = core // 4, core % 4
        m = dict(common)
        m["sen_fm"] = np.ascontiguousarray(sen[g].T.astype(f16))
        m["own_fm0"] = np.ascontiguousarray(sen[g, c * CH:(c + 1) * CH].T.astype(f16))
        m["own_tm0"] = np.ascontiguousarray(sen[g, c * CH:(c + 1) * CH].astype(f16))
        m["know_fm"] = np.ascontiguousarray(know[g].T.astype(f16))
        in_maps.append(m)
    return in_maps


def _build_general():
    """Fallback for inputs with non-zero biases / non-unit LN gamma."""
    nc = bacc.Bacc("TRN2", target_bir_lowering=False, debug=False, num_devices=NCORES)

    def din(name, shape, dt=FP16):
        return nc.dram_tensor(name, shape, dt, kind="ExternalInput").ap()

    sen_fm = din("sen_fm", [E, S])
    own_fm0 = din("own_fm0", [E, CH])
    own_tm0 = din("own_tm0", [CH, E])
    know_fm_d = din("know_fm", [E, SK])
    ident_d = din("ident", [128, 128])
    ones_d = din("ones", [1, 128])

    wq_sa = din("wq_sa", [L, ET, ET, 128, 128])
    wk_sa = din("wk_sa", [L, ET, ET, 128, 128])
    wv_sa = din("wv_sa", [L, ET, 128, H * HW])
    wo_sa = din("wo_sa", [L, ET, 128, E])
    wq_ca = din("wq_ca", [L, ET, ET, 128, 128])
    wk_ca = din("wk_ca", [L, ET, ET, 128, 128])
    wv_ca = din("wv_ca", [L, ET, 128, H * HW])
    wo_ca = din("wo_ca", [L, ET, 128, E])
    w1_d = din("w1", [L, ET, FT, 128, 128])
    w2_d = din("w2", [L, FT, 128, E])

    bq_sa = din("bq_sa", [L, 128, ET], FP32)
    bk_sa = din("bk_sa", [L, 128, ET], FP32)
    bq_ca = din("bq_ca", [L, 128, ET], FP32)
    bk_ca = din("bk_ca", [L, 128, ET], FP32)
    b1_d = din("b1", [L, 128, FT], FP32)
    rbv_sa = din("rbv_sa", [L, 1, H * HW])
    rbo_sa = din("rbo_sa", [L, 1, E])
    rbv_ca = din("rbv_ca", [L, 1, H * HW])
    rbo_ca = din("rbo_ca", [L, 1, E])
    rb2_d = din("rb2", [L, 1, E])
    lng_d = din("lng", [L, 1, E], FP32)
    lnb_d = din("lnb", [L, 1, E], FP32)

    out_d = nc.dram_tensor("out_tm", [CH, E], FP32, kind="ExternalOutput").ap()

    with tile.TileContext(nc) as tc:
        from contextlib import ExitStack
        with ExitStack() as ctx:
            ep = ctx.enter_context
            const_p = ep(tc.tile_pool(name="const", bufs=1))
            know_p = ep(tc.tile_pool(name="know", bufs=4))
            kfm_p = ep(tc.tile_pool(name="kfm", bufs=4))
            kca_p = ep(tc.tile_pool(name="kca", bufs=4))
            vp_p = ep(tc.tile_pool(name="vp", bufs=27))
            hch_p = ep(tc.tile_pool(name="hch", bufs=6))
            qfm_p = ep(tc.tile_pool(name="qfm", bufs=8))
            attn_p = ep(tc.tile_pool(name="attn", bufs=4))
            ofm_p = ep(tc.tile_pool(name="ofm", bufs=8))
            xfm_p = ep(tc.tile_pool(name="xfm", bufs=5))
            stm_p = ep(tc.tile_pool(name="stm", bufs=8))
            out32_p = ep(tc.tile_pool(name="out32", bufs=2))
            pt_p = ep(tc.tile_pool(name="pt", bufs=6))
            gel_p = ep(tc.tile_pool(name="gel", bufs=17))
            wl_p = ep(tc.tile_pool(name="wl", bufs=16))
            wr_p = ep(tc.tile_pool(name="wr", bufs=6))
            row_p = ep(tc.tile_pool(name="row", bufs=4))
            gb_p = ep(tc.tile_pool(name="gb", bufs=2))
            sc_p = ep(tc.tile_pool(name="sc", bufs=3))
            s1_p = ep(tc.tile_pool(name="s1", bufs=2))
            st_p = ep(tc.tile_pool(name="st", bufs=8))
            ps_p = ep(tc.tile_pool(name="ps", bufs=8, space="PSUM"))
            dram_p = ep(tc.tile_pool(name="dram", bufs=2, space="DRAM"))

            identt = const_p.tile([128, 128], FP16, tag="ident", name="ident")
            nc.sync.dma_start(identt[:], ident_d[:])
            onest = const_p.tile([1, 128], FP16, tag="ones", name="ones")
            nc.sync.dma_start(onest[:], ones_d[:])
            knowfm = []
            for e in range(ET):
                t = know_p.tile([128, SK], FP16, tag="know", name="know")
                nc.sync.dma_start(t[:], know_fm_d[e * 128:(e + 1) * 128, :])
                knowfm.append(t)

            hid = []
            for t in range(TT):
                h = stm_p.tile([128, E], FP16, tag="stm", name="stm")
                nc.sync.dma_start(h[:], own_tm0[t * 128:(t + 1) * 128, :])
                hid.append(h)
            ownfm = []
            for e in range(ET):
                t = ofm_p.tile([128, CH], FP16, tag="ofm", name="ofm")
                nc.sync.dma_start(t[:], own_fm0[e * 128:(e + 1) * 128, :])
                ownfm.append(t)

            def ln_norm(xres, G, Bt, out):
                stt = st_p.tile([128, 6], FP32, tag="bnst", name="bnst")
                nc.vector.bn_stats(out=stt[:], in_=xres[:])
                mv = st_p.tile([128, 2], FP32, tag="bnmv", name="bnmv")
                nc.vector.bn_aggr(out=mv[:], in_=stt[:])
                sd = st_p.tile([128, 1], FP32, tag="sd", name="sd")
                nc.scalar.activation(sd[:], mv[:, 1:2], AF.Sqrt,
                                     scale=float(E) / (E - 1))
                nc.vector.tensor_scalar_add(sd[:], sd[:], 1e-6)
                inv = st_p.tile([128, 1], FP32, tag="inv", name="inv")
                nc.vector.reciprocal_approx_fast(inv[:], sd[:])
                minv = st_p.tile([128, 1], FP32, tag="minv", name="minv")
                nc.vector.tensor_mul(minv[:], mv[:, 0:1], inv[:])
                tmp = sc_p.tile([128, E], FP32, tag="lntmp", name="lntmp")
                nc.vector.tensor_scalar(tmp[:], in0=xres[:], scalar1=inv[:],
                                        scalar2=minv[:], op0=OP.mult,
                                        op1=OP.subtract)
                nc.vector.tensor_mul(tmp[:], tmp[:], G[:])
                nc.vector.tensor_add(out[:], tmp[:], Bt[:])

            def transpose_to(dst_tiles, src_tile, t):
                for e in range(ET):
                    tp = ps_p.tile([128, 128], FP16, tag="ps", name="ps")
                    nc.tensor.transpose(tp[:], src_tile[:, e * 128:(e + 1) * 128],
                                        identt[:])
                    nc.vector.tensor_copy(dst_tiles[e][:, t * 128:(t + 1) * 128],
                                          tp[:])

            def load_w16(wdram, l):
                ts = {}
                for ei in range(ET):
                    for e in range(ET):
                        wt = wl_p.tile([128, 128], FP16, tag="wl", name="wl")
                        nc.sync.dma_start(wt[:], wdram[l, ei, e])
                        ts[ei, e] = wt
                return ts

            def load_bias(bdram, l, n):
                bt = st_p.tile([128, n], FP32, tag="bias", name="bias", bufs=6)
                nc.sync.dma_start(bt[:], bdram[l])
                return bt

            def kv_proj(kdst, n_tok, src_tiles, src_col0, wk_tiles, bkt):
                nch = n_tok // 512
                for e in range(ET):
                    for c2 in range(nch):
                        pst = ps_p.tile([128, 512], FP32, tag="ps", name="ps")
                        for ei in range(ET):
                            nc.tensor.matmul(pst[:], wk_tiles[ei, e][:],
                                             src_tiles[ei][:, c2 * 512:(c2 + 1) * 512],
                                             start=(ei == 0), stop=(ei == ET - 1))
                        nc.vector.tensor_scalar_add(
                            kdst[e][:, src_col0 + c2 * 512:src_col0 + (c2 + 1) * 512],
                            pst[:], bkt[:, e:e + 1])

            def v_proj(vdst, kt0, nkt, src_tiles, wv_tiles, rbv):
                for ktl in range(nkt):
                    vt = vdst[kt0 + ktl]
                    for half in range(2):
                        pst = ps_p.tile([128, H * HW // 2], FP32, tag="ps",
                                        name="ps")
                        cs = half * (H * HW // 2)
                        for ei in range(ET):
                            nc.tensor.matmul(
                                pst[:], src_tiles[ei][:, ktl * 128:(ktl + 1) * 128],
                                wv_tiles[ei][:, cs:cs + H * HW // 2],
                                start=(ei == 0), stop=False)
                        nc.tensor.matmul(pst[:], onest[:],
                                         rbv[:, cs:cs + H * HW // 2],
                                         start=False, stop=True)
                        nc.vector.tensor_copy(vt[:, cs:cs + H * HW // 2], pst[:])

            def attention(qfm, kfm, vp, nkt, attn_tiles):
                for hs in range(2):
                    attps = [ps_p.tile([HW, 512], FP32, tag="ps", name="ps")
                             for _ in range(4)]
                    for kt in range(nkt):
                        for h4 in range(4):
                            h = hs * 4 + h4
                            e, r = h // 2, (h % 2) * 64
                            spt = ps_p.tile([128, 512], FP32, tag="ps", name="ps")
                            nc.tensor.matmul(
                                spt[:], kfm[e][r:r + 64, kt * 128:(kt + 1) * 128],
                                qfm[e][r:r + 64, :], start=True, stop=True)
                            pt = pt_p.tile([128, 512], FP16, tag="pt", name="pt")
                            nc.scalar.activation(pt[:], spt[:], AF.Exp, scale=0.125)
                            nc.tensor.matmul(attps[h4][:],
                                             vp[kt][:, h * HW:(h + 1) * HW],
                                             pt[:], start=(kt == 0),
                                             stop=(kt == nkt - 1))
                    for h4 in range(4):
                        h = hs * 4 + h4
                        e, r = h // 2, (h % 2) * 64
                        ats = sc_p.tile([64, 512], FP32, tag="ats", name="ats",
                                        bufs=4)
                        nc.scalar.activation(ats[:], attps[h4][0:64, :], AF.Copy)
                        den = s1_p.tile([1, 512], FP32, tag="den", name="den")
                        nc.vector.tensor_copy(den[:], attps[h4][64:65, :])
                        rec = s1_p.tile([1, 512], FP32, tag="rec", name="rec")
                        nc.vector.reciprocal_approx_fast(rec[:], den[:])
                        rb = sc_p.tile([64, 512], FP32, tag="rb", name="rb")
                        nc.gpsimd.partition_broadcast(rb[:], rec[:])
                        nc.vector.tensor_mul(attn_tiles[e][r:r + 64, :],
                                             ats[:], rb[:])

            def out_proj_ln(attn_tiles, wo_tiles, rbo, res_tiles, G, Bt, out_tiles):
                for t in range(TT):
                    pst = ps_p.tile([128, E], FP32, tag="ps", name="ps")
                    for ei in range(ET):
                        nc.tensor.matmul(pst[:],
                                         attn_tiles[ei][:, t * 128:(t + 1) * 128],
                                         wo_tiles[ei][:], start=(ei == 0),
                                         stop=False)
                    nc.tensor.matmul(pst[:], onest[:], rbo[:], start=False,
                                     stop=True)
                    xres = sc_p.tile([128, E], FP32, tag="xres", name="xres")
                    nc.vector.tensor_add(xres[:], pst[:], res_tiles[t][:])
                    ln_norm(xres, G, Bt, out_tiles[t])

            def make_ca_kv(l):
                kca = [kca_p.tile([128, SK], FP16, tag="kca", name="kca")
                       for _ in range(ET)]
                wkt_ca = load_w16(wk_ca, l)
                bkt_ca = load_bias(bk_ca, l, ET)
                kv_proj(kca, SK, knowfm, 0, wkt_ca, bkt_ca)
                vp_ca = [vp_p.tile([128, H * HW], FP16, tag="vp", name="vp")
                         for _ in range(KT_CA)]
                wvt_ca = []
                for ei in range(ET):
                    wt = wr_p.tile([128, H * HW], FP16, tag="wr", name="wr")
                    nc.sync.dma_start(wt[:], wv_ca[l, ei])
                    wvt_ca.append(wt)
                rbv = row_p.tile([1, H * HW], FP16, tag="row", name="row")
                nc.sync.dma_start(rbv[:], rbv_ca[l])
                v_proj(vp_ca, 0, KT_CA, knowfm, wvt_ca, rbv)
                return kca, vp_ca

            ag_out_prev = None
            ca_kv_next = None
            for l in range(L):
                with nc.named_scope(f"L{l}"):
                    if l == 0:
                        kca, vp_ca = make_ca_kv(0)
                    else:
                        kca, vp_ca = ca_kv_next
                    lr = s1_p.tile([1, E], FP32, tag="lnrow", name="lnrow")
                    nc.sync.dma_start(lr[:], lng_d[l])
                    G = gb_p.tile([128, E], FP32, tag="G", name="G")
                    nc.gpsimd.partition_broadcast(G[:], lr[:])
                    lr2 = s1_p.tile([1, E], FP32, tag="lnrow", name="lnrow")
                    nc.sync.dma_start(lr2[:], lnb_d[l])
                    Bt = gb_p.tile([128, E], FP32, tag="B", name="B")
                    nc.gpsimd.partition_broadcast(Bt[:], lr2[:])

                    ksa = [kfm_p.tile([128, S], FP16, tag="kfm", name="kfm")
                           for _ in range(ET)]
                    vp_sa = [vp_p.tile([128, H * HW], FP16, tag="vp", name="vp")
                             for _ in range(KT_SA)]
                    wkt_sa = load_w16(wk_sa, l)
                    wvt_sa = []
                    for ei in range(ET):
                        wt = wr_p.tile([128, H * HW], FP16, tag="wr", name="wr")
                        nc.sync.dma_start(wt[:], wv_sa[l, ei])
                        wvt_sa.append(wt)
                    rbvs = row_p.tile([1, H * HW], FP16, tag="row", name="row")
                    nc.sync.dma_start(rbvs[:], rbv_sa[l])
                    bkt_sa = load_bias(bk_sa, l, ET)
                    for ch in range(4):
                        hch = []
                        for ei in range(ET):
                            ht = hch_p.tile([128, 512], FP16, tag="hch", name="hch")
                            if l == 0:
                                nc.sync.dma_start(
                                    ht[:], sen_fm[ei * 128:(ei + 1) * 128,
                                                  ch * 512:(ch + 1) * 512])
                            else:
                                nc.sync.dma_start(
                                    ht[:], ag_out_prev[ch * 512 + ei * 128:
                                                       ch * 512 + (ei + 1) * 128, :])
                            hch.append(ht)
                        kv_proj(ksa, 512, hch, ch * 512, wkt_sa, bkt_sa)
                        v_proj(vp_sa, ch * 4, 4, hch, wvt_sa, rbvs)

                    if l == 0:
                        qsa = [qfm_p.tile([128, 512], FP16, tag="qfm", name="qfm")
                               for _ in range(ET)]
                        wqt_sa = load_w16(wq_sa, l)
                        bqt = load_bias(bq_sa, l, ET)
                        for e in range(ET):
                            pst = ps_p.tile([128, 512], FP32, tag="ps", name="ps")
                            for ei in range(ET):
                                nc.tensor.matmul(pst[:], wqt_sa[ei, e][:],
                                                 ownfm[ei][:],
                                                 start=(ei == 0),
                                                 stop=(ei == ET - 1))
                            nc.vector.tensor_scalar_add(qsa[e][:], pst[:],
                                                        bqt[:, e:e + 1])
                    else:
                        qsa = qsa_next

                    attn = [attn_p.tile([128, 512], FP16, tag="attn", name="attn")
                            for _ in range(ET)]
                    attention(qsa, ksa, vp_sa, KT_SA, attn)
                    wot = []
                    for ei in range(ET):
                        wt = wr_p.tile([128, E], FP16, tag="wr", name="wr")
                        nc.sync.dma_start(wt[:], wo_sa[l, ei])
                        wot.append(wt)
                    rbo = row_p.tile([1, E], FP16, tag="row", name="row")
                    nc.sync.dma_start(rbo[:], rbo_sa[l])
                    inter = [stm_p.tile([128, E], FP16, tag="stm", name="stm")
                             for _ in range(TT)]
                    out_proj_ln(attn, wot, rbo, hid, G, Bt, inter)

                    interfm = [xfm_p.tile([128, CH], FP16, tag="xfm", name="xfm")
                               for _ in range(ET)]
                    for t in range(TT):
                        transpose_to(interfm, inter[t], t)

                    qca = [qfm_p.tile([128, 512], FP16, tag="qfm", name="qfm")
                           for _ in range(ET)]
                    wqt_ca = load_w16(wq_ca, l)
                    bqt_ca = load_bias(bq_ca, l, ET)
                    for e in range(ET):
                        pst = ps_p.tile([128, 512], FP32, tag="ps", name="ps")
                        for ei in range(ET):
                            nc.tensor.matmul(pst[:], wqt_ca[ei, e][:],
                                             interfm[ei][:],
                                             start=(ei == 0), stop=(ei == ET - 1))
                        nc.vector.tensor_scalar_add(qca[e][:], pst[:],
                                                    bqt_ca[:, e:e + 1])

                    attn2 = [attn_p.tile([128, 512], FP16, tag="attn", name="attn")
                             for _ in range(ET)]
                    attention(qca, kca, vp_ca, KT_CA, attn2)
                    wot2 = []
                    for ei in range(ET):
                        wt = wr_p.tile([128, E], FP16, tag="wr", name="wr")
                        nc.sync.dma_start(wt[:], wo_ca[l, ei])
                        wot2.append(wt)
                    rbo2 = row_p.tile([1, E], FP16, tag="row", name="row")
                    nc.sync.dma_start(rbo2[:], rbo_ca[l])
                    co = [stm_p.tile([128, E], FP16, tag="stm", name="stm")
                          for _ in range(TT)]
                    out_proj_ln(attn2, wot2, rbo2, inter, G, Bt, co)

                    cofm = [xfm_p.tile([128, CH], FP16, tag="xfm", name="xfm")
                            for _ in range(ET)]
                    for t in range(TT):
                        transpose_to(cofm, co[t], t)

                    rb2 = row_p.tile([1, E], FP16, tag="row", name="row")
                    nc.sync.dma_start(rb2[:], rb2_d[l])
                    b1t = load_bias(b1_d, l, FT)
                    gel = []
                    for ft in range(FT):
                        pst = ps_p.tile([128, 512], FP32, tag="ps", name="ps")
                        for ei in range(ET):
                            wt = wl_p.tile([128, 128], FP16, tag="wl", name="wl")
                            nc.sync.dma_start(wt[:], w1_d[l, ei, ft])
                            nc.tensor.matmul(pst[:], wt[:], cofm[ei][:],
                                             start=(ei == 0), stop=(ei == ET - 1))
                        gt = gel_p.tile([128, 512], FP16, tag="gel", name="gel")
                        nc.scalar.activation(gt[:], pst[:], AF.Gelu,
                                             bias=b1t[:, ft:ft + 1])
                        gel.append(gt)
                    w2ts = []
                    for ft in range(FT):
                        w2t = wr_p.tile([128, E], FP16, tag="w2r", name="w2r",
                                        bufs=17)
                        nc.sync.dma_start(w2t[:], w2_d[l, ft])
                        w2ts.append(w2t)
                    h2ps = [ps_p.tile([128, E], FP32, tag="ps", name="ps")
                            for _ in range(TT)]
                    for t in range(TT):
                        for ft in range(FT):
                            nc.tensor.matmul(h2ps[t][:],
                                             gel[ft][:, t * 128:(t + 1) * 128],
                                             w2ts[ft][:], start=(ft == 0),
                                             stop=False)
                    if l == L - 1:
                        hidn = [out32_p.tile([128, E], FP32, tag="out32",
                                             name="out32") for _ in range(TT)]
                    else:
                        hidn = [stm_p.tile([128, E], FP16, tag="stm", name="stm")
                                for _ in range(TT)]
                    for t in range(TT):
                        nc.tensor.matmul(h2ps[t][:], onest[:], rb2[:],
                                         start=False, stop=True)
                        xres = sc_p.tile([128, E], FP32, tag="xres", name="xres")
                        nc.vector.tensor_add(xres[:], h2ps[t][:], co[t][:])
                        ln_norm(xres, G, Bt, hidn[t])
                        if l == L - 1:
                            nc.sync.dma_start(out_d[t * 128:(t + 1) * 128, :],
                                              hidn[t][:])

                    if l < L - 1:
                        ownfm_n = [ofm_p.tile([128, CH], FP16, tag="ofm",
                                              name="ofm") for _ in range(ET)]
                        for t in range(TT):
                            transpose_to(ownfm_n, hidn[t], t)
                        ag_in = dram_p.tile([CH, E], FP16, tag="agin", name="agin")
                        for e in range(ET):
                            nc.sync.dma_start(ag_in[e * 128:(e + 1) * 128, :],
                                              ownfm_n[e][:])
                        ag_out = dram_p.tile([S, E], FP16, tag="agout",
                                             name="agout")
                        nc.gpsimd.collective_compute(
                            "AllGather", OP.bypass, replica_groups=GROUPS,
                            ins=[ag_in.opt()], outs=[ag_out.opt()])
                        ca_kv_next = make_ca_kv(l + 1)
                        qsa_next = [qfm_p.tile([128, 512], FP16, tag="qfm",
                                               name="qfm") for _ in range(ET)]
                        wqt_n = load_w16(wq_sa, l + 1)
                        bqt_n = load_bias(bq_sa, l + 1, ET)
                        for e in range(ET):
                            pst = ps_p.tile([128, 512], FP32, tag="ps", name="ps")
                            for ei in range(ET):
                                nc.tensor.matmul(pst[:], wqt_n[ei, e][:],
                                                 ownfm_n[ei][:],
                                                 start=(ei == 0),
                                                 stop=(ei == ET - 1))
                            nc.vector.tensor_scalar_add(qsa_next[e][:], pst[:],
                                                        bqt_n[:, e:e + 1])
                        ag_out_prev = ag_out
                        ownfm = ownfm_n
                        hid = hidn

    nc.compile()
    return nc


def _prep_inputs(sen, know, sa_qkv_w, sa_qkv_b, sa_out_w, sa_out_b,
                 ca_qkv_w, ca_qkv_b, ca_out_w, ca_out_b,
                 ff_w1, ff_b1, ff_w2, ff_b2, ln_g, ln_b):
    """Host-side weight packing for the general fallback kernel."""
    f16, f32 = np.float16, np.float32

    def tile4(w):
        return np.ascontiguousarray(
            w.reshape(L, ET, 128, ET, 128).transpose(0, 1, 3, 2, 4).astype(f16))

    def padv(w, b):
        wp = np.zeros((L, E, H, HW), f32)
        wp[:, :, :, :D] = w.reshape(L, E, H, D)
        bp = np.zeros((L, H, HW), f32)
        bp[:, :, :D] = b.reshape(L, H, D)
        bp[:, :, D] = 1.0
        return (np.ascontiguousarray(wp.reshape(L, ET, 128, H * HW).astype(f16)),
                np.ascontiguousarray(bp.reshape(L, 1, H * HW).astype(f16)))

    wv_sa_p, rbv_sa_h = padv(sa_qkv_w[:, 2], sa_qkv_b[:, 2])
    wv_ca_p, rbv_ca_h = padv(ca_qkv_w[:, 2], ca_qkv_b[:, 2])

    common = {
        "ident": np.eye(128, dtype=f16),
        "ones": np.ones((1, 128), f16),
        "wq_sa": tile4(sa_qkv_w[:, 0]), "wk_sa": tile4(sa_qkv_w[:, 1]),
        "wv_sa": wv_sa_p,
        "wo_sa": np.ascontiguousarray(sa_out_w.reshape(L, ET, 128, E).astype(f16)),
        "wq_ca": tile4(ca_qkv_w[:, 0]), "wk_ca": tile4(ca_qkv_w[:, 1]),
        "wv_ca": wv_ca_p,
        "wo_ca": np.ascontiguousarray(ca_out_w.reshape(L, ET, 128, E).astype(f16)),
        "w1": np.ascontiguousarray(
            ff_w1.reshape(L, ET, 128, FT, 128).transpose(0, 1, 3, 2, 4).astype(f16)),
        "w2": np.ascontiguousarray(ff_w2.reshape(L, FT, 128, E).astype(f16)),
        "bq_sa": np.ascontiguousarray(
            sa_qkv_b[:, 0].reshape(L, ET, 128).transpose(0, 2, 1)),
        "bk_sa": np.ascontiguousarray(
            sa_qkv_b[:, 1].reshape(L, ET, 128).transpose(0, 2, 1)),
        "bq_ca": np.ascontiguousarray(
            ca_qkv_b[:, 0].reshape(L, ET, 128).transpose(0, 2, 1)),
        "bk_ca": np.ascontiguousarray(
            ca_qkv_b[:, 1].reshape(L, ET, 128).transpose(0, 2, 1)),
        "b1": np.ascontiguousarray(
            ff_b1.reshape(L, FT, 128).transpose(0, 2, 1)),
        "rbv_sa": rbv_sa_h, "rbv_ca": rbv_ca_h,
        "rbo_sa": np.ascontiguousarray(sa_out_b[:, None, :].astype(f16)),
        "rbo_ca": np.ascontiguousarray(ca_out_b[:, None, :].astype(f16)),
        "rb2": np.ascontiguousarray(ff_b2[:, None, :].astype(f16)),
        "lng": np.ascontiguousarray(ln_g[:, None, :]),
        "lnb": np.ascontiguousarray(ln_b[:, None, :]),
    }
    in_maps = []
    for core in range(NCORES):
        g, c = core // 4, core % 4
        m = dict(common)
        m["sen_fm"] = np.ascontiguousarray(sen[g].T.astype(f16))
        m["own_fm0"] = np.ascontiguousarray(sen[g, c * CH:(c + 1) * CH].T.astype(f16))
        m["own_tm0"] = np.ascontiguousarray(sen[g, c * CH:(c + 1) * CH].astype(f16))
        m["know_fm"] = np.ascontiguousarray(know[g].T.astype(f16))
        in_maps.append(m)
    return in_maps


def _inputs_are_fast(sa_qkv_b, sa_out_b, ca_qkv_b, ca_out_b,
                     ff_b1, ff_b2, ln_g, ln_b, **_):
    zeros = [sa_qkv_b, sa_out_b, ca_qkv_b, ca_out_b, ff_b1, ff_b2, ln_b]
    return all(not np.any(z) for z in zeros) and np.all(ln_g == 1.0)


def kernel(**inputs):
    inputs = {k: np.asarray(v, dtype=np.float32) for k, v in inputs.items()}
    if _inputs_are_fast(**inputs):
        if "nc" not in _CACHE:
            _CACHE["nc"] = _build_fast()
        nc = _CACHE["nc"]
        in_maps = _prep_inputs_fast(**inputs)
    else:
        if "nc_gen" not in _CACHE:
            _CACHE["nc_gen"] = _build_general()
        nc = _CACHE["nc_gen"]
        in_maps = _prep_inputs(**inputs)
        _CACHE["nc"] = nc
    res = run_bass_kernel_spmd(nc, in_maps, list(range(NCORES)))
    out = np.empty((B, S, E), np.float32)
    for core in range(NCORES):
        g, c = core // 4, core % 4
        out[g, c * CH:(c + 1) * CH] = res.results[core]["out_tm"]
    return out


# revision 54
# speedup vs baseline: 1.0150x; 1.0150x over previous
"""Trainium2 Bass kernel for a 4-layer hierarchical-attention encoder.

Sharding: 8 cores = 2 batch groups x 4 sequence chunks of 512 query tokens.
Each core runs the full layer stack for its 512 tokens; the hidden state is
all-gathered (per batch group) at each layer boundary so every core can
compute full-sequence self-attention K/V locally.

Fast path (inputs with zero biases, unit LN gamma, zero LN beta — which is
what setup_inputs() produces):
 - no bias matmul rows; K-bias is dropped (exactly free under softmax)
 - residual adds ride on the PE via identity matmuls into the out-proj PSUM
 - LN sqrt computed as exp(0.5*ln(v)) so the Act engine never leaves the
   exp table on the critical path
 - exp/gelu processed on 2-bank (1024-wide) PSUM regions
 - all TM->FM transposes on the DMA transpose engine (PE/DVE freed)
 - attention denominator via a ones-mask added during the V' PSUM drain
 - batched weight DMA layouts ([128, E]-row tiles)
A general fallback (the previous kernel) handles arbitrary bias/gamma.
"""
import os
import sys

for _p in ("/root/.axon_site/_ro/trn_rl_repo", "/opt/trn_rl_repo", "/opt/pypackages",
           "/root/.axon_site/_ro/pypackages"):
    if os.path.isdir(_p) and _p not in sys.path:
        sys.path.append(_p)

import numpy as np

import concourse.bass as bass
import concourse.mybir as mybir
import concourse.tile as tile
from concourse import bacc
from concourse.bass_utils import run_bass_kernel_spmd

L, E, H, D, F = 4, 512, 8, 64, 2048
B, S, SK = 2, 2048, 1024
NCORES = 8
GROUPS = [[0, 1, 2, 3], [4, 5, 6, 7]]
CH = 512          # tokens per core
ET = E // 128     # 4 feature tiles
TT = CH // 128    # 4 token tiles in own chunk
FT = F // 128     # 16 ffn tiles
KT_SA = S // 128  # 16 key tiles (self)
KT_CA = SK // 128  # 8 key tiles (cross)
HW = 65           # head width incl. denominator column
HWP = 80          # fp8 DoubleRow head stride (M%16==0 requirement)

FP32 = mybir.dt.float32
FP16 = mybir.dt.float16
FP8 = mybir.dt.float8e4
AF = mybir.ActivationFunctionType
OP = mybir.AluOpType
PM = mybir.MatmulPerfMode
W2_SCALE = 32.0  # fp8 w2 pre-scale; exact under LN's scale invariance

_CACHE = {}

# Feature toggles for HW bring-up bisection.
# EXP_2BANK stays off: a single Act instruction reading a PSUM access
# pattern that crosses a bank boundary wedges the exec unit on HW.
USE_DMA_TRANSPOSE = os.environ.get("K_DMA_T", "1") == "1"
EXP_2BANK = os.environ.get("K_EXP2", "0") == "1"
DEBUG_DUMPS = os.environ.get("K_DEBUG", "0") == "1"
FP8_AV = os.environ.get("K_FP8AV", "0") == "1"
FP8_H2 = os.environ.get("K_FP8H2", "0") == "1"
HS = HWP if FP8_AV else HW  # per-(head,pair-slot) stride in V tiles


def _patch_act_tables():
    """Steer the act-table-load pass away from the exp-less `natural_log`
    table so Ln resolves to `natural_log_exp_and_others` and the LN
    ln/exp pair never swaps tables against the attention exps.

    Only the bass-side selector sees the emptied entry; table ids and the
    hardware table contents (walrus reads act_info.json directly) are
    unchanged, so every emitted load remains valid.
    """
    import concourse.bacc as bacc_mod
    if getattr(bacc_mod, "_ln_exp_patched", False):
        return
    orig = bacc_mod.get_activation_tables

    def patched(arch):
        tables = dict(orig(arch))
        shared = "natural_log_exp_and_others"
        if shared in tables and {mybir.ActivationFunctionType.Exp,
                                 mybir.ActivationFunctionType.Ln} <= tables[shared]:
            for name, fns in tables.items():
                if name != shared:
                    tables[name] = fns - {mybir.ActivationFunctionType.Exp,
                                          mybir.ActivationFunctionType.Ln}
        return tables

    bacc_mod.get_activation_tables = patched
    bacc_mod._ln_exp_patched = True


def _build_fast():
    _patch_act_tables()
    nc = bacc.Bacc("TRN2", target_bir_lowering=False, debug=False, num_devices=NCORES)

    def din(name, shape, dt=FP16):
        return nc.dram_tensor(name, shape, dt, kind="ExternalInput").ap()

    sen_fm = din("sen_fm", [E, S])            # full batch sequence, feature-major
    own_fm0 = din("own_fm0", [E, CH])         # own chunk, feature-major
    own_tm0 = din("own_tm0", [CH, E])         # own chunk, token-major
    know_fm_d = din("know_fm", [E, SK])
    ident_d = din("ident", [128, 128])
    ident32_d = din("ident32", [128, 128])

    wq_sa = din("wq_sa", [L, ET, 128, E])
    wk_sa = din("wk_sa", [L, ET, 128, E])
    wv_sa = din("wv_sa", [L, ET, 128, H * HW])
    wo_sa = din("wo_sa", [L, ET, 128, E])
    wq_ca = din("wq_ca", [L, ET, 128, E])
    wk_ca = din("wk_ca", [L, ET, 128, E])
    wv_ca = din("wv_ca", [L, ET, 128, H * HW])
    wo_ca = din("wo_ca", [L, ET, 128, E])
    w1_d = din("w1", [L, ET, 128, F])
    w2_d = din("w2", [L, FT, 128, E], FP8 if FP8_H2 else FP16)

    out_d = nc.dram_tensor("out_tm", [CH, E], FP32, kind="ExternalOutput").ap()
    dbg = {}
    if DEBUG_DUMPS:
        for nm, shape in [("dbg_q", [128, 512]), ("dbg_k", [128, S]),
                          ("dbg_v", [128, H * HW]), ("dbg_attn", [128, 512]),
                          ("dbg_inter", [128, E]), ("dbg_co", [128, E]),
                          ("dbg_hid1", [128, E])]:
            dbg[nm] = nc.dram_tensor(nm, shape, FP16,
                                     kind="ExternalOutput").ap()

    HH = H * HW // 2  # 260, half of the padded V width

    with tile.TileContext(nc) as tc:
        from contextlib import ExitStack
        with ExitStack() as ctx:
            ep = ctx.enter_context
            const_p = ep(tc.tile_pool(name="const", bufs=1))
            know_p = ep(tc.tile_pool(name="know", bufs=1))    # [128,4096] know FM
            kfm_p = ep(tc.tile_pool(name="kfm", bufs=4))      # [128,2048] SA K fp16
            kca_p = ep(tc.tile_pool(name="kca", bufs=4))      # [128,1024] CA K fp16
            vp_p = ep(tc.tile_pool(name="vp", bufs=12))       # resident V' pairs
            hch_p = ep(tc.tile_pool(name="hch", bufs=2))      # [128,2048] H_fm chunk
            qfm_p = ep(tc.tile_pool(name="qfm", bufs=8))
            attn_p = ep(tc.tile_pool(name="attn", bufs=8))
            pt_p = ep(tc.tile_pool(name="pt", bufs=4))        # exp out [128,1024] fp16
            gel_p = ep(tc.tile_pool(name="gel", bufs=2))      # [128,1024] fp16
            stm_p = ep(tc.tile_pool(name="stm", bufs=12))     # hid/inter/co TM fp16
            xfm_p = ep(tc.tile_pool(name="xfm", bufs=2))      # inter_fm / co_fm
            ofm_p = ep(tc.tile_pool(name="ofm", bufs=2))      # own_fm
            out32_p = ep(tc.tile_pool(name="out32", bufs=2))  # final layer fp32 out
            wbig_p = ep(tc.tile_pool(name="wbig", bufs=1))    # [128,2048] weights
            wsm_p = ep(tc.tile_pool(name="wsm", bufs=1))      # [128,520] wv weights
            st_p = ep(tc.tile_pool(name="st", bufs=8))        # small stats
            rr_p = ep(tc.tile_pool(name="rr", bufs=4))        # recip rows / bcast
            ps_p = ep(tc.tile_pool(name="ps", bufs=8, space="PSUM"))
            dram_p = ep(tc.tile_pool(name="dram", bufs=2, space="DRAM"))

            def big_ps():
                return ps_p.tile([128, 1024], FP32, tag="big", name="big",
                                 bufs=2 if USE_DMA_TRANSPOSE else 1)

            def small_ps():
                return ps_p.tile([128, 512], FP32, tag="small", name="small", bufs=4)

            def fm_from_tm(out_fm, tm_tile, t):
                """FM[:, e*512 + t*128 + b] = TM[b, e*128 + p]: one batched
                DMA transpose per TM tile (out is a 3D strided AP whose
                (partition, e) dims cover the 512 transposed rows)."""
                if USE_DMA_TRANSPOSE:
                    out3 = out_fm[:].rearrange("p (e c) -> p e c", e=ET)
                    nc.sync.dma_start_transpose(
                        out3[:, :, t * 128:(t + 1) * 128], tm_tile[:])
                else:
                    for e in range(ET):
                        tp = ps_p.tile([128, 128], FP16, tag="tp", name="tp",
                                       bufs=2)
                        nc.tensor.transpose(tp[:], tm_tile[:, e * 128:(e + 1) * 128],
                                            identt[:])
                        nc.vector.tensor_copy(
                            out_fm[:, e * 512 + t * 128:e * 512 + (t + 1) * 128],
                            tp[:])

            identt = const_p.tile([128, 128], FP16, tag="ident", name="ident")
            nc.sync.dma_start(identt[:], ident_d[:])
            ident32t = const_p.tile([128, 128], FP16, tag="ident32",
                                    name="ident32")
            nc.sync.dma_start(ident32t[:], ident32_d[:])
            # ones-mask for the V' drain: 1.0 at each head's denominator
            # column (rel. cols 64,129,194,259 in each 260-wide half)
            vmask = const_p.tile([128, HH], FP16, tag="vmask", name="vmask")
            nc.vector.memset(vmask[:], 0.0)
            for hh in range(4):
                nc.vector.memset(vmask[:, hh * HW + D:hh * HW + D + 1], 1.0)

            knowfm = know_p.tile([128, ET * SK], FP16, tag="know", name="know")
            for e in range(ET):
                nc.sync.dma_start(knowfm[:, e * SK:(e + 1) * SK],
                                  know_fm_d[e * 128:(e + 1) * 128, :])

            hid = []
            for t in range(TT):
                h = stm_p.tile([128, E], FP16, tag="stm", name="stm")
                nc.sync.dma_start(h[:], own_tm0[t * 128:(t + 1) * 128, :])
                hid.append(h)
            ownfm = ofm_p.tile([128, ET * CH], FP16, tag="ofm", name="ofm")
            for e in range(ET):
                nc.sync.dma_start(ownfm[:, e * CH:(e + 1) * CH],
                                  own_fm0[e * 128:(e + 1) * 128, :])

            def load_w(dram, l, cols, tag, bufs=1):
                """One [128, ET*cols] tile; slice (ei, c) = [:, ei*cols+c]."""
                wt = wbig_p.tile([128, ET * cols], FP16, tag=tag, name=tag, bufs=bufs)
                for ei in range(ET):
                    nc.sync.dma_start(wt[:, ei * cols:(ei + 1) * cols], dram[l, ei])
                return wt

            def load_wv(dram, l, tag):
                """Four [128, H*HW] tiles, one per input-feature block ei."""
                wts = []
                for ei in range(ET):
                    wt = wsm_p.tile([128, H * HW], FP16, tag=tag, name=tag, bufs=4)
                    nc.sync.dma_start(wt[:], dram[l, ei])
                    wts.append(wt)
                return wts

            def q_proj(wq_t, src_fm):
                """Q_fm tiles [128, 512] from a single [128, ET*512] FM tile."""
                qs = []
                for e in range(ET):
                    ps = small_ps()
                    for ei in range(ET):
                        nc.tensor.matmul(
                            ps[:],
                            wq_t[:, ei * E + e * 128:ei * E + (e + 1) * 128],
                            src_fm[:, ei * 512:(ei + 1) * 512],
                            start=(ei == 0), stop=(ei == ET - 1))
                    qt = qfm_p.tile([128, 512], FP16, tag="qfm", name="qfm")
                    nc.vector.tensor_copy(qt[:], ps[:])
                    qs.append(qt)
                return qs

            def kv_chunk(kdst, col0, hch, wk_t):
                """K_fm columns [col0:col0+512) from one FM chunk tile."""
                bps = [big_ps(), big_ps()]
                for e in range(ET):
                    ps = bps[e // 2][:, (e % 2) * 512:(e % 2 + 1) * 512]
                    for ei in range(ET):
                        nc.tensor.matmul(
                            ps, wk_t[:, ei * E + e * 128:ei * E + (e + 1) * 128],
                            hch[:, ei * 512:(ei + 1) * 512],
                            start=(ei == 0), stop=(ei == ET - 1))
                    nc.vector.tensor_copy(kdst[e][:, col0:col0 + 512], ps)

            def v_chunk(vdst, kt0, hch, wv_ts):
                """V' token tiles kt0..kt0+3 into kt-pair tiles [*, 2*H*HW]."""
                for ktl in range(4):
                    psA = small_ps()
                    psB = small_ps()
                    for ei in range(ET):
                        lhs = hch[:, ei * 512 + ktl * 128:ei * 512 + (ktl + 1) * 128]
                        nc.tensor.matmul(psA[:, 0:HH], lhs, wv_ts[ei][:, 0:HH],
                                         start=(ei == 0), stop=(ei == ET - 1))
                        nc.tensor.matmul(psB[:, 0:HH], lhs, wv_ts[ei][:, HH:2 * HH],
                                         start=(ei == 0), stop=(ei == ET - 1))
                    kt = kt0 + ktl
                    vt = vdst[kt // 2]
                    j = kt % 2
                    vt4 = vt[:].rearrange("p (h two c) -> p h two c", h=H, two=2)
                    if j == 0 and FP8_AV:
                        nc.vector.memset(vt4[:, :, :, HW:HWP], 0.0)
                    m3 = vmask[:].rearrange("p (h c) -> p h c", h=4)
                    for half, psX in ((0, psA), (1, psB)):
                        p3 = psX[:, 0:HH].rearrange("p (h c) -> p h c", h=4)
                        nc.vector.tensor_add(
                            vt4[:, half * 4:half * 4 + 4, j, 0:HW], p3, m3)

            def attention(qfm, kfm, vp, nkt, attn_tiles):
                """vp: with FP8_AV, kt-PAIR tiles [128, 2*H*HW] fp8 (one per
                2 key tiles); AV runs one fp8 DoubleRow matmul per pair.
                Without FP8_AV, per-kt fp16 tiles as before."""
                nkp = nkt // 2
                for hs in range(2):
                    attps = [small_ps() for _ in range(4)]
                    pts = {}
                    for kp in range(nkp + 1):
                        for h4 in range(4):
                            h = hs * 4 + h4
                            e, r = h // 2, (h % 2) * 64
                            if kp < nkp:
                                sp2 = big_ps()
                                for j in range(2):
                                    kt = kp * 2 + j
                                    nc.tensor.matmul(
                                        sp2[:, j * 512:(j + 1) * 512],
                                        kfm[e][r:r + 64, kt * 128:(kt + 1) * 128],
                                        qfm[e][r:r + 64, :],
                                        start=True, stop=True)
                                pt = pt_p.tile([128, 1024],
                                               FP8 if FP8_AV else FP16,
                                               tag="pt", name="pt",
                                               bufs=8 if FP8_AV else 6)
                                for j in range(2):
                                    nc.scalar.activation(
                                        pt[:, j * 512:(j + 1) * 512],
                                        sp2[:, j * 512:(j + 1) * 512],
                                        AF.Exp, scale=0.125)
                                pts[kp, h4] = pt
                            if kp >= 1:
                                pt = pts.pop((kp - 1, h4))
                                if FP8_AV:
                                    lhs3 = vp[kp - 1][:, h * 2 * HS:
                                                      (h + 1) * 2 * HS] \
                                        .rearrange("p (two c) -> p two c",
                                                   two=2)
                                    rhs3 = pt[:].rearrange(
                                        "p (two c) -> p two c", two=2)
                                    nc.tensor.matmul(
                                        attps[h4][0:HWP, :], lhs3, rhs3,
                                        start=(kp == 1), stop=(kp == nkp),
                                        perf_mode=PM.DoubleRow)
                                else:
                                    for j in range(2):
                                        c0 = h * 2 * HS + j * HS
                                        nc.tensor.matmul(
                                            attps[h4][0:HW, :],
                                            vp[kp - 1][:, c0:c0 + HW],
                                            pt[:, j * 512:(j + 1) * 512],
                                            start=(kp == 1 and j == 0),
                                            stop=(kp == nkp and j == 1))
                    for h4 in range(4):
                        h = hs * 4 + h4
                        e, r = h // 2, (h % 2) * 64
                        # den must be copied to a partition-0 SBUF tile first:
                        # custom-DVE ops mishandle partition-offset PSUM reads
                        den = rr_p.tile([1, 512], FP32, tag="den", name="den",
                                        bufs=2)
                        nc.vector.tensor_copy(den[:], attps[h4][D:D + 1, :])
                        rec = rr_p.tile([1, 512], FP32, tag="rec", name="rec", bufs=2)
                        nc.vector.reciprocal_approx_fast(rec[:], den[:])
                        rb = rr_p.tile([64, 512], FP32, tag="rb", name="rb", bufs=2)
                        nc.gpsimd.partition_broadcast(rb[:], rec[:])
                        nc.vector.tensor_mul(attn_tiles[e][r:r + 64, :],
                                             attps[h4][0:64, :], rb[:])

            def ln_tile(ps, out_t):
                """out = (ps - mean)/(bessel_std + eps), LN gamma=1 beta=0.

                sqrt runs as exp(0.5*ln(v)) so the Act engine stays in the
                ln+exp table; the final scale/shift rides on Act (Copy with
                per-partition scale/bias) to keep the serial DVE chain short.
                """
                stt = st_p.tile([128, 6], FP32, tag="bnst", name="bnst")
                nc.vector.bn_stats(out=stt[:], in_=ps)
                mv = st_p.tile([128, 2], FP32, tag="bnmv", name="bnmv")
                nc.vector.bn_aggr(out=mv[:], in_=stt[:])
                lnv = st_p.tile([128, 1], FP32, tag="lnv", name="lnv")
                nc.scalar.activation(lnv[:], mv[:, 1:2], AF.Ln,
                                     scale=float(E) / (E - 1))
                sd = st_p.tile([128, 1], FP32, tag="sd", name="sd")
                nc.scalar.activation(sd[:], lnv[:], AF.Exp, scale=0.5)
                nc.vector.tensor_scalar_add(sd[:], sd[:], 1e-6)
                inv = st_p.tile([128, 1], FP32, tag="inv", name="inv")
                nc.vector.reciprocal_approx_fast(inv[:], sd[:])
                negm = st_p.tile([128, 1], FP32, tag="negm", name="negm")
                nc.vector.tensor_scalar_mul(negm[:], mv[:, 0:1], -1.0)
                nm = st_p.tile([128, 1], FP32, tag="nm", name="nm")
                nc.vector.tensor_mul(nm[:], negm[:], inv[:])
                nc.scalar.activation(out_t, ps, AF.Identity, scale=inv[:],
                                     bias=nm[:])

            def out_ln(attn_tiles, wo_t, res_tiles, out_tm, out_fm):
                """out-proj + residual (ident matmul) + LN + DMA transpose."""
                bps = [big_ps(), big_ps()]
                pss = []
                for t in range(TT):
                    ps = bps[t // 2][:, (t % 2) * 512:(t % 2 + 1) * 512]
                    for ei in range(ET):
                        nc.tensor.matmul(
                            ps, attn_tiles[ei][:, t * 128:(t + 1) * 128],
                            wo_t[:, ei * E:(ei + 1) * E],
                            start=(ei == 0), stop=False)
                    nc.tensor.matmul(ps, identt[:], res_tiles[t][:],
                                     start=False, stop=True)
                    pss.append(ps)
                for t in range(TT):
                    ln_tile(pss[t], out_tm[t][:])
                    if out_fm is not None:
                        fm_from_tm(out_fm, out_tm[t], t)

            # --- CA K/V (uses knowfm, which is a 2-chunk FM source) ---
            def ca_kv(l, wk_t, wv_ts):
                kca = [kca_p.tile([128, SK], FP16, tag="kca", name="kca")
                       for _ in range(ET)]
                for c2 in range(2):
                    bps = [big_ps(), big_ps()]
                    for e in range(ET):
                        ps = bps[e // 2][:, (e % 2) * 512:(e % 2 + 1) * 512]
                        for ei in range(ET):
                            nc.tensor.matmul(
                                ps, wk_t[:, ei * E + e * 128:ei * E + (e + 1) * 128],
                                knowfm[:, ei * SK + c2 * 512:ei * SK + (c2 + 1) * 512],
                                start=(ei == 0), stop=(ei == ET - 1))
                        nc.vector.tensor_copy(kca[e][:, c2 * 512:(c2 + 1) * 512], ps)
                vp_ca = [vp_p.tile([128, 2 * H * HS], FP8 if FP8_AV else FP16,
                                   tag="vp", name="vp")
                         for _ in range(KT_CA // 2)]
                for kt in range(KT_CA):
                    psA = small_ps()
                    psB = small_ps()
                    for ei in range(ET):
                        lhs = knowfm[:, ei * SK + kt * 128:ei * SK + (kt + 1) * 128]
                        nc.tensor.matmul(psA[:, 0:HH], lhs, wv_ts[ei][:, 0:HH],
                                         start=(ei == 0), stop=(ei == ET - 1))
                        nc.tensor.matmul(psB[:, 0:HH], lhs,
                                         wv_ts[ei][:, HH:2 * HH],
                                         start=(ei == 0), stop=(ei == ET - 1))
                    vt = vp_ca[kt // 2]
                    j = kt % 2
                    vt4 = vt[:].rearrange("p (h two c) -> p h two c", h=H, two=2)
                    if j == 0 and FP8_AV:
                        nc.vector.memset(vt4[:, :, :, HW:HWP], 0.0)
                    m3 = vmask[:].rearrange("p (h c) -> p h c", h=4)
                    for half, psX in ((0, psA), (1, psB)):
                        p3 = psX[:, 0:HH].rearrange("p (h c) -> p h c", h=4)
                        nc.vector.tensor_add(
                            vt4[:, half * 4:half * 4 + 4, j, 0:HW], p3, m3)
                return kca, vp_ca

            # --- layer 0 prologue: weights + CA KV + own Q ---
            wq_sa_t = load_w(wq_sa, 0, E, "wq_sa")
            wk_sa_t = load_w(wk_sa, 0, E, "wk_sa")
            wv_sa_t = load_wv(wv_sa, 0, "wv_sa")
            wo_sa_t = load_w(wo_sa, 0, E, "wo_sa")
            wq_ca_t = load_w(wq_ca, 0, E, "wq_ca")
            wk_ca_t = load_w(wk_ca, 0, E, "wk_ca")
            wv_ca_t = load_wv(wv_ca, 0, "wv_ca")
            wo_ca_t = load_w(wo_ca, 0, E, "wo_ca")

            ca_state = ca_kv(0, wk_ca_t, wv_ca_t)
            qsa = q_proj(wq_sa_t, ownfm)

            ag_out_prev = None
            for l in range(L):
                with nc.named_scope(f"L{l}"):
                    kca, vp_ca = ca_state
                    # ---- SA K/V from the gathered hidden state ----
                    ksa = [kfm_p.tile([128, S], FP16, tag="kfm", name="kfm")
                           for _ in range(ET)]
                    vp_sa = [vp_p.tile([128, 2 * H * HS],
                                       FP8 if FP8_AV else FP16,
                                       tag="vp", name="vp")
                             for _ in range(KT_SA // 2)]
                    for ch in range(4):
                        hch = hch_p.tile([128, ET * 512], FP16, tag="hch",
                                         name="hch")
                        for ei in range(ET):
                            if l == 0:
                                nc.sync.dma_start(
                                    hch[:, ei * 512:(ei + 1) * 512],
                                    sen_fm[ei * 128:(ei + 1) * 128,
                                           ch * 512:(ch + 1) * 512])
                            else:
                                nc.sync.dma_start(
                                    hch[:, ei * 512:(ei + 1) * 512],
                                    ag_out_prev[ch * 512 + ei * 128:
                                                ch * 512 + (ei + 1) * 128, :])
                        kv_chunk(ksa, ch * 512, hch, wk_sa_t)
                        v_chunk(vp_sa, ch * 4, hch, wv_sa_t)

                    # prefetch next layer K/V/Q weights (rings just freed)
                    if l < L - 1:
                        wk_sa_t = load_w(wk_sa, l + 1, E, "wk_sa")
                        wv_sa_t = load_wv(wv_sa, l + 1, "wv_sa")
                        wq_sa_t = load_w(wq_sa, l + 1, E, "wq_sa")
                        wk_ca_t = load_w(wk_ca, l + 1, E, "wk_ca")
                        wv_ca_t = load_wv(wv_ca, l + 1, "wv_ca")

                    # ---- SA attention + out-proj + LN1 ----
                    attn = [attn_p.tile([128, 512], FP16, tag="attn", name="attn")
                            for _ in range(ET)]
                    attention(qsa, ksa, vp_sa, KT_SA, attn)
                    inter = [stm_p.tile([128, E], FP16, tag="stm", name="stm")
                             for _ in range(TT)]
                    interfm = xfm_p.tile([128, ET * CH], FP16, tag="xfm",
                                         name="xfm")
                    out_ln(attn, wo_sa_t, hid, inter, interfm)
                    if DEBUG_DUMPS and l == 0:
                        nc.sync.dma_start(dbg["dbg_q"][:], qsa[0][:])
                        nc.sync.dma_start(dbg["dbg_k"][:], ksa[0][:])
                        nc.sync.dma_start(dbg["dbg_v"][:], vp_sa[0][:])
                        nc.sync.dma_start(dbg["dbg_attn"][:], attn[0][:])
                        nc.sync.dma_start(dbg["dbg_inter"][:], inter[0][:])
                    if l < L - 1:
                        wo_sa_t = load_w(wo_sa, l + 1, E, "wo_sa")

                    # ---- CA Q + attention + out-proj + LN2 ----
                    qca = q_proj(wq_ca_t, interfm)
                    if l < L - 1:
                        wq_ca_t = load_w(wq_ca, l + 1, E, "wq_ca")
                    attn2 = [attn_p.tile([128, 512], FP16, tag="attn", name="attn")
                             for _ in range(ET)]
                    attention(qca, kca, vp_ca, KT_CA, attn2)
                    co = [stm_p.tile([128, E], FP16, tag="stm", name="stm")
                          for _ in range(TT)]
                    cofm = xfm_p.tile([128, ET * CH], FP16, tag="xfm",
                                      name="xfm")
                    out_ln(attn2, wo_ca_t, inter, co, cofm)
                    if DEBUG_DUMPS and l == 0:
                        nc.sync.dma_start(dbg["dbg_co"][:], co[0][:])
                    if l < L - 1:
                        wo_ca_t = load_w(wo_ca, l + 1, E, "wo_ca")

                    # ---- FFN: software-pipelined h1 -> gelu -> h2 ----
                    w1_ts = []
                    for ei in range(ET):
                        wt = wbig_p.tile([128, F], FP16, tag="w1", name="w1",
                                         bufs=4)
                        nc.sync.dma_start(wt[:], w1_d[l, ei])
                        w1_ts.append(wt)
                    w2_t = wbig_p.tile([128, FT * E], FP8 if FP8_H2 else FP16,
                                       tag="w2", name="w2", bufs=1)
                    for ft in range(FT):
                        nc.sync.dma_start(w2_t[:, ft * E:(ft + 1) * E],
                                          w2_d[l, ft])
                    h2ps = [small_ps() for _ in range(TT)]
                    gts = {}
                    for fp in range(9):
                        if fp < 8:
                            sp2 = big_ps()
                            for j in range(2):
                                ft = fp * 2 + j
                                for ei in range(ET):
                                    nc.tensor.matmul(
                                        sp2[:, j * 512:(j + 1) * 512],
                                        w1_ts[ei][:, ft * 128:(ft + 1) * 128],
                                        cofm[:, ei * 512:(ei + 1) * 512],
                                        start=(ei == 0), stop=(ei == ET - 1))
                            gt = gel_p.tile([128, 1024],
                                            FP8 if FP8_H2 else FP16,
                                            tag="gel", name="gel")
                            gt4 = gt[:].rearrange(
                                "p (t two c) -> p t two c", t=TT, two=2)
                            for j in range(2):
                                nc.scalar.activation(
                                    gt4[:, :, j, :],
                                    sp2[:, j * 512:(j + 1) * 512]
                                    .rearrange("p (t c) -> p t c", t=TT),
                                    AF.Gelu)
                            gts[fp] = gt
                        if fp >= 1:
                            gt = gts.pop(fp - 1)
                            if FP8_H2:
                                w23 = w2_t[:, (fp - 1) * 2 * E:fp * 2 * E] \
                                    .rearrange("p (two c) -> p two c", two=2)
                                for t in range(TT):
                                    nc.tensor.matmul(
                                        h2ps[t][:],
                                        gt[:, t * 256:(t + 1) * 256]
                                        .rearrange("p (two c) -> p two c",
                                                   two=2),
                                        w23, start=(fp == 1), stop=False,
                                        perf_mode=PM.DoubleRow)
                            else:
                                for j in range(2):
                                    ft = (fp - 1) * 2 + j
                                    for t in range(TT):
                                        nc.tensor.matmul(
                                            h2ps[t][:],
                                            gt[:, t * 256 + j * 128:
                                               t * 256 + (j + 1) * 128],
                                            w2_t[:, ft * E:(ft + 1) * E],
                                            start=(ft == 0), stop=False)
                    for t in range(TT):
                        nc.tensor.matmul(h2ps[t][:],
                                         ident32t[:] if FP8_H2 else identt[:],
                                         co[t][:], start=False, stop=True)
                    if l == L - 1:
                        for t in range(TT):
                            o32 = out32_p.tile([128, E], FP32, tag="out32",
                                               name="out32")
                            ln_tile(h2ps[t][:], o32[:])
                            nc.sync.dma_start(out_d[t * 128:(t + 1) * 128, :],
                                              o32[:])
                    else:
                        hidn = [stm_p.tile([128, E], FP16, tag="stm", name="stm")
                                for _ in range(TT)]
                        ownfm_n = ofm_p.tile([128, ET * CH], FP16, tag="ofm",
                                             name="ofm")
                        for t in range(TT):
                            ln_tile(h2ps[t][:], hidn[t][:])
                            fm_from_tm(ownfm_n, hidn[t], t)
                        if DEBUG_DUMPS and l == 0:
                            nc.sync.dma_start(dbg["dbg_hid1"][:], hidn[0][:])
                        ag_in = dram_p.tile([CH, E], FP16, tag="agin", name="agin")
                        for e in range(ET):
                            nc.sync.dma_start(ag_in[e * 128:(e + 1) * 128, :],
                                              ownfm_n[:, e * CH:(e + 1) * CH])
                        ag_out = dram_p.tile([S, E], FP16, tag="agout",
                                             name="agout")
                        nc.gpsimd.collective_compute(
                            "AllGather", OP.bypass, replica_groups=GROUPS,
                            ins=[ag_in.opt()], outs=[ag_out.opt()])
                        # AG-independent work fills the collective latency
                        ca_state = ca_kv(l + 1, wk_ca_t, wv_ca_t)
                        qsa = q_proj(wq_sa_t, ownfm_n)
                        ag_out_prev = ag_out
                        hid = hidn

    nc.compile()
    return nc


def _prep_inputs_fast(sen, know, sa_qkv_w, sa_qkv_b, sa_out_w, sa_out_b,
                      ca_qkv_w, ca_qkv_b, ca_out_w, ca_out_b,
                      ff_w1, ff_b1, ff_w2, ff_b2, ln_g, ln_b):
    f16 = np.float16

    def rowtile(w):  # [L,E,cols] -> [L,ET,128,cols]
        return np.ascontiguousarray(w.reshape(L, ET, 128, -1).astype(f16))

    def padv(w):  # [L,E,E] -> [L,ET,128,H*HW], no bias/ones (mask adds ones)
        wp = np.zeros((L, E, H, HW), np.float32)
        wp[:, :, :, :D] = w.reshape(L, E, H, D)
        return np.ascontiguousarray(wp.reshape(L, ET, 128, H * HW).astype(f16))

    f8 = mybir.dt.np(FP8)
    common = {
        "ident": np.eye(128, dtype=f16),
        "ident32": (np.eye(128) * (W2_SCALE if FP8_H2 else 1.0)).astype(f16),
        "wq_sa": rowtile(sa_qkv_w[:, 0]), "wk_sa": rowtile(sa_qkv_w[:, 1]),
        "wv_sa": padv(sa_qkv_w[:, 2]),
        "wo_sa": rowtile(sa_out_w),
        "wq_ca": rowtile(ca_qkv_w[:, 0]), "wk_ca": rowtile(ca_qkv_w[:, 1]),
        "wv_ca": padv(ca_qkv_w[:, 2]),
        "wo_ca": rowtile(ca_out_w),
        "w1": rowtile(ff_w1),
        "w2": np.ascontiguousarray(
            (ff_w2 * W2_SCALE).reshape(L, FT, 128, E).astype(f8))
        if FP8_H2 else
        np.ascontiguousarray(ff_w2.reshape(L, FT, 128, E).astype(f16)),
    }
    in_maps = []
    for core in range(NCORES):
        g, c = core // 4, core % 4
        m = dict(common)
        m["sen_fm"] = np.ascontiguousarray(sen[g].T.astype(f16))
        m["own_fm0"] = np.ascontiguousarray(sen[g, c * CH:(c + 1) * CH].T.astype(f16))
        m["own_tm0"] = np.ascontiguousarray(sen[g, c * CH:(c + 1) * CH].astype(f16))
        m["know_fm"] = np.ascontiguousarray(know[g].T.astype(f16))
        in_maps.append(m)
    return in_maps


def _build_general():
    """Fallback for inputs with non-zero biases / non-unit LN gamma."""
    nc = bacc.Bacc("TRN2", target_bir_lowering=False, debug=False, num_devices=NCORES)

    def din(name, shape, dt=FP16):
        return nc.dram_tensor(name, shape, dt, kind="ExternalInput").ap()

    sen_fm = din("sen_fm", [E, S])
    own_fm0 = din("own_fm0", [E, CH])
    own_tm0 = din("own_tm0", [CH, E])
    know_fm_d = din("know_fm", [E, SK])
    ident_d = din("ident", [128, 128])
    ones_d = din("ones", [1, 128])

    wq_sa = din("wq_sa", [L, ET, ET, 128, 128])
    wk_sa = din("wk_sa", [L, ET, ET, 128, 128])
    wv_sa = din("wv_sa", [L, ET, 128, H * HW])
    wo_sa = din("wo_sa", [L, ET, 128, E])
    wq_ca = din("wq_ca", [L, ET, ET, 128, 128])
    wk_ca = din("wk_ca", [L, ET, ET, 128, 128])
    wv_ca = din("wv_ca", [L, ET, 128, H * HW])
    wo_ca = din("wo_ca", [L, ET, 128, E])
    w1_d = din("w1", [L, ET, FT, 128, 128])
    w2_d = din("w2", [L, FT, 128, E])

    bq_sa = din("bq_sa", [L, 128, ET], FP32)
    bk_sa = din("bk_sa", [L, 128, ET], FP32)
    bq_ca = din("bq_ca", [L, 128, ET], FP32)
    bk_ca = din("bk_ca", [L, 128, ET], FP32)
    b1_d = din("b1", [L, 128, FT], FP32)
    rbv_sa = din("rbv_sa", [L, 1, H * HW])
    rbo_sa = din("rbo_sa", [L, 1, E])
    rbv_ca = din("rbv_ca", [L, 1, H * HW])
    rbo_ca = din("rbo_ca", [L, 1, E])
    rb2_d = din("rb2", [L, 1, E])
    lng_d = din("lng", [L, 1, E], FP32)
    lnb_d = din("lnb", [L, 1, E], FP32)

    out_d = nc.dram_tensor("out_tm", [CH, E], FP32, kind="ExternalOutput").ap()

    with tile.TileContext(nc) as tc:
        from contextlib import ExitStack
        with ExitStack() as ctx:
            ep = ctx.enter_context
            const_p = ep(tc.tile_pool(name="const", bufs=1))
            know_p = ep(tc.tile_pool(name="know", bufs=4))
            kfm_p = ep(tc.tile_pool(name="kfm", bufs=4))
            kca_p = ep(tc.tile_pool(name="kca", bufs=4))
            vp_p = ep(tc.tile_pool(name="vp", bufs=27))
            hch_p = ep(tc.tile_pool(name="hch", bufs=6))
            qfm_p = ep(tc.tile_pool(name="qfm", bufs=8))
            attn_p = ep(tc.tile_pool(name="attn", bufs=4))
            ofm_p = ep(tc.tile_pool(name="ofm", bufs=8))
            xfm_p = ep(tc.tile_pool(name="xfm", bufs=5))
            stm_p = ep(tc.tile_pool(name="stm", bufs=8))
            out32_p = ep(tc.tile_pool(name="out32", bufs=2))
            pt_p = ep(tc.tile_pool(name="pt", bufs=6))
            gel_p = ep(tc.tile_pool(name="gel", bufs=17))
            wl_p = ep(tc.tile_pool(name="wl", bufs=16))
            wr_p = ep(tc.tile_pool(name="wr", bufs=6))
            row_p = ep(tc.tile_pool(name="row", bufs=4))
            gb_p = ep(tc.tile_pool(name="gb", bufs=2))
            sc_p = ep(tc.tile_pool(name="sc", bufs=3))
            s1_p = ep(tc.tile_pool(name="s1", bufs=2))
            st_p = ep(tc.tile_pool(name="st", bufs=8))
            ps_p = ep(tc.tile_pool(name="ps", bufs=8, space="PSUM"))
            dram_p = ep(tc.tile_pool(name="dram", bufs=2, space="DRAM"))

            identt = const_p.tile([128, 128], FP16, tag="ident", name="ident")
            nc.sync.dma_start(identt[:], ident_d[:])
            onest = const_p.tile([1, 128], FP16, tag="ones", name="ones")
            nc.sync.dma_start(onest[:], ones_d[:])
            knowfm = []
            for e in range(ET):
                t = know_p.tile([128, SK], FP16, tag="know", name="know")
                nc.sync.dma_start(t[:], know_fm_d[e * 128:(e + 1) * 128, :])
                knowfm.append(t)

            hid = []
            for t in range(TT):
                h = stm_p.tile([128, E], FP16, tag="stm", name="stm")
                nc.sync.dma_start(h[:], own_tm0[t * 128:(t + 1) * 128, :])
                hid.append(h)
            ownfm = []
            for e in range(ET):
                t = ofm_p.tile([128, CH], FP16, tag="ofm", name="ofm")
                nc.sync.dma_start(t[:], own_fm0[e * 128:(e + 1) * 128, :])
                ownfm.append(t)

            def ln_norm(xres, G, Bt, out):
                stt = st_p.tile([128, 6], FP32, tag="bnst", name="bnst")
                nc.vector.bn_stats(out=stt[:], in_=xres[:])
                mv = st_p.tile([128, 2], FP32, tag="bnmv", name="bnmv")
                nc.vector.bn_aggr(out=mv[:], in_=stt[:])
                sd = st_p.tile([128, 1], FP32, tag="sd", name="sd")
                nc.scalar.activation(sd[:], mv[:, 1:2], AF.Sqrt,
                                     scale=float(E) / (E - 1))
                nc.vector.tensor_scalar_add(sd[:], sd[:], 1e-6)
                inv = st_p.tile([128, 1], FP32, tag="inv", name="inv")
                nc.vector.reciprocal_approx_fast(inv[:], sd[:])
                minv = st_p.tile([128, 1], FP32, tag="minv", name="minv")
                nc.vector.tensor_mul(minv[:], mv[:, 0:1], inv[:])
                tmp = sc_p.tile([128, E], FP32, tag="lntmp", name="lntmp")
                nc.vector.tensor_scalar(tmp[:], in0=xres[:], scalar1=inv[:],
                                        scalar2=minv[:], op0=OP.mult,
                                        op1=OP.subtract)
                nc.vector.tensor_mul(tmp[:], tmp[:], G[:])
                nc.vector.tensor_add(out[:], tmp[:], Bt[:])

            def transpose_to(dst_tiles, src_tile, t):
                for e in range(ET):
                    tp = ps_p.tile([128, 128], FP16, tag="ps", name="ps")
                    nc.tensor.transpose(tp[:], src_tile[:, e * 128:(e + 1) * 128],
                                        identt[:])
                    nc.vector.tensor_copy(dst_tiles[e][:, t * 128:(t + 1) * 128],
                                          tp[:])

            def load_w16(wdram, l):
                ts = {}
                for ei in range(ET):
                    for e in range(ET):
                        wt = wl_p.tile([128, 128], FP16, tag="wl", name="wl")
                        nc.sync.dma_start(wt[:], wdram[l, ei, e])
                        ts[ei, e] = wt
                return ts

            def load_bias(bdram, l, n):
                bt = st_p.tile([128, n], FP32, tag="bias", name="bias", bufs=6)
                nc.sync.dma_start(bt[:], bdram[l])
                return bt

            def kv_proj(kdst, n_tok, src_tiles, src_col0, wk_tiles, bkt):
                nch = n_tok // 512
                for e in range(ET):
                    for c2 in range(nch):
                        pst = ps_p.tile([128, 512], FP32, tag="ps", name="ps")
                        for ei in range(ET):
                            nc.tensor.matmul(pst[:], wk_tiles[ei, e][:],
                                             src_tiles[ei][:, c2 * 512:(c2 + 1) * 512],
                                             start=(ei == 0), stop=(ei == ET - 1))
                        nc.vector.tensor_scalar_add(
                            kdst[e][:, src_col0 + c2 * 512:src_col0 + (c2 + 1) * 512],
                            pst[:], bkt[:, e:e + 1])

            def v_proj(vdst, kt0, nkt, src_tiles, wv_tiles, rbv):
                for ktl in range(nkt):
                    vt = vdst[kt0 + ktl]
                    for half in range(2):
                        pst = ps_p.tile([128, H * HW // 2], FP32, tag="ps",
                                        name="ps")
                        cs = half * (H * HW // 2)
                        for ei in range(ET):
                            nc.tensor.matmul(
                                pst[:], src_tiles[ei][:, ktl * 128:(ktl + 1) * 128],
                                wv_tiles[ei][:, cs:cs + H * HW // 2],
                                start=(ei == 0), stop=False)
                        nc.tensor.matmul(pst[:], onest[:],
                                         rbv[:, cs:cs + H * HW // 2],
                                         start=False, stop=True)
                        nc.vector.tensor_copy(vt[:, cs:cs + H * HW // 2], pst[:])

            def attention(qfm, kfm, vp, nkt, attn_tiles):
                for hs in range(2):
                    attps = [ps_p.tile([HW, 512], FP32, tag="ps", name="ps")
                             for _ in range(4)]
                    for kt in range(nkt):
                        for h4 in range(4):
                            h = hs * 4 + h4
                            e, r = h // 2, (h % 2) * 64
                            spt = ps_p.tile([128, 512], FP32, tag="ps", name="ps")
                            nc.tensor.matmul(
                                spt[:], kfm[e][r:r + 64, kt * 128:(kt + 1) * 128],
                                qfm[e][r:r + 64, :], start=True, stop=True)
                            pt = pt_p.tile([128, 512], FP16, tag="pt", name="pt")
                            nc.scalar.activation(pt[:], spt[:], AF.Exp, scale=0.125)
                            nc.tensor.matmul(attps[h4][:],
                                             vp[kt][:, h * HW:(h + 1) * HW],
                                             pt[:], start=(kt == 0),
                                             stop=(kt == nkt - 1))
                    for h4 in range(4):
                        h = hs * 4 + h4
                        e, r = h // 2, (h % 2) * 64
                        ats = sc_p.tile([64, 512], FP32, tag="ats", name="ats",
                                        bufs=4)
                        nc.scalar.activation(ats[:], attps[h4][0:64, :], AF.Copy)
                        den = s1_p.tile([1, 512], FP32, tag="den", name="den")
                        nc.vector.tensor_copy(den[:], attps[h4][64:65, :])
                        rec = s1_p.tile([1, 512], FP32, tag="rec", name="rec")
                        nc.vector.reciprocal_approx_fast(rec[:], den[:])
                        rb = sc_p.tile([64, 512], FP32, tag="rb", name="rb")
                        nc.gpsimd.partition_broadcast(rb[:], rec[:])
                        nc.vector.tensor_mul(attn_tiles[e][r:r + 64, :],
                                             ats[:], rb[:])

            def out_proj_ln(attn_tiles, wo_tiles, rbo, res_tiles, G, Bt, out_tiles):
                for t in range(TT):
                    pst = ps_p.tile([128, E], FP32, tag="ps", name="ps")
                    for ei in range(ET):
                        nc.tensor.matmul(pst[:],
                                         attn_tiles[ei][:, t * 128:(t + 1) * 128],
                                         wo_tiles[ei][:], start=(ei == 0),
                                         stop=False)
                    nc.tensor.matmul(pst[:], onest[:], rbo[:], start=False,
                                     stop=True)
                    xres = sc_p.tile([128, E], FP32, tag="xres", name="xres")
                    nc.vector.tensor_add(xres[:], pst[:], res_tiles[t][:])
                    ln_norm(xres, G, Bt, out_tiles[t])

            def make_ca_kv(l):
                kca = [kca_p.tile([128, SK], FP16, tag="kca", name="kca")
                       for _ in range(ET)]
                wkt_ca = load_w16(wk_ca, l)
                bkt_ca = load_bias(bk_ca, l, ET)
                kv_proj(kca, SK, knowfm, 0, wkt_ca, bkt_ca)
                vp_ca = [vp_p.tile([128, H * HW], FP16, tag="vp", name="vp")
                         for _ in range(KT_CA)]
                wvt_ca = []
                for ei in range(ET):
                    wt = wr_p.tile([128, H * HW], FP16, tag="wr", name="wr")
                    nc.sync.dma_start(wt[:], wv_ca[l, ei])
                    wvt_ca.append(wt)
                rbv = row_p.tile([1, H * HW], FP16, tag="row", name="row")
                nc.sync.dma_start(rbv[:], rbv_ca[l])
                v_proj(vp_ca, 0, KT_CA, knowfm, wvt_ca, rbv)
                return kca, vp_ca

            ag_out_prev = None
            ca_kv_next = None
            for l in range(L):
                with nc.named_scope(f"L{l}"):
                    if l == 0:
                        kca, vp_ca = make_ca_kv(0)
                    else:
                        kca, vp_ca = ca_kv_next
                    lr = s1_p.tile([1, E], FP32, tag="lnrow", name="lnrow")
                    nc.sync.dma_start(lr[:], lng_d[l])
                    G = gb_p.tile([128, E], FP32, tag="G", name="G")
                    nc.gpsimd.partition_broadcast(G[:], lr[:])
                    lr2 = s1_p.tile([1, E], FP32, tag="lnrow", name="lnrow")
                    nc.sync.dma_start(lr2[:], lnb_d[l])
                    Bt = gb_p.tile([128, E], FP32, tag="B", name="B")
                    nc.gpsimd.partition_broadcast(Bt[:], lr2[:])

                    ksa = [kfm_p.tile([128, S], FP16, tag="kfm", name="kfm")
                           for _ in range(ET)]
                    vp_sa = [vp_p.tile([128, H * HW], FP16, tag="vp", name="vp")
                             for _ in range(KT_SA)]
                    wkt_sa = load_w16(wk_sa, l)
                    wvt_sa = []
                    for ei in range(ET):
                        wt = wr_p.tile([128, H * HW], FP16, tag="wr", name="wr")
                        nc.sync.dma_start(wt[:], wv_sa[l, ei])
                        wvt_sa.append(wt)
                    rbvs = row_p.tile([1, H * HW], FP16, tag="row", name="row")
                    nc.sync.dma_start(rbvs[:], rbv_sa[l])
                    bkt_sa = load_bias(bk_sa, l, ET)
                    for ch in range(4):
                        hch = []
                        for ei in range(ET):
                            ht = hch_p.tile([128, 512], FP16, tag="hch", name="hch")
                            if l == 0:
                                nc.sync.dma_start(
                                    ht[:], sen_fm[ei * 128:(ei + 1) * 128,
                                                  ch * 512:(ch + 1) * 512])
                            else:
                                nc.sync.dma_start(
                                    ht[:], ag_out_prev[ch * 512 + ei * 128:
                                                       ch * 512 + (ei + 1) * 128, :])
                            hch.append(ht)
                        kv_proj(ksa, 512, hch, ch * 512, wkt_sa, bkt_sa)
                        v_proj(vp_sa, ch * 4, 4, hch, wvt_sa, rbvs)

                    if l == 0:
                        qsa = [qfm_p.tile([128, 512], FP16, tag="qfm", name="qfm")
                               for _ in range(ET)]
                        wqt_sa = load_w16(wq_sa, l)
                        bqt = load_bias(bq_sa, l, ET)
                        for e in range(ET):
                            pst = ps_p.tile([128, 512], FP32, tag="ps", name="ps")
                            for ei in range(ET):
                                nc.tensor.matmul(pst[:], wqt_sa[ei, e][:],
                                                 ownfm[ei][:],
                                                 start=(ei == 0),
                                                 stop=(ei == ET - 1))
                            nc.vector.tensor_scalar_add(qsa[e][:], pst[:],
                                                        bqt[:, e:e + 1])
                    else:
                        qsa = qsa_next

                    attn = [attn_p.tile([128, 512], FP16, tag="attn", name="attn")
                            for _ in range(ET)]
                    attention(qsa, ksa, vp_sa, KT_SA, attn)
                    wot = []
                    for ei in range(ET):
                        wt = wr_p.tile([128, E], FP16, tag="wr", name="wr")
                        nc.sync.dma_start(wt[:], wo_sa[l, ei])
                        wot.append(wt)
                    rbo = row_p.tile([1, E], FP16, tag="row", name="row")
                    nc.sync.dma_start(rbo[:], rbo_sa[l])
                    inter = [stm_p.tile([128, E], FP16, tag="stm", name="stm")
                             for _ in range(TT)]
                    out_proj_ln(attn, wot, rbo, hid, G, Bt, inter)

                    interfm = [xfm_p.tile([128, CH], FP16, tag="xfm", name="xfm")
                               for _ in range(ET)]
                    for t in range(TT):
                        transpose_to(interfm, inter[t], t)

                    qca = [qfm_p.tile([128, 512], FP16, tag="qfm", name="qfm")
                           for _ in range(ET)]
                    wqt_ca = load_w16(wq_ca, l)
                    bqt_ca = load_bias(bq_ca, l, ET)
                    for e in range(ET):
                        pst = ps_p.tile([128, 512], FP32, tag="ps", name="ps")
                        for ei in range(ET):
                            nc.tensor.matmul(pst[:], wqt_ca[ei, e][:],
                                             interfm[ei][:],
                                             start=(ei == 0), stop=(ei == ET - 1))
                        nc.vector.tensor_scalar_add(qca[e][:], pst[:],
                                                    bqt_ca[:, e:e + 1])

                    attn2 = [attn_p.tile([128, 512], FP16, tag="attn", name="attn")
                             for _ in range(ET)]
                    attention(qca, kca, vp_ca, KT_CA, attn2)
                    wot2 = []
                    for ei in range(ET):
                        wt = wr_p.tile([128, E], FP16, tag="wr", name="wr")
                        nc.sync.dma_start(wt[:], wo_ca[l, ei])
                        wot2.append(wt)
                    rbo2 = row_p.tile([1, E], FP16, tag="row", name="row")
                    nc.sync.dma_start(rbo2[:], rbo_ca[l])
                    co = [stm_p.tile([128, E], FP16, tag="stm", name="stm")
                          for _ in range(TT)]
                    out_proj_ln(attn2, wot2, rbo2, inter, G, Bt, co)

                    cofm = [xfm_p.tile([128, CH], FP16, tag="xfm", name="xfm")
                            for _ in range(ET)]
                    for t in range(TT):
                        transpose_to(cofm, co[t], t)

                    rb2 = row_p.tile([1, E], FP16, tag="row", name="row")
                    nc.sync.dma_start(rb2[:], rb2_d[l])
                    b1t = load_bias(b1_d, l, FT)
                    gel = []
                    for ft in range(FT):
                        pst = ps_p.tile([128, 512], FP32, tag="ps", name="ps")
                        for ei in range(ET):
                            wt = wl_p.tile([128, 128], FP16, tag="wl", name="wl")
                            nc.sync.dma_start(wt[:], w1_d[l, ei, ft])
                            nc.tensor.matmul(pst[:], wt[:], cofm[ei][:],
                                             start=(ei == 0), stop=(ei == ET - 1))
                        gt = gel_p.tile([128, 512], FP16, tag="gel", name="gel")
                        nc.scalar.activation(gt[:], pst[:], AF.Gelu,
                                             bias=b1t[:, ft:ft + 1])
                        gel.append(gt)
                    w2ts = []
                    for ft in range(FT):
                        w2t = wr_p.tile([128, E], FP16, tag="w2r", name="w2r",
                                        bufs=17)
                        nc.sync.dma_start(w2t[:], w2_d[l, ft])
                        w2ts.append(w2t)
                    h2ps = [ps_p.tile([128, E], FP32, tag="ps", name="ps")
                            for _ in range(TT)]
                    for t in range(TT):
                        for ft in range(FT):
                            nc.tensor.matmul(h2ps[t][:],
                                             gel[ft][:, t * 128:(t + 1) * 128],
                                             w2ts[ft][:], start=(ft == 0),
                                             stop=False)
                    if l == L - 1:
                        hidn = [out32_p.tile([128, E], FP32, tag="out32",
                                             name="out32") for _ in range(TT)]
                    else:
                        hidn = [stm_p.tile([128, E], FP16, tag="stm", name="stm")
                                for _ in range(TT)]
                    for t in range(TT):
                        nc.tensor.matmul(h2ps[t][:], onest[:], rb2[:],
                                         start=False, stop=True)
                        xres = sc_p.tile([128, E], FP32, tag="xres", name="xres")
                        nc.vector.tensor_add(xres[:], h2ps[t][:], co[t][:])
                        ln_norm(xres, G, Bt, hidn[t])
                        if l == L - 1:
                            nc.sync.dma_start(out_d[t * 128:(t + 1) * 128, :],
                                              hidn[t][:])

                    if l < L - 1:
                        ownfm_n = [ofm_p.tile([128, CH], FP16, tag="ofm",
                                              name="ofm") for _ in range(ET)]
                        for t in range(TT):
                            transpose_to(ownfm_n, hidn[t], t)
                        ag_in = dram_p.tile([CH, E], FP16, tag="agin", name="agin")
                        for e in range(ET):
                            nc.sync.dma_start(ag_in[e * 128:(e + 1) * 128, :],
                                              ownfm_n[e][:])
                        ag_out = dram_p.tile([S, E], FP16, tag="agout",
                                             name="agout")
                        nc.gpsimd.collective_compute(
                            "AllGather", OP.bypass, replica_groups=GROUPS,
                            ins=[ag_in.opt()], outs=[ag_out.opt()])
                        ca_kv_next = make_ca_kv(l + 1)
                        qsa_next = [qfm_p.tile([128, 512], FP16, tag="qfm",
                                               name="qfm") for _ in range(ET)]
                        wqt_n = load_w16(wq_sa, l + 1)
                        bqt_n = load_bias(bq_sa, l + 1, ET)
                        for e in range(ET):
                            pst = ps_p.tile([128, 512], FP32, tag="ps", name="ps")
                            for ei in range(ET):
                                nc.tensor.matmul(pst[:], wqt_n[ei, e][:],
                                                 ownfm_n[ei][:],
                                                 start=(ei == 0),
                                                 stop=(ei == ET - 1))
                            nc.vector.tensor_scalar_add(qsa_next[e][:], pst[:],
                                                        bqt_n[:, e:e + 1])
                        ag_out_prev = ag_out
                        ownfm = ownfm_n
                        hid = hidn

    nc.compile()
    return nc


def _prep_inputs(sen, know, sa_qkv_w, sa_qkv_b, sa_out_w, sa_out_b,
                 ca_qkv_w, ca_qkv_b, ca_out_w, ca_out_b,
                 ff_w1, ff_b1, ff_w2, ff_b2, ln_g, ln_b):
    """Host-side weight packing for the general fallback kernel."""
    f16, f32 = np.float16, np.float32

    def tile4(w):
        return np.ascontiguousarray(
            w.reshape(L, ET, 128, ET, 128).transpose(0, 1, 3, 2, 4).astype(f16))

    def padv(w, b):
        wp = np.zeros((L, E, H, HW), f32)
        wp[:, :, :, :D] = w.reshape(L, E, H, D)
        bp = np.zeros((L, H, HW), f32)
        bp[:, :, :D] = b.reshape(L, H, D)
        bp[:, :, D] = 1.0
        return (np.ascontiguousarray(wp.reshape(L, ET, 128, H * HW).astype(f16)),
                np.ascontiguousarray(bp.reshape(L, 1, H * HW).astype(f16)))

    wv_sa_p, rbv_sa_h = padv(sa_qkv_w[:, 2], sa_qkv_b[:, 2])
    wv_ca_p, rbv_ca_h = padv(ca_qkv_w[:, 2], ca_qkv_b[:, 2])

    common = {
        "ident": np.eye(128, dtype=f16),
        "ones": np.ones((1, 128), f16),
        "wq_sa": tile4(sa_qkv_w[:, 0]), "wk_sa": tile4(sa_qkv_w[:, 1]),
        "wv_sa": wv_sa_p,
        "wo_sa": np.ascontiguousarray(sa_out_w.reshape(L, ET, 128, E).astype(f16)),
        "wq_ca": tile4(ca_qkv_w[:, 0]), "wk_ca": tile4(ca_qkv_w[:, 1]),
        "wv_ca": wv_ca_p,
        "wo_ca": np.ascontiguousarray(ca_out_w.reshape(L, ET, 128, E).astype(f16)),
        "w1": np.ascontiguousarray(
            ff_w1.reshape(L, ET, 128, FT, 128).transpose(0, 1, 3, 2, 4).astype(f16)),
        "w2": np.ascontiguousarray(ff_w2.reshape(L, FT, 128, E).astype(f16)),
        "bq_sa": np.ascontiguousarray(
            sa_qkv_b[:, 0].reshape(L, ET, 128).transpose(0, 2, 1)),
        "bk_sa": np.ascontiguousarray(
            sa_qkv_b[:, 1].reshape(L, ET, 128).transpose(0, 2, 1)),
        "bq_ca": np.ascontiguousarray(
            ca_qkv_b[:, 0].reshape(L, ET, 128).transpose(0, 2, 1)),
        "bk_ca": np.ascontiguousarray(
            ca_qkv_b[:, 1].reshape(L, ET, 128).transpose(0, 2, 1)),
        "b1": np.ascontiguousarray(
            ff_b1.reshape(L, FT, 128).transpose(0, 2, 1)),
        "rbv_sa": rbv_sa_h, "rbv_ca": rbv_ca_h,
        "rbo_sa": np.ascontiguousarray(sa_out_b[:, None, :].astype(f16)),
        "rbo_ca": np.ascontiguousarray(ca_out_b[:, None, :].astype(f16)),
        "rb2": np.ascontiguousarray(ff_b2[:, None, :].astype(f16)),
        "lng": np.ascontiguousarray(ln_g[:, None, :]),
        "lnb": np.ascontiguousarray(ln_b[:, None, :]),
    }
    in_maps = []
    for core in range(NCORES):
        g, c = core // 4, core % 4
        m = dict(common)
        m["sen_fm"] = np.ascontiguousarray(sen[g].T.astype(f16))
        m["own_fm0"] = np.ascontiguousarray(sen[g, c * CH:(c + 1) * CH].T.astype(f16))
        m["own_tm0"] = np.ascontiguousarray(sen[g, c * CH:(c + 1) * CH].astype(f16))
        m["know_fm"] = np.ascontiguousarray(know[g].T.astype(f16))
        in_maps.append(m)
    return in_maps


def _inputs_are_fast(sa_qkv_b, sa_out_b, ca_qkv_b, ca_out_b,
                     ff_b1, ff_b2, ln_g, ln_b, **_):
    zeros = [sa_qkv_b, sa_out_b, ca_qkv_b, ca_out_b, ff_b1, ff_b2, ln_b]
    return all(not np.any(z) for z in zeros) and np.all(ln_g == 1.0)


def kernel(**inputs):
    inputs = {k: np.asarray(v, dtype=np.float32) for k, v in inputs.items()}
    if _inputs_are_fast(**inputs):
        if "nc" not in _CACHE:
            _CACHE["nc"] = _build_fast()
        nc = _CACHE["nc"]
        in_maps = _prep_inputs_fast(**inputs)
    else:
        if "nc_gen" not in _CACHE:
            _CACHE["nc_gen"] = _build_general()
        nc = _CACHE["nc_gen"]
        in_maps = _prep_inputs(**inputs)
        _CACHE["nc"] = nc
    res = run_bass_kernel_spmd(nc, in_maps, list(range(NCORES)))
    out = np.empty((B, S, E), np.float32)
    for core in range(NCORES):
        g, c = core // 4, core % 4
        out[g, c * CH:(c + 1) * CH] = res.results[core]["out_tm"]
    return out


# revision 55
# speedup vs baseline: 1.0243x; 1.0092x over previous
"""Trainium2 Bass kernel for a 4-layer hierarchical-attention encoder.

Sharding: 8 cores = 2 batch groups x 4 sequence chunks of 512 query tokens.
Each core runs the full layer stack for its 512 tokens; the hidden state is
all-gathered (per batch group) at each layer boundary so every core can
compute full-sequence self-attention K/V locally.

Fast path (inputs with zero biases, unit LN gamma, zero LN beta — which is
what setup_inputs() produces):
 - no bias matmul rows; K-bias is dropped (exactly free under softmax)
 - residual adds ride on the PE via identity matmuls into the out-proj PSUM
 - LN sqrt computed as exp(0.5*ln(v)) so the Act engine never leaves the
   exp table on the critical path
 - exp/gelu processed on 2-bank (1024-wide) PSUM regions
 - all TM->FM transposes on the DMA transpose engine (PE/DVE freed)
 - attention denominator via a ones-mask added during the V' PSUM drain
 - batched weight DMA layouts ([128, E]-row tiles)
A general fallback (the previous kernel) handles arbitrary bias/gamma.
"""
import os
import sys

for _p in ("/root/.axon_site/_ro/trn_rl_repo", "/opt/trn_rl_repo", "/opt/pypackages",
           "/root/.axon_site/_ro/pypackages"):
    if os.path.isdir(_p) and _p not in sys.path:
        sys.path.append(_p)

import numpy as np

import concourse.bass as bass
import concourse.mybir as mybir
import concourse.tile as tile
from concourse import bacc
from concourse.bass_utils import run_bass_kernel_spmd

L, E, H, D, F = 4, 512, 8, 64, 2048
B, S, SK = 2, 2048, 1024
NCORES = 8
GROUPS = [[0, 1, 2, 3], [4, 5, 6, 7]]
CH = 512          # tokens per core
ET = E // 128     # 4 feature tiles
TT = CH // 128    # 4 token tiles in own chunk
FT = F // 128     # 16 ffn tiles
KT_SA = S // 128  # 16 key tiles (self)
KT_CA = SK // 128  # 8 key tiles (cross)
HW = 65           # head width incl. denominator column
HWP = 80          # fp8 DoubleRow head stride (M%16==0 requirement)

FP32 = mybir.dt.float32
FP16 = mybir.dt.float16
FP8 = mybir.dt.float8e4
AF = mybir.ActivationFunctionType
OP = mybir.AluOpType
PM = mybir.MatmulPerfMode
W2_SCALE = 32.0  # fp8 w2 pre-scale; exact under LN's scale invariance

_CACHE = {}

# Feature toggles for HW bring-up bisection.
# EXP_2BANK stays off: a single Act instruction reading a PSUM access
# pattern that crosses a bank boundary wedges the exec unit on HW.
USE_DMA_TRANSPOSE = os.environ.get("K_DMA_T", "1") == "1"
EXP_2BANK = os.environ.get("K_EXP2", "0") == "1"
DEBUG_DUMPS = os.environ.get("K_DEBUG", "0") == "1"
FP8_AV = os.environ.get("K_FP8AV", "0") == "1"
FP8_H2 = os.environ.get("K_FP8H2", "0") == "1"
HS = HWP if FP8_AV else HW  # per-(head,pair-slot) stride in V tiles


def _patch_act_tables():
    """Steer the act-table-load pass away from the exp-less `natural_log`
    table so Ln resolves to `natural_log_exp_and_others` and the LN
    ln/exp pair never swaps tables against the attention exps.

    Only the bass-side selector sees the emptied entry; table ids and the
    hardware table contents (walrus reads act_info.json directly) are
    unchanged, so every emitted load remains valid.
    """
    import concourse.bacc as bacc_mod
    if getattr(bacc_mod, "_ln_exp_patched", False):
        return
    orig = bacc_mod.get_activation_tables

    def patched(arch):
        tables = dict(orig(arch))
        shared = "natural_log_exp_and_others"
        if shared in tables and {mybir.ActivationFunctionType.Exp,
                                 mybir.ActivationFunctionType.Ln} <= tables[shared]:
            for name, fns in tables.items():
                if name != shared:
                    tables[name] = fns - {mybir.ActivationFunctionType.Exp,
                                          mybir.ActivationFunctionType.Ln}
        return tables

    bacc_mod.get_activation_tables = patched
    bacc_mod._ln_exp_patched = True


def _build_fast():
    _patch_act_tables()
    nc = bacc.Bacc("TRN2", target_bir_lowering=False, debug=False, num_devices=NCORES)

    def din(name, shape, dt=FP16):
        return nc.dram_tensor(name, shape, dt, kind="ExternalInput").ap()

    sen_fm = din("sen_fm", [E, S])            # full batch sequence, feature-major
    own_fm0 = din("own_fm0", [E, CH])         # own chunk, feature-major
    own_tm0 = din("own_tm0", [CH, E])         # own chunk, token-major
    know_fm_d = din("know_fm", [E, SK])
    ident_d = din("ident", [128, 128])
    ident32_d = din("ident32", [128, 128])

    wq_sa = din("wq_sa", [L, ET, 128, E])
    wk_sa = din("wk_sa", [L, ET, 128, E])
    wv_sa = din("wv_sa", [L, ET, 128, H * HW])
    wo_sa = din("wo_sa", [L, ET, 128, E])
    wq_ca = din("wq_ca", [L, ET, 128, E])
    wk_ca = din("wk_ca", [L, ET, 128, E])
    wv_ca = din("wv_ca", [L, ET, 128, H * HW])
    wo_ca = din("wo_ca", [L, ET, 128, E])
    w1_d = din("w1", [L, ET, 128, F])
    w2_d = din("w2", [L, FT, 128, E], FP8 if FP8_H2 else FP16)

    out_d = nc.dram_tensor("out_tm", [CH, E], FP32, kind="ExternalOutput").ap()
    dbg = {}
    if DEBUG_DUMPS:
        for nm, shape in [("dbg_q", [128, 512]), ("dbg_k", [128, S]),
                          ("dbg_v", [128, H * HW]), ("dbg_attn", [128, 512]),
                          ("dbg_inter", [128, E]), ("dbg_co", [128, E]),
                          ("dbg_hid1", [128, E])]:
            dbg[nm] = nc.dram_tensor(nm, shape, FP16,
                                     kind="ExternalOutput").ap()

    HH = H * HW // 2  # 260, half of the padded V width

    with tile.TileContext(nc) as tc:
        from contextlib import ExitStack
        with ExitStack() as ctx:
            ep = ctx.enter_context
            const_p = ep(tc.tile_pool(name="const", bufs=1))
            know_p = ep(tc.tile_pool(name="know", bufs=1))    # [128,4096] know FM
            kfm_p = ep(tc.tile_pool(name="kfm", bufs=4))      # [128,2048] SA K fp16
            kca_p = ep(tc.tile_pool(name="kca", bufs=4))      # [128,1024] CA K fp16
            vp_p = ep(tc.tile_pool(name="vp", bufs=12))       # resident V' pairs
            hch_p = ep(tc.tile_pool(name="hch", bufs=2))      # [128,2048] H_fm chunk
            qfm_p = ep(tc.tile_pool(name="qfm", bufs=8))
            attn_p = ep(tc.tile_pool(name="attn", bufs=8))
            pt_p = ep(tc.tile_pool(name="pt", bufs=4))        # exp out [128,1024] fp16
            gel_p = ep(tc.tile_pool(name="gel", bufs=3))      # [128,1024] fp16
            stm_p = ep(tc.tile_pool(name="stm", bufs=12))     # hid/inter/co TM fp16
            xfm_p = ep(tc.tile_pool(name="xfm", bufs=2))      # inter_fm / co_fm
            ofm_p = ep(tc.tile_pool(name="ofm", bufs=2))      # own_fm
            out32_p = ep(tc.tile_pool(name="out32", bufs=2))  # final layer fp32 out
            wbig_p = ep(tc.tile_pool(name="wbig", bufs=1))    # [128,2048] weights
            wsm_p = ep(tc.tile_pool(name="wsm", bufs=1))      # [128,520] wv weights
            st_p = ep(tc.tile_pool(name="st", bufs=8))        # small stats
            rr_p = ep(tc.tile_pool(name="rr", bufs=4))        # recip rows / bcast
            ps_p = ep(tc.tile_pool(name="ps", bufs=8, space="PSUM"))
            dram_p = ep(tc.tile_pool(name="dram", bufs=2, space="DRAM"))

            def big_ps():
                return ps_p.tile([128, 1024], FP32, tag="big", name="big",
                                 bufs=2 if USE_DMA_TRANSPOSE else 1)

            def small_ps():
                return ps_p.tile([128, 512], FP32, tag="small", name="small", bufs=4)

            def fm_from_tm(out_fm, tm_tile, t):
                """FM[:, e*512 + t*128 + b] = TM[b, e*128 + p]: one batched
                DMA transpose per TM tile (out is a 3D strided AP whose
                (partition, e) dims cover the 512 transposed rows)."""
                if USE_DMA_TRANSPOSE:
                    out3 = out_fm[:].rearrange("p (e c) -> p e c", e=ET)
                    nc.sync.dma_start_transpose(
                        out3[:, :, t * 128:(t + 1) * 128], tm_tile[:])
                else:
                    for e in range(ET):
                        tp = ps_p.tile([128, 128], FP16, tag="tp", name="tp",
                                       bufs=2)
                        nc.tensor.transpose(tp[:], tm_tile[:, e * 128:(e + 1) * 128],
                                            identt[:])
                        nc.vector.tensor_copy(
                            out_fm[:, e * 512 + t * 128:e * 512 + (t + 1) * 128],
                            tp[:])

            identt = const_p.tile([128, 128], FP16, tag="ident", name="ident")
            nc.sync.dma_start(identt[:], ident_d[:])
            ident32t = const_p.tile([128, 128], FP16, tag="ident32",
                                    name="ident32")
            nc.sync.dma_start(ident32t[:], ident32_d[:])
            # ones-mask for the V' drain: 1.0 at each head's denominator
            # column (rel. cols 64,129,194,259 in each 260-wide half)
            vmask = const_p.tile([128, HH], FP16, tag="vmask", name="vmask")
            nc.vector.memset(vmask[:], 0.0)
            for hh in range(4):
                nc.vector.memset(vmask[:, hh * HW + D:hh * HW + D + 1], 1.0)

            knowfm = know_p.tile([128, ET * SK], FP16, tag="know", name="know")
            for e in range(ET):
                nc.sync.dma_start(knowfm[:, e * SK:(e + 1) * SK],
                                  know_fm_d[e * 128:(e + 1) * 128, :])

            hid = []
            for t in range(TT):
                h = stm_p.tile([128, E], FP16, tag="stm", name="stm")
                nc.sync.dma_start(h[:], own_tm0[t * 128:(t + 1) * 128, :])
                hid.append(h)
            ownfm = ofm_p.tile([128, ET * CH], FP16, tag="ofm", name="ofm")
            for e in range(ET):
                nc.sync.dma_start(ownfm[:, e * CH:(e + 1) * CH],
                                  own_fm0[e * 128:(e + 1) * 128, :])

            def load_w(dram, l, cols, tag, bufs=1):
                """One [128, ET*cols] tile; slice (ei, c) = [:, ei*cols+c]."""
                wt = wbig_p.tile([128, ET * cols], FP16, tag=tag, name=tag, bufs=bufs)
                for ei in range(ET):
                    nc.sync.dma_start(wt[:, ei * cols:(ei + 1) * cols], dram[l, ei])
                return wt

            def load_wv(dram, l, tag):
                """Four [128, H*HW] tiles, one per input-feature block ei."""
                wts = []
                for ei in range(ET):
                    wt = wsm_p.tile([128, H * HW], FP16, tag=tag, name=tag, bufs=4)
                    nc.sync.dma_start(wt[:], dram[l, ei])
                    wts.append(wt)
                return wts

            def q_proj(wq_t, src_fm):
                """Q_fm tiles [128, 512] from a single [128, ET*512] FM tile."""
                qs = []
                for e in range(ET):
                    ps = small_ps()
                    for ei in range(ET):
                        nc.tensor.matmul(
                            ps[:],
                            wq_t[:, ei * E + e * 128:ei * E + (e + 1) * 128],
                            src_fm[:, ei * 512:(ei + 1) * 512],
                            start=(ei == 0), stop=(ei == ET - 1))
                    qt = qfm_p.tile([128, 512], FP16, tag="qfm", name="qfm")
                    nc.vector.tensor_copy(qt[:], ps[:])
                    qs.append(qt)
                return qs

            def kv_chunk(kdst, col0, hch, wk_t):
                """K_fm columns [col0:col0+512) from one FM chunk tile."""
                bps = [big_ps(), big_ps()]
                for e in range(ET):
                    ps = bps[e // 2][:, (e % 2) * 512:(e % 2 + 1) * 512]
                    for ei in range(ET):
                        nc.tensor.matmul(
                            ps, wk_t[:, ei * E + e * 128:ei * E + (e + 1) * 128],
                            hch[:, ei * 512:(ei + 1) * 512],
                            start=(ei == 0), stop=(ei == ET - 1))
                    nc.vector.tensor_copy(kdst[e][:, col0:col0 + 512], ps)

            def v_chunk(vdst, kt0, hch, wv_ts):
                """V' token tiles kt0..kt0+3 into kt-pair tiles [*, 2*H*HW]."""
                for ktl in range(4):
                    psA = small_ps()
                    psB = small_ps()
                    for ei in range(ET):
                        lhs = hch[:, ei * 512 + ktl * 128:ei * 512 + (ktl + 1) * 128]
                        nc.tensor.matmul(psA[:, 0:HH], lhs, wv_ts[ei][:, 0:HH],
                                         start=(ei == 0), stop=(ei == ET - 1))
                        nc.tensor.matmul(psB[:, 0:HH], lhs, wv_ts[ei][:, HH:2 * HH],
                                         start=(ei == 0), stop=(ei == ET - 1))
                    kt = kt0 + ktl
                    vt = vdst[kt // 2]
                    j = kt % 2
                    vt4 = vt[:].rearrange("p (h two c) -> p h two c", h=H, two=2)
                    if j == 0 and FP8_AV:
                        nc.vector.memset(vt4[:, :, :, HW:HWP], 0.0)
                    m3 = vmask[:].rearrange("p (h c) -> p h c", h=4)
                    for half, psX in ((0, psA), (1, psB)):
                        p3 = psX[:, 0:HH].rearrange("p (h c) -> p h c", h=4)
                        nc.vector.tensor_add(
                            vt4[:, half * 4:half * 4 + 4, j, 0:HW], p3, m3)

            def attention(qfm, kfm, vp, nkt, attn_tiles):
                """vp: with FP8_AV, kt-PAIR tiles [128, 2*H*HW] fp8 (one per
                2 key tiles); AV runs one fp8 DoubleRow matmul per pair.
                Without FP8_AV, per-kt fp16 tiles as before."""
                nkp = nkt // 2
                for hs in range(2):
                    attps = [small_ps() for _ in range(4)]
                    pts = {}
                    for kp in range(nkp + 1):
                        for h4 in range(4):
                            h = hs * 4 + h4
                            e, r = h // 2, (h % 2) * 64
                            if kp < nkp:
                                sp2 = big_ps()
                                for j in range(2):
                                    kt = kp * 2 + j
                                    nc.tensor.matmul(
                                        sp2[:, j * 512:(j + 1) * 512],
                                        kfm[e][r:r + 64, kt * 128:(kt + 1) * 128],
                                        qfm[e][r:r + 64, :],
                                        start=True, stop=True)
                                pt = pt_p.tile([128, 1024],
                                               FP8 if FP8_AV else FP16,
                                               tag="pt", name="pt",
                                               bufs=8 if FP8_AV else 4)
                                for j in range(2):
                                    nc.scalar.activation(
                                        pt[:, j * 512:(j + 1) * 512],
                                        sp2[:, j * 512:(j + 1) * 512],
                                        AF.Exp, scale=0.125)
                                pts[kp, h4] = pt
                            if kp >= 1:
                                pt = pts.pop((kp - 1, h4))
                                if FP8_AV:
                                    lhs3 = vp[kp - 1][:, h * 2 * HS:
                                                      (h + 1) * 2 * HS] \
                                        .rearrange("p (two c) -> p two c",
                                                   two=2)
                                    rhs3 = pt[:].rearrange(
                                        "p (two c) -> p two c", two=2)
                                    nc.tensor.matmul(
                                        attps[h4][0:HWP, :], lhs3, rhs3,
                                        start=(kp == 1), stop=(kp == nkp),
                                        perf_mode=PM.DoubleRow)
                                else:
                                    for j in range(2):
                                        c0 = h * 2 * HS + j * HS
                                        nc.tensor.matmul(
                                            attps[h4][0:HW, :],
                                            vp[kp - 1][:, c0:c0 + HW],
                                            pt[:, j * 512:(j + 1) * 512],
                                            start=(kp == 1 and j == 0),
                                            stop=(kp == nkp and j == 1))
                    for h4 in range(4):
                        h = hs * 4 + h4
                        e, r = h // 2, (h % 2) * 64
                        # den must be copied to a partition-0 SBUF tile first:
                        # custom-DVE ops mishandle partition-offset PSUM reads
                        den = rr_p.tile([1, 512], FP32, tag="den", name="den",
                                        bufs=2)
                        nc.vector.tensor_copy(den[:], attps[h4][D:D + 1, :])
                        rec = rr_p.tile([1, 512], FP32, tag="rec", name="rec", bufs=2)
                        nc.vector.reciprocal_approx_fast(rec[:], den[:])
                        rb = rr_p.tile([64, 512], FP32, tag="rb", name="rb", bufs=2)
                        nc.gpsimd.partition_broadcast(rb[:], rec[:])
                        nc.vector.tensor_mul(attn_tiles[e][r:r + 64, :],
                                             attps[h4][0:64, :], rb[:])

            def ln_tile(ps, out_t):
                """out = (ps - mean)/(bessel_std + eps), LN gamma=1 beta=0.

                sqrt runs as exp(0.5*ln(v)) so the Act engine stays in the
                ln+exp table; the final scale/shift rides on Act (Copy with
                per-partition scale/bias) to keep the serial DVE chain short.
                """
                stt = st_p.tile([128, 6], FP32, tag="bnst", name="bnst")
                nc.vector.bn_stats(out=stt[:], in_=ps)
                mv = st_p.tile([128, 2], FP32, tag="bnmv", name="bnmv")
                nc.vector.bn_aggr(out=mv[:], in_=stt[:])
                lnv = st_p.tile([128, 1], FP32, tag="lnv", name="lnv")
                nc.scalar.activation(lnv[:], mv[:, 1:2], AF.Ln,
                                     scale=float(E) / (E - 1))
                sd = st_p.tile([128, 1], FP32, tag="sd", name="sd")
                nc.scalar.activation(sd[:], lnv[:], AF.Exp, scale=0.5)
                nc.vector.tensor_scalar_add(sd[:], sd[:], 1e-6)
                inv = st_p.tile([128, 1], FP32, tag="inv", name="inv")
                nc.vector.reciprocal_approx_fast(inv[:], sd[:])
                negm = st_p.tile([128, 1], FP32, tag="negm", name="negm")
                nc.vector.tensor_scalar_mul(negm[:], mv[:, 0:1], -1.0)
                nm = st_p.tile([128, 1], FP32, tag="nm", name="nm")
                nc.vector.tensor_mul(nm[:], negm[:], inv[:])
                nc.scalar.activation(out_t, ps, AF.Identity, scale=inv[:],
                                     bias=nm[:])

            def out_ln(attn_tiles, wo_t, res_tiles, out_tm, out_fm):
                """out-proj + residual (ident matmul) + LN + DMA transpose."""
                bps = [big_ps(), big_ps()]
                pss = []
                for t in range(TT):
                    ps = bps[t // 2][:, (t % 2) * 512:(t % 2 + 1) * 512]
                    for ei in range(ET):
                        nc.tensor.matmul(
                            ps, attn_tiles[ei][:, t * 128:(t + 1) * 128],
                            wo_t[:, ei * E:(ei + 1) * E],
                            start=(ei == 0), stop=False)
                    nc.tensor.matmul(ps, identt[:], res_tiles[t][:],
                                     start=False, stop=True)
                    pss.append(ps)
                for t in range(TT):
                    ln_tile(pss[t], out_tm[t][:])
                    if out_fm is not None:
                        fm_from_tm(out_fm, out_tm[t], t)

            # --- CA K/V (uses knowfm, which is a 2-chunk FM source) ---
            def ca_kv(l, wk_t, wv_ts):
                kca = [kca_p.tile([128, SK], FP16, tag="kca", name="kca")
                       for _ in range(ET)]
                for c2 in range(2):
                    bps = [big_ps(), big_ps()]
                    for e in range(ET):
                        ps = bps[e // 2][:, (e % 2) * 512:(e % 2 + 1) * 512]
                        for ei in range(ET):
                            nc.tensor.matmul(
                                ps, wk_t[:, ei * E + e * 128:ei * E + (e + 1) * 128],
                                knowfm[:, ei * SK + c2 * 512:ei * SK + (c2 + 1) * 512],
                                start=(ei == 0), stop=(ei == ET - 1))
                        nc.vector.tensor_copy(kca[e][:, c2 * 512:(c2 + 1) * 512], ps)
                vp_ca = [vp_p.tile([128, 2 * H * HS], FP8 if FP8_AV else FP16,
                                   tag="vp", name="vp")
                         for _ in range(KT_CA // 2)]
                for kt in range(KT_CA):
                    psA = small_ps()
                    psB = small_ps()
                    for ei in range(ET):
                        lhs = knowfm[:, ei * SK + kt * 128:ei * SK + (kt + 1) * 128]
                        nc.tensor.matmul(psA[:, 0:HH], lhs, wv_ts[ei][:, 0:HH],
                                         start=(ei == 0), stop=(ei == ET - 1))
                        nc.tensor.matmul(psB[:, 0:HH], lhs,
                                         wv_ts[ei][:, HH:2 * HH],
                                         start=(ei == 0), stop=(ei == ET - 1))
                    vt = vp_ca[kt // 2]
                    j = kt % 2
                    vt4 = vt[:].rearrange("p (h two c) -> p h two c", h=H, two=2)
                    if j == 0 and FP8_AV:
                        nc.vector.memset(vt4[:, :, :, HW:HWP], 0.0)
                    m3 = vmask[:].rearrange("p (h c) -> p h c", h=4)
                    for half, psX in ((0, psA), (1, psB)):
                        p3 = psX[:, 0:HH].rearrange("p (h c) -> p h c", h=4)
                        nc.vector.tensor_add(
                            vt4[:, half * 4:half * 4 + 4, j, 0:HW], p3, m3)
                return kca, vp_ca

            # --- layer 0 prologue: weights + CA KV + own Q ---
            wq_sa_t = load_w(wq_sa, 0, E, "wq_sa")
            wk_sa_t = load_w(wk_sa, 0, E, "wk_sa")
            wv_sa_t = load_wv(wv_sa, 0, "wv_sa")
            wo_sa_t = load_w(wo_sa, 0, E, "wo_sa")
            wq_ca_t = load_w(wq_ca, 0, E, "wq_ca")
            wk_ca_t = load_w(wk_ca, 0, E, "wk_ca")
            wv_ca_t = load_wv(wv_ca, 0, "wv_ca")
            wo_ca_t = load_w(wo_ca, 0, E, "wo_ca")

            ca_state = ca_kv(0, wk_ca_t, wv_ca_t)
            qsa = q_proj(wq_sa_t, ownfm)

            ag_out_prev = None
            for l in range(L):
                with nc.named_scope(f"L{l}"):
                    kca, vp_ca = ca_state
                    # ---- SA K/V from the gathered hidden state ----
                    ksa = [kfm_p.tile([128, S], FP16, tag="kfm", name="kfm")
                           for _ in range(ET)]
                    vp_sa = [vp_p.tile([128, 2 * H * HS],
                                       FP8 if FP8_AV else FP16,
                                       tag="vp", name="vp")
                             for _ in range(KT_SA // 2)]
                    for ch in range(4):
                        hch = hch_p.tile([128, ET * 512], FP16, tag="hch",
                                         name="hch")
                        for ei in range(ET):
                            if l == 0:
                                nc.sync.dma_start(
                                    hch[:, ei * 512:(ei + 1) * 512],
                                    sen_fm[ei * 128:(ei + 1) * 128,
                                           ch * 512:(ch + 1) * 512])
                            else:
                                nc.sync.dma_start(
                                    hch[:, ei * 512:(ei + 1) * 512],
                                    ag_out_prev[ch * 512 + ei * 128:
                                                ch * 512 + (ei + 1) * 128, :])
                        kv_chunk(ksa, ch * 512, hch, wk_sa_t)
                        v_chunk(vp_sa, ch * 4, hch, wv_sa_t)

                    # prefetch next layer K/V/Q weights (rings just freed)
                    if l < L - 1:
                        wk_sa_t = load_w(wk_sa, l + 1, E, "wk_sa")
                        wv_sa_t = load_wv(wv_sa, l + 1, "wv_sa")
                        wq_sa_t = load_w(wq_sa, l + 1, E, "wq_sa")
                        wk_ca_t = load_w(wk_ca, l + 1, E, "wk_ca")
                        wv_ca_t = load_wv(wv_ca, l + 1, "wv_ca")

                    # ---- SA attention + out-proj + LN1 ----
                    attn = [attn_p.tile([128, 512], FP16, tag="attn", name="attn")
                            for _ in range(ET)]
                    attention(qsa, ksa, vp_sa, KT_SA, attn)
                    inter = [stm_p.tile([128, E], FP16, tag="stm", name="stm")
                             for _ in range(TT)]
                    interfm = xfm_p.tile([128, ET * CH], FP16, tag="xfm",
                                         name="xfm")
                    out_ln(attn, wo_sa_t, hid, inter, interfm)
                    if DEBUG_DUMPS and l == 0:
                        nc.sync.dma_start(dbg["dbg_q"][:], qsa[0][:])
                        nc.sync.dma_start(dbg["dbg_k"][:], ksa[0][:])
                        nc.sync.dma_start(dbg["dbg_v"][:], vp_sa[0][:])
                        nc.sync.dma_start(dbg["dbg_attn"][:], attn[0][:])
                        nc.sync.dma_start(dbg["dbg_inter"][:], inter[0][:])
                    if l < L - 1:
                        wo_sa_t = load_w(wo_sa, l + 1, E, "wo_sa")

                    # ---- CA Q + attention + out-proj + LN2 ----
                    qca = q_proj(wq_ca_t, interfm)
                    if l < L - 1:
                        wq_ca_t = load_w(wq_ca, l + 1, E, "wq_ca")
                    attn2 = [attn_p.tile([128, 512], FP16, tag="attn", name="attn")
                             for _ in range(ET)]
                    attention(qca, kca, vp_ca, KT_CA, attn2)
                    co = [stm_p.tile([128, E], FP16, tag="stm", name="stm")
                          for _ in range(TT)]
                    cofm = xfm_p.tile([128, ET * CH], FP16, tag="xfm",
                                      name="xfm")
                    out_ln(attn2, wo_ca_t, inter, co, cofm)
                    if DEBUG_DUMPS and l == 0:
                        nc.sync.dma_start(dbg["dbg_co"][:], co[0][:])
                    if l < L - 1:
                        wo_ca_t = load_w(wo_ca, l + 1, E, "wo_ca")

                    # ---- FFN: software-pipelined h1 -> gelu -> h2 ----
                    w1_ts = []
                    for ei in range(ET):
                        wt = wbig_p.tile([128, F], FP16, tag="w1", name="w1",
                                         bufs=4)
                        nc.sync.dma_start(wt[:], w1_d[l, ei])
                        w1_ts.append(wt)
                    w2_t = wbig_p.tile([128, FT * E], FP8 if FP8_H2 else FP16,
                                       tag="w2", name="w2", bufs=1)
                    for ft in range(FT):
                        nc.sync.dma_start(w2_t[:, ft * E:(ft + 1) * E],
                                          w2_d[l, ft])
                    h2ps = [small_ps() for _ in range(TT)]
                    gts = {}
                    for fp in range(9):
                        if fp < 8:
                            sp2 = big_ps()
                            for j in range(2):
                                ft = fp * 2 + j
                                for ei in range(ET):
                                    nc.tensor.matmul(
                                        sp2[:, j * 512:(j + 1) * 512],
                                        w1_ts[ei][:, ft * 128:(ft + 1) * 128],
                                        cofm[:, ei * 512:(ei + 1) * 512],
                                        start=(ei == 0), stop=(ei == ET - 1))
                            gt = gel_p.tile([128, 1024],
                                            FP8 if FP8_H2 else FP16,
                                            tag="gel", name="gel")
                            if FP8_H2:
                                gt4 = gt[:].rearrange(
                                    "p (t two c) -> p t two c", t=TT, two=2)
                                for j in range(2):
                                    nc.scalar.activation(
                                        gt4[:, :, j, :],
                                        sp2[:, j * 512:(j + 1) * 512]
                                        .rearrange("p (t c) -> p t c", t=TT),
                                        AF.Gelu)
                            else:
                                for j in range(2):
                                    nc.scalar.activation(
                                        gt[:, j * 512:(j + 1) * 512],
                                        sp2[:, j * 512:(j + 1) * 512], AF.Gelu)
                            gts[fp] = gt
                        if fp >= 1:
                            gt = gts.pop(fp - 1)
                            if FP8_H2:
                                w23 = w2_t[:, (fp - 1) * 2 * E:fp * 2 * E] \
                                    .rearrange("p (two c) -> p two c", two=2)
                                for t in range(TT):
                                    nc.tensor.matmul(
                                        h2ps[t][:],
                                        gt[:, t * 256:(t + 1) * 256]
                                        .rearrange("p (two c) -> p two c",
                                                   two=2),
                                        w23, start=(fp == 1), stop=False,
                                        perf_mode=PM.DoubleRow)
                            else:
                                for j in range(2):
                                    ft = (fp - 1) * 2 + j
                                    for t in range(TT):
                                        nc.tensor.matmul(
                                            h2ps[t][:],
                                            gt[:, j * 512 + t * 128:
                                               j * 512 + (t + 1) * 128],
                                            w2_t[:, ft * E:(ft + 1) * E],
                                            start=(ft == 0), stop=False)
                    for t in range(TT):
                        nc.tensor.matmul(h2ps[t][:],
                                         ident32t[:] if FP8_H2 else identt[:],
                                         co[t][:], start=False, stop=True)
                    if l == L - 1:
                        for t in range(TT):
                            o32 = out32_p.tile([128, E], FP32, tag="out32",
                                               name="out32")
                            ln_tile(h2ps[t][:], o32[:])
                            nc.sync.dma_start(out_d[t * 128:(t + 1) * 128, :],
                                              o32[:])
                    else:
                        hidn = [stm_p.tile([128, E], FP16, tag="stm", name="stm")
                                for _ in range(TT)]
                        ownfm_n = ofm_p.tile([128, ET * CH], FP16, tag="ofm",
                                             name="ofm")
                        for t in range(TT):
                            ln_tile(h2ps[t][:], hidn[t][:])
                            fm_from_tm(ownfm_n, hidn[t], t)
                        if DEBUG_DUMPS and l == 0:
                            nc.sync.dma_start(dbg["dbg_hid1"][:], hidn[0][:])
                        ag_in = dram_p.tile([CH, E], FP16, tag="agin", name="agin")
                        for e in range(ET):
                            nc.sync.dma_start(ag_in[e * 128:(e + 1) * 128, :],
                                              ownfm_n[:, e * CH:(e + 1) * CH])
                        ag_out = dram_p.tile([S, E], FP16, tag="agout",
                                             name="agout")
                        nc.gpsimd.collective_compute(
                            "AllGather", OP.bypass, replica_groups=GROUPS,
                            ins=[ag_in.opt()], outs=[ag_out.opt()])
                        # AG-independent work fills the collective latency
                        ca_state = ca_kv(l + 1, wk_ca_t, wv_ca_t)
                        qsa = q_proj(wq_sa_t, ownfm_n)
                        ag_out_prev = ag_out
                        hid = hidn

    nc.compile()
    return nc


def _prep_inputs_fast(sen, know, sa_qkv_w, sa_qkv_b, sa_out_w, sa_out_b,
                      ca_qkv_w, ca_qkv_b, ca_out_w, ca_out_b,
                      ff_w1, ff_b1, ff_w2, ff_b2, ln_g, ln_b):
    f16 = np.float16

    def rowtile(w):  # [L,E,cols] -> [L,ET,128,cols]
        return np.ascontiguousarray(w.reshape(L, ET, 128, -1).astype(f16))

    def padv(w):  # [L,E,E] -> [L,ET,128,H*HW], no bias/ones (mask adds ones)
        wp = np.zeros((L, E, H, HW), np.float32)
        wp[:, :, :, :D] = w.reshape(L, E, H, D)
        return np.ascontiguousarray(wp.reshape(L, ET, 128, H * HW).astype(f16))

    f8 = mybir.dt.np(FP8)
    common = {
        "ident": np.eye(128, dtype=f16),
        "ident32": (np.eye(128) * (W2_SCALE if FP8_H2 else 1.0)).astype(f16),
        "wq_sa": rowtile(sa_qkv_w[:, 0]), "wk_sa": rowtile(sa_qkv_w[:, 1]),
        "wv_sa": padv(sa_qkv_w[:, 2]),
        "wo_sa": rowtile(sa_out_w),
        "wq_ca": rowtile(ca_qkv_w[:, 0]), "wk_ca": rowtile(ca_qkv_w[:, 1]),
        "wv_ca": padv(ca_qkv_w[:, 2]),
        "wo_ca": rowtile(ca_out_w),
        "w1": rowtile(ff_w1),
        "w2": np.ascontiguousarray(
            (ff_w2 * W2_SCALE).reshape(L, FT, 128, E).astype(f8))
        if FP8_H2 else
        np.ascontiguousarray(ff_w2.reshape(L, FT, 128, E).astype(f16)),
    }
    in_maps = []
    for core in range(NCORES):
        g, c = core // 4, core % 4
        m = dict(common)
        m["sen_fm"] = np.ascontiguousarray(sen[g].T.astype(f16))
        m["own_fm0"] = np.ascontiguousarray(sen[g, c * CH:(c + 1) * CH].T.astype(f16))
        m["own_tm0"] = np.ascontiguousarray(sen[g, c * CH:(c + 1) * CH].astype(f16))
        m["know_fm"] = np.ascontiguousarray(know[g].T.astype(f16))
        in_maps.append(m)
    return in_maps


def _build_general():
    """Fallback for inputs with non-zero biases / non-unit LN gamma."""
    nc = bacc.Bacc("TRN2", target_bir_lowering=False, debug=False, num_devices=NCORES)

    def din(name, shape, dt=FP16):
        return nc.dram_tensor(name, shape, dt, kind="ExternalInput").ap()

    sen_fm = din("sen_fm", [E, S])
    own_fm0 = din("own_fm0", [E, CH])
    own_tm0 = din("own_tm0", [CH, E])
    know_fm_d = din("know_fm", [E, SK])
    ident_d = din("ident", [128, 128])
    ones_d = din("ones", [1, 128])

    wq_sa = din("wq_sa", [L, ET, ET, 128, 128])
    wk_sa = din("wk_sa", [L, ET, ET, 128, 128])
    wv_sa = din("wv_sa", [L, ET, 128, H * HW])
    wo_sa = din("wo_sa", [L, ET, 128, E])
    wq_ca = din("wq_ca", [L, ET, ET, 128, 128])
    wk_ca = din("wk_ca", [L, ET, ET, 128, 128])
    wv_ca = din("wv_ca", [L, ET, 128, H * HW])
    wo_ca = din("wo_ca", [L, ET, 128, E])
    w1_d = din("w1", [L, ET, FT, 128, 128])
    w2_d = din("w2", [L, FT, 128, E])

    bq_sa = din("bq_sa", [L, 128, ET], FP32)
    bk_sa = din("bk_sa", [L, 128, ET], FP32)
    bq_ca = din("bq_ca", [L, 128, ET], FP32)
    bk_ca = din("bk_ca", [L, 128, ET], FP32)
    b1_d = din("b1", [L, 128, FT], FP32)
    rbv_sa = din("rbv_sa", [L, 1, H * HW])
    rbo_sa = din("rbo_sa", [L, 1, E])
    rbv_ca = din("rbv_ca", [L, 1, H * HW])
    rbo_ca = din("rbo_ca", [L, 1, E])
    rb2_d = din("rb2", [L, 1, E])
    lng_d = din("lng", [L, 1, E], FP32)
    lnb_d = din("lnb", [L, 1, E], FP32)

    out_d = nc.dram_tensor("out_tm", [CH, E], FP32, kind="ExternalOutput").ap()

    with tile.TileContext(nc) as tc:
        from contextlib import ExitStack
        with ExitStack() as ctx:
            ep = ctx.enter_context
            const_p = ep(tc.tile_pool(name="const", bufs=1))
            know_p = ep(tc.tile_pool(name="know", bufs=4))
            kfm_p = ep(tc.tile_pool(name="kfm", bufs=4))
            kca_p = ep(tc.tile_pool(name="kca", bufs=4))
            vp_p = ep(tc.tile_pool(name="vp", bufs=27))
            hch_p = ep(tc.tile_pool(name="hch", bufs=6))
            qfm_p = ep(tc.tile_pool(name="qfm", bufs=8))
            attn_p = ep(tc.tile_pool(name="attn", bufs=4))
            ofm_p = ep(tc.tile_pool(name="ofm", bufs=8))
            xfm_p = ep(tc.tile_pool(name="xfm", bufs=5))
            stm_p = ep(tc.tile_pool(name="stm", bufs=8))
            out32_p = ep(tc.tile_pool(name="out32", bufs=2))
            pt_p = ep(tc.tile_pool(name="pt", bufs=6))
            gel_p = ep(tc.tile_pool(name="gel", bufs=17))
            wl_p = ep(tc.tile_pool(name="wl", bufs=16))
            wr_p = ep(tc.tile_pool(name="wr", bufs=6))
            row_p = ep(tc.tile_pool(name="row", bufs=4))
            gb_p = ep(tc.tile_pool(name="gb", bufs=2))
            sc_p = ep(tc.tile_pool(name="sc", bufs=3))
            s1_p = ep(tc.tile_pool(name="s1", bufs=2))
            st_p = ep(tc.tile_pool(name="st", bufs=8))
            ps_p = ep(tc.tile_pool(name="ps", bufs=8, space="PSUM"))
            dram_p = ep(tc.tile_pool(name="dram", bufs=2, space="DRAM"))

            identt = const_p.tile([128, 128], FP16, tag="ident", name="ident")
            nc.sync.dma_start(identt[:], ident_d[:])
            onest = const_p.tile([1, 128], FP16, tag="ones", name="ones")
            nc.sync.dma_start(onest[:], ones_d[:])
            knowfm = []
            for e in range(ET):
                t = know_p.tile([128, SK], FP16, tag="know", name="know")
                nc.sync.dma_start(t[:], know_fm_d[e * 128:(e + 1) * 128, :])
                knowfm.append(t)

            hid = []
            for t in range(TT):
                h = stm_p.tile([128, E], FP16, tag="stm", name="stm")
                nc.sync.dma_start(h[:], own_tm0[t * 128:(t + 1) * 128, :])
                hid.append(h)
            ownfm = []
            for e in range(ET):
                t = ofm_p.tile([128, CH], FP16, tag="ofm", name="ofm")
                nc.sync.dma_start(t[:], own_fm0[e * 128:(e + 1) * 128, :])
                ownfm.append(t)

            def ln_norm(xres, G, Bt, out):
                stt = st_p.tile([128, 6], FP32, tag="bnst", name="bnst")
                nc.vector.bn_stats(out=stt[:], in_=xres[:])
                mv = st_p.tile([128, 2], FP32, tag="bnmv", name="bnmv")
                nc.vector.bn_aggr(out=mv[:], in_=stt[:])
                sd = st_p.tile([128, 1], FP32, tag="sd", name="sd")
                nc.scalar.activation(sd[:], mv[:, 1:2], AF.Sqrt,
                                     scale=float(E) / (E - 1))
                nc.vector.tensor_scalar_add(sd[:], sd[:], 1e-6)
                inv = st_p.tile([128, 1], FP32, tag="inv", name="inv")
                nc.vector.reciprocal_approx_fast(inv[:], sd[:])
                minv = st_p.tile([128, 1], FP32, tag="minv", name="minv")
                nc.vector.tensor_mul(minv[:], mv[:, 0:1], inv[:])
                tmp = sc_p.tile([128, E], FP32, tag="lntmp", name="lntmp")
                nc.vector.tensor_scalar(tmp[:], in0=xres[:], scalar1=inv[:],
                                        scalar2=minv[:], op0=OP.mult,
                                        op1=OP.subtract)
                nc.vector.tensor_mul(tmp[:], tmp[:], G[:])
                nc.vector.tensor_add(out[:], tmp[:], Bt[:])

            def transpose_to(dst_tiles, src_tile, t):
                for e in range(ET):
                    tp = ps_p.tile([128, 128], FP16, tag="ps", name="ps")
                    nc.tensor.transpose(tp[:], src_tile[:, e * 128:(e + 1) * 128],
                                        identt[:])
                    nc.vector.tensor_copy(dst_tiles[e][:, t * 128:(t + 1) * 128],
                                          tp[:])

            def load_w16(wdram, l):
                ts = {}
                for ei in range(ET):
                    for e in range(ET):
                        wt = wl_p.tile([128, 128], FP16, tag="wl", name="wl")
                        nc.sync.dma_start(wt[:], wdram[l, ei, e])
                        ts[ei, e] = wt
                return ts

            def load_bias(bdram, l, n):
                bt = st_p.tile([128, n], FP32, tag="bias", name="bias", bufs=6)
                nc.sync.dma_start(bt[:], bdram[l])
                return bt

            def kv_proj(kdst, n_tok, src_tiles, src_col0, wk_tiles, bkt):
                nch = n_tok // 512
                for e in range(ET):
                    for c2 in range(nch):
                        pst = ps_p.tile([128, 512], FP32, tag="ps", name="ps")
                        for ei in range(ET):
                            nc.tensor.matmul(pst[:], wk_tiles[ei, e][:],
                                             src_tiles[ei][:, c2 * 512:(c2 + 1) * 512],
                                             start=(ei == 0), stop=(ei == ET - 1))
                        nc.vector.tensor_scalar_add(
                            kdst[e][:, src_col0 + c2 * 512:src_col0 + (c2 + 1) * 512],
                            pst[:], bkt[:, e:e + 1])

            def v_proj(vdst, kt0, nkt, src_tiles, wv_tiles, rbv):
                for ktl in range(nkt):
                    vt = vdst[kt0 + ktl]
                    for half in range(2):
                        pst = ps_p.tile([128, H * HW // 2], FP32, tag="ps",
                                        name="ps")
                        cs = half * (H * HW // 2)
                        for ei in range(ET):
                            nc.tensor.matmul(
                                pst[:], src_tiles[ei][:, ktl * 128:(ktl + 1) * 128],
                                wv_tiles[ei][:, cs:cs + H * HW // 2],
                                start=(ei == 0), stop=False)
                        nc.tensor.matmul(pst[:], onest[:],
                                         rbv[:, cs:cs + H * HW // 2],
                                         start=False, stop=True)
                        nc.vector.tensor_copy(vt[:, cs:cs + H * HW // 2], pst[:])

            def attention(qfm, kfm, vp, nkt, attn_tiles):
                for hs in range(2):
                    attps = [ps_p.tile([HW, 512], FP32, tag="ps", name="ps")
                             for _ in range(4)]
                    for kt in range(nkt):
                        for h4 in range(4):
                            h = hs * 4 + h4
                            e, r = h // 2, (h % 2) * 64
                            spt = ps_p.tile([128, 512], FP32, tag="ps", name="ps")
                            nc.tensor.matmul(
                                spt[:], kfm[e][r:r + 64, kt * 128:(kt + 1) * 128],
                                qfm[e][r:r + 64, :], start=True, stop=True)
                            pt = pt_p.tile([128, 512], FP16, tag="pt", name="pt")
                            nc.scalar.activation(pt[:], spt[:], AF.Exp, scale=0.125)
                            nc.tensor.matmul(attps[h4][:],
                                             vp[kt][:, h * HW:(h + 1) * HW],
                                             pt[:], start=(kt == 0),
                                             stop=(kt == nkt - 1))
                    for h4 in range(4):
                        h = hs * 4 + h4
                        e, r = h // 2, (h % 2) * 64
                        ats = sc_p.tile([64, 512], FP32, tag="ats", name="ats",
                                        bufs=4)
                        nc.scalar.activation(ats[:], attps[h4][0:64, :], AF.Copy)
                        den = s1_p.tile([1, 512], FP32, tag="den", name="den")
                        nc.vector.tensor_copy(den[:], attps[h4][64:65, :])
                        rec = s1_p.tile([1, 512], FP32, tag="rec", name="rec")
                        nc.vector.reciprocal_approx_fast(rec[:], den[:])
                        rb = sc_p.tile([64, 512], FP32, tag="rb", name="rb")
                        nc.gpsimd.partition_broadcast(rb[:], rec[:])
                        nc.vector.tensor_mul(attn_tiles[e][r:r + 64, :],
                                             ats[:], rb[:])

            def out_proj_ln(attn_tiles, wo_tiles, rbo, res_tiles, G, Bt, out_tiles):
                for t in range(TT):
                    pst = ps_p.tile([128, E], FP32, tag="ps", name="ps")
                    for ei in range(ET):
                        nc.tensor.matmul(pst[:],
                                         attn_tiles[ei][:, t * 128:(t + 1) * 128],
                                         wo_tiles[ei][:], start=(ei == 0),
                                         stop=False)
                    nc.tensor.matmul(pst[:], onest[:], rbo[:], start=False,
                                     stop=True)
                    xres = sc_p.tile([128, E], FP32, tag="xres", name="xres")
                    nc.vector.tensor_add(xres[:], pst[:], res_tiles[t][:])
                    ln_norm(xres, G, Bt, out_tiles[t])

            def make_ca_kv(l):
                kca = [kca_p.tile([128, SK], FP16, tag="kca", name="kca")
                       for _ in range(ET)]
                wkt_ca = load_w16(wk_ca, l)
                bkt_ca = load_bias(bk_ca, l, ET)
                kv_proj(kca, SK, knowfm, 0, wkt_ca, bkt_ca)
                vp_ca = [vp_p.tile([128, H * HW], FP16, tag="vp", name="vp")
                         for _ in range(KT_CA)]
                wvt_ca = []
                for ei in range(ET):
                    wt = wr_p.tile([128, H * HW], FP16, tag="wr", name="wr")
                    nc.sync.dma_start(wt[:], wv_ca[l, ei])
                    wvt_ca.append(wt)
                rbv = row_p.tile([1, H * HW], FP16, tag="row", name="row")
                nc.sync.dma_start(rbv[:], rbv_ca[l])
                v_proj(vp_ca, 0, KT_CA, knowfm, wvt_ca, rbv)
                return kca, vp_ca

            ag_out_prev = None
            ca_kv_next = None
            for l in range(L):
                with nc.named_scope(f"L{l}"):
                    if l == 0:
                        kca, vp_ca = make_ca_kv(0)
                    else:
                        kca, vp_ca = ca_kv_next
                    lr = s1_p.tile([1, E], FP32, tag="lnrow", name="lnrow")
                    nc.sync.dma_start(lr[:], lng_d[l])
                    G = gb_p.tile([128, E], FP32, tag="G", name="G")
                    nc.gpsimd.partition_broadcast(G[:], lr[:])
                    lr2 = s1_p.tile([1, E], FP32, tag="lnrow", name="lnrow")
                    nc.sync.dma_start(lr2[:], lnb_d[l])
                    Bt = gb_p.tile([128, E], FP32, tag="B", name="B")
                    nc.gpsimd.partition_broadcast(Bt[:], lr2[:])

                    ksa = [kfm_p.tile([128, S], FP16, tag="kfm", name="kfm")
                           for _ in range(ET)]
                    vp_sa = [vp_p.tile([128, H * HW], FP16, tag="vp", name="vp")
                             for _ in range(KT_SA)]
                    wkt_sa = load_w16(wk_sa, l)
                    wvt_sa = []
                    for ei in range(ET):
                        wt = wr_p.tile([128, H * HW], FP16, tag="wr", name="wr")
                        nc.sync.dma_start(wt[:], wv_sa[l, ei])
                        wvt_sa.append(wt)
                    rbvs = row_p.tile([1, H * HW], FP16, tag="row", name="row")
                    nc.sync.dma_start(rbvs[:], rbv_sa[l])
                    bkt_sa = load_bias(bk_sa, l, ET)
                    for ch in range(4):
                        hch = []
                        for ei in range(ET):
                            ht = hch_p.tile([128, 512], FP16, tag="hch", name="hch")
                            if l == 0:
                                nc.sync.dma_start(
                                    ht[:], sen_fm[ei * 128:(ei + 1) * 128,
                                                  ch * 512:(ch + 1) * 512])
                            else:
                                nc.sync.dma_start(
                                    ht[:], ag_out_prev[ch * 512 + ei * 128:
                                                       ch * 512 + (ei + 1) * 128, :])
                            hch.append(ht)
                        kv_proj(ksa, 512, hch, ch * 512, wkt_sa, bkt_sa)
                        v_proj(vp_sa, ch * 4, 4, hch, wvt_sa, rbvs)

                    if l == 0:
                        qsa = [qfm_p.tile([128, 512], FP16, tag="qfm", name="qfm")
                               for _ in range(ET)]
                        wqt_sa = load_w16(wq_sa, l)
                        bqt = load_bias(bq_sa, l, ET)
                        for e in range(ET):
                            pst = ps_p.tile([128, 512], FP32, tag="ps", name="ps")
                            for ei in range(ET):
                                nc.tensor.matmul(pst[:], wqt_sa[ei, e][:],
                                                 ownfm[ei][:],
                                                 start=(ei == 0),
                                                 stop=(ei == ET - 1))
                            nc.vector.tensor_scalar_add(qsa[e][:], pst[:],
                                                        bqt[:, e:e + 1])
                    else:
                        qsa = qsa_next

                    attn = [attn_p.tile([128, 512], FP16, tag="attn", name="attn")
                            for _ in range(ET)]
                    attention(qsa, ksa, vp_sa, KT_SA, attn)
                    wot = []
                    for ei in range(ET):
                        wt = wr_p.tile([128, E], FP16, tag="wr", name="wr")
                        nc.sync.dma_start(wt[:], wo_sa[l, ei])
                        wot.append(wt)
                    rbo = row_p.tile([1, E], FP16, tag="row", name="row")
                    nc.sync.dma_start(rbo[:], rbo_sa[l])
                    inter = [stm_p.tile([128, E], FP16, tag="stm", name="stm")
                             for _ in range(TT)]
                    out_proj_ln(attn, wot, rbo, hid, G, Bt, inter)

                    interfm = [xfm_p.tile([128, CH], FP16, tag="xfm", name="xfm")
                               for _ in range(ET)]
                    for t in range(TT):
                        transpose_to(interfm, inter[t], t)

                    qca = [qfm_p.tile([128, 512], FP16, tag="qfm", name="qfm")
                           for _ in range(ET)]
                    wqt_ca = load_w16(wq_ca, l)
                    bqt_ca = load_bias(bq_ca, l, ET)
                    for e in range(ET):
                        pst = ps_p.tile([128, 512], FP32, tag="ps", name="ps")
                        for ei in range(ET):
                            nc.tensor.matmul(pst[:], wqt_ca[ei, e][:],
                                             interfm[ei][:],
                                             start=(ei == 0), stop=(ei == ET - 1))
                        nc.vector.tensor_scalar_add(qca[e][:], pst[:],
                                                    bqt_ca[:, e:e + 1])

                    attn2 = [attn_p.tile([128, 512], FP16, tag="attn", name="attn")
                             for _ in range(ET)]
                    attention(qca, kca, vp_ca, KT_CA, attn2)
                    wot2 = []
                    for ei in range(ET):
                        wt = wr_p.tile([128, E], FP16, tag="wr", name="wr")
                        nc.sync.dma_start(wt[:], wo_ca[l, ei])
                        wot2.append(wt)
                    rbo2 = row_p.tile([1, E], FP16, tag="row", name="row")
                    nc.sync.dma_start(rbo2[:], rbo_ca[l])
                    co = [stm_p.tile([128, E], FP16, tag="stm", name="stm")
                          for _ in range(TT)]
                    out_proj_ln(attn2, wot2, rbo2, inter, G, Bt, co)

                    cofm = [xfm_p.tile([128, CH], FP16, tag="xfm", name="xfm")
                            for _ in range(ET)]
                    for t in range(TT):
                        transpose_to(cofm, co[t], t)

                    rb2 = row_p.tile([1, E], FP16, tag="row", name="row")
                    nc.sync.dma_start(rb2[:], rb2_d[l])
                    b1t = load_bias(b1_d, l, FT)
                    gel = []
                    for ft in range(FT):
                        pst = ps_p.tile([128, 512], FP32, tag="ps", name="ps")
                        for ei in range(ET):
                            wt = wl_p.tile([128, 128], FP16, tag="wl", name="wl")
                            nc.sync.dma_start(wt[:], w1_d[l, ei, ft])
                            nc.tensor.matmul(pst[:], wt[:], cofm[ei][:],
                                             start=(ei == 0), stop=(ei == ET - 1))
                        gt = gel_p.tile([128, 512], FP16, tag="gel", name="gel")
                        nc.scalar.activation(gt[:], pst[:], AF.Gelu,
                                             bias=b1t[:, ft:ft + 1])
                        gel.append(gt)
                    w2ts = []
                    for ft in range(FT):
                        w2t = wr_p.tile([128, E], FP16, tag="w2r", name="w2r",
                                        bufs=17)
                        nc.sync.dma_start(w2t[:], w2_d[l, ft])
                        w2ts.append(w2t)
                    h2ps = [ps_p.tile([128, E], FP32, tag="ps", name="ps")
                            for _ in range(TT)]
                    for t in range(TT):
                        for ft in range(FT):
                            nc.tensor.matmul(h2ps[t][:],
                                             gel[ft][:, t * 128:(t + 1) * 128],
                                             w2ts[ft][:], start=(ft == 0),
                                             stop=False)
                    if l == L - 1:
                        hidn = [out32_p.tile([128, E], FP32, tag="out32",
                                             name="out32") for _ in range(TT)]
                    else:
                        hidn = [stm_p.tile([128, E], FP16, tag="stm", name="stm")
                                for _ in range(TT)]
                    for t in range(TT):
                        nc.tensor.matmul(h2ps[t][:], onest[:], rb2[:],
                                         start=False, stop=True)
                        xres = sc_p.tile([128, E], FP32, tag="xres", name="xres")
                        nc.vector.tensor_add(xres[:], h2ps[t][:], co[t][:])
                        ln_norm(xres, G, Bt, hidn[t])
                        if l == L - 1:
                            nc.sync.dma_start(out_d[t * 128:(t + 1) * 128, :],
                                              hidn[t][:])

                    if l < L - 1:
                        ownfm_n = [ofm_p.tile([128, CH], FP16, tag="ofm",
                                              name="ofm") for _ in range(ET)]
                        for t in range(TT):
                            transpose_to(ownfm_n, hidn[t], t)
                        ag_in = dram_p.tile([CH, E], FP16, tag="agin", name="agin")
                        for e in range(ET):
                            nc.sync.dma_start(ag_in[e * 128:(e + 1) * 128, :],
                                              ownfm_n[e][:])
                        ag_out = dram_p.tile([S, E], FP16, tag="agout",
                                             name="agout")
                        nc.gpsimd.collective_compute(
                            "AllGather", OP.bypass, replica_groups=GROUPS,
                            ins=[ag_in.opt()], outs=[ag_out.opt()])
                        ca_kv_next = make_ca_kv(l + 1)
                        qsa_next = [qfm_p.tile([128, 512], FP16, tag="qfm",
                                               name="qfm") for _ in range(ET)]
                        wqt_n = load_w16(wq_sa, l + 1)
                        bqt_n = load_bias(bq_sa, l + 1, ET)
                        for e in range(ET):
                            pst = ps_p.tile([128, 512], FP32, tag="ps", name="ps")
                            for ei in range(ET):
                                nc.tensor.matmul(pst[:], wqt_n[ei, e][:],
                                                 ownfm_n[ei][:],
                                                 start=(ei == 0),
                                                 stop=(ei == ET - 1))
                            nc.vector.tensor_scalar_add(qsa_next[e][:], pst[:],
                                                        bqt_n[:, e:e + 1])
                        ag_out_prev = ag_out
                        ownfm = ownfm_n
                        hid = hidn

    nc.compile()
    return nc


def _prep_inputs(sen, know, sa_qkv_w, sa_qkv_b, sa_out_w, sa_out_b,
                 ca_qkv_w, ca_qkv_b, ca_out_w, ca_out_b,
                 ff_w1, ff_b1, ff_w2, ff_b2, ln_g, ln_b):
    """Host-side weight packing for the general fallback kernel."""
    f16, f32 = np.float16, np.float32

    def tile4(w):
        return np.ascontiguousarray(
            w.reshape(L, ET, 128, ET, 128).transpose(0, 1, 3, 2, 4).astype(f16))

    def padv(w, b):
        wp = np.zeros((L, E, H, HW), f32)
        wp[:, :, :, :D] = w.reshape(L, E, H, D)
        bp = np.zeros((L, H, HW), f32)
        bp[:, :, :D] = b.reshape(L, H, D)
        bp[:, :, D] = 1.0
        return (np.ascontiguousarray(wp.reshape(L, ET, 128, H * HW).astype(f16)),
                np.ascontiguousarray(bp.reshape(L, 1, H * HW).astype(f16)))

    wv_sa_p, rbv_sa_h = padv(sa_qkv_w[:, 2], sa_qkv_b[:, 2])
    wv_ca_p, rbv_ca_h = padv(ca_qkv_w[:, 2], ca_qkv_b[:, 2])

    common = {
        "ident": np.eye(128, dtype=f16),
        "ones": np.ones((1, 128), f16),
        "wq_sa": tile4(sa_qkv_w[:, 0]), "wk_sa": tile4(sa_qkv_w[:, 1]),
        "wv_sa": wv_sa_p,
        "wo_sa": np.ascontiguousarray(sa_out_w.reshape(L, ET, 128, E).astype(f16)),
        "wq_ca": tile4(ca_qkv_w[:, 0]), "wk_ca": tile4(ca_qkv_w[:, 1]),
        "wv_ca": wv_ca_p,
        "wo_ca": np.ascontiguousarray(ca_out_w.reshape(L, ET, 128, E).astype(f16)),
        "w1": np.ascontiguousarray(
            ff_w1.reshape(L, ET, 128, FT, 128).transpose(0, 1, 3, 2, 4).astype(f16)),
        "w2": np.ascontiguousarray(ff_w2.reshape(L, FT, 128, E).astype(f16)),
        "bq_sa": np.ascontiguousarray(
            sa_qkv_b[:, 0].reshape(L, ET, 128).transpose(0, 2, 1)),
        "bk_sa": np.ascontiguousarray(
            sa_qkv_b[:, 1].reshape(L, ET, 128).transpose(0, 2, 1)),
        "bq_ca": np.ascontiguousarray(
            ca_qkv_b[:, 0].reshape(L, ET, 128).transpose(0, 2, 1)),
        "bk_ca": np.ascontiguousarray(
            ca_qkv_b[:, 1].reshape(L, ET, 128).transpose(0, 2, 1)),
        "b1": np.ascontiguousarray(
            ff_b1.reshape(L, FT, 128).transpose(0, 2, 1)),
        "rbv_sa": rbv_sa_h, "rbv_ca": rbv_ca_h,
        "rbo_sa": np.ascontiguousarray(sa_out_b[:, None, :].astype(f16)),
        "rbo_ca": np.ascontiguousarray(ca_out_b[:, None, :].astype(f16)),
        "rb2": np.ascontiguousarray(ff_b2[:, None, :].astype(f16)),
        "lng": np.ascontiguousarray(ln_g[:, None, :]),
        "lnb": np.ascontiguousarray(ln_b[:, None, :]),
    }
    in_maps = []
    for core in range(NCORES):
        g, c = core // 4, core % 4
        m = dict(common)
        m["sen_fm"] = np.ascontiguousarray(sen[g].T.astype(f16))
        m["own_fm0"] = np.ascontiguousarray(sen[g, c * CH:(c + 1) * CH].T.astype(f16))
        m["own_tm0"] = np.ascontiguousarray(sen[g, c * CH:(c + 1) * CH].astype(f16))
        m["know_fm"] = np.ascontiguousarray(know[g].T.astype(f16))
        in_maps.append(m)
    return in_maps


def _inputs_are_fast(sa_qkv_b, sa_out_b, ca_qkv_b, ca_out_b,
                     ff_b1, ff_b2, ln_g, ln_b, **_):
    zeros = [sa_qkv_b, sa_out_b, ca_qkv_b, ca_out_b, ff_b1, ff_b2, ln_b]
    return all(not np.any(z) for z in zeros) and np.all(ln_g == 1.0)


def kernel(**inputs):
    inputs = {k: np.asarray(v, dtype=np.float32) for k, v in inputs.items()}
    if _inputs_are_fast(**inputs):
        if "nc" not in _CACHE:
            _CACHE["nc"] = _build_fast()
        nc = _CACHE["nc"]
        in_maps = _prep_inputs_fast(**inputs)
    else:
        if "nc_gen" not in _CACHE:
            _CACHE["nc_gen"] = _build_general()
        nc = _CACHE["nc_gen"]
        in_maps = _prep_inputs(**inputs)
        _CACHE["nc"] = nc
    res = run_bass_kernel_spmd(nc, in_maps, list(range(NCORES)))
    out = np.empty((B, S, E), np.float32)
    for core in range(NCORES):
        g, c = core // 4, core % 4
        out[g, c * CH:(c + 1) * CH] = res.results[core]["out_tm"]
    return out


# revision 56
# speedup vs baseline: 1.0996x; 1.0735x over previous
"""Trainium2 Bass kernel for a 4-layer hierarchical-attention encoder.

Sharding: 8 cores = 2 batch groups x 4 sequence chunks of 512 query tokens.
Each core runs the full layer stack for its 512 tokens; the hidden state is
all-gathered (per batch group) at each layer boundary so every core can
compute full-sequence self-attention K/V locally.

Fast path (inputs with zero biases, unit LN gamma, zero LN beta — which is
what setup_inputs() produces):
 - no bias matmul rows; K-bias is dropped (exactly free under softmax)
 - residual adds ride on the PE via identity matmuls into the out-proj PSUM
 - LN sqrt computed as exp(0.5*ln(v)) so the Act engine never leaves the
   exp table on the critical path
 - exp/gelu processed on 2-bank (1024-wide) PSUM regions
 - all TM->FM transposes on the DMA transpose engine (PE/DVE freed)
 - attention denominator via a ones-mask added during the V' PSUM drain
 - batched weight DMA layouts ([128, E]-row tiles)
A general fallback (the previous kernel) handles arbitrary bias/gamma.
"""
import os
import sys

for _p in ("/root/.axon_site/_ro/trn_rl_repo", "/opt/trn_rl_repo", "/opt/pypackages",
           "/root/.axon_site/_ro/pypackages"):
    if os.path.isdir(_p) and _p not in sys.path:
        sys.path.append(_p)

import numpy as np

import concourse.bass as bass
import concourse.mybir as mybir
import concourse.tile as tile
from concourse import bacc
from concourse.bass_utils import run_bass_kernel_spmd

L, E, H, D, F = 4, 512, 8, 64, 2048
B, S, SK = 2, 2048, 1024
NCORES = 8
GROUPS = [[0, 1, 2, 3], [4, 5, 6, 7]]
CH = 512          # tokens per core
ET = E // 128     # 4 feature tiles
TT = CH // 128    # 4 token tiles in own chunk
FT = F // 128     # 16 ffn tiles
KT_SA = S // 128  # 16 key tiles (self)
KT_CA = SK // 128  # 8 key tiles (cross)
HW = 65           # head width incl. denominator column
HWP = 80          # fp8 DoubleRow head stride (M%16==0 requirement)

FP32 = mybir.dt.float32
FP16 = mybir.dt.float16
FP8 = mybir.dt.float8e4
AF = mybir.ActivationFunctionType
OP = mybir.AluOpType
PM = mybir.MatmulPerfMode
W2_SCALE = 32.0  # fp8 w2 pre-scale; exact under LN's scale invariance

_CACHE = {}

# Feature toggles for HW bring-up bisection.
# EXP_2BANK stays off: a single Act instruction reading a PSUM access
# pattern that crosses a bank boundary wedges the exec unit on HW.
USE_DMA_TRANSPOSE = os.environ.get("K_DMA_T", "1") == "1"
EXP_2BANK = os.environ.get("K_EXP2", "0") == "1"
DEBUG_DUMPS = os.environ.get("K_DEBUG", "0") == "1"
FP8_AV = os.environ.get("K_FP8AV", "0") == "1"
FP8_H2 = os.environ.get("K_FP8H2", "0") == "1"
HS = HWP if FP8_AV else HW  # per-(head,pair-slot) stride in V tiles


def _patch_act_tables():
    """Steer the act-table-load pass away from the exp-less `natural_log`
    table so Ln resolves to `natural_log_exp_and_others` and the LN
    ln/exp pair never swaps tables against the attention exps.

    Only the bass-side selector sees the emptied entry; table ids and the
    hardware table contents (walrus reads act_info.json directly) are
    unchanged, so every emitted load remains valid.
    """
    import concourse.bacc as bacc_mod
    if getattr(bacc_mod, "_ln_exp_patched", False):
        return
    orig = bacc_mod.get_activation_tables

    def patched(arch):
        tables = dict(orig(arch))
        shared = "natural_log_exp_and_others"
        if shared in tables and {mybir.ActivationFunctionType.Exp,
                                 mybir.ActivationFunctionType.Ln} <= tables[shared]:
            for name, fns in tables.items():
                if name != shared:
                    tables[name] = fns - {mybir.ActivationFunctionType.Exp,
                                          mybir.ActivationFunctionType.Ln}
        return tables

    bacc_mod.get_activation_tables = patched
    bacc_mod._ln_exp_patched = True


def _build_fast():
    _patch_act_tables()
    nc = bacc.Bacc("TRN2", target_bir_lowering=False, debug=False, num_devices=NCORES)

    def din(name, shape, dt=FP16):
        return nc.dram_tensor(name, shape, dt, kind="ExternalInput").ap()

    sen_fm = din("sen_fm", [E, S])            # full batch sequence, feature-major
    own_fm0 = din("own_fm0", [E, CH])         # own chunk, feature-major
    own_tm0 = din("own_tm0", [CH, E])         # own chunk, token-major
    know_fm_d = din("know_fm", [E, SK])
    ident_d = din("ident", [128, 128])
    ident32_d = din("ident32", [128, 128])

    wq_sa = din("wq_sa", [L, ET, 128, E])
    wk_sa = din("wk_sa", [L, ET, 128, E])
    wv_sa = din("wv_sa", [L, ET, 128, H * HW])
    wo_sa = din("wo_sa", [L, ET, 128, E])
    wq_ca = din("wq_ca", [L, ET, 128, E])
    wk_ca = din("wk_ca", [L, ET, 128, E])
    wv_ca = din("wv_ca", [L, ET, 128, H * HW])
    wo_ca = din("wo_ca", [L, ET, 128, E])
    w1_d = din("w1", [L, ET, 128, F])
    w2_d = din("w2", [L, FT, 128, E], FP8 if FP8_H2 else FP16)

    out_d = nc.dram_tensor("out_tm", [CH, E], FP32, kind="ExternalOutput").ap()
    dbg = {}
    if DEBUG_DUMPS:
        for nm, shape in [("dbg_q", [128, 512]), ("dbg_k", [128, S]),
                          ("dbg_v", [128, H * HW]), ("dbg_attn", [128, 512]),
                          ("dbg_inter", [128, E]), ("dbg_co", [128, E]),
                          ("dbg_hid1", [128, E])]:
            dbg[nm] = nc.dram_tensor(nm, shape, FP16,
                                     kind="ExternalOutput").ap()

    HH = H * HW // 2  # 260, half of the padded V width

    with tile.TileContext(nc) as tc:
        from contextlib import ExitStack
        with ExitStack() as ctx:
            ep = ctx.enter_context
            const_p = ep(tc.tile_pool(name="const", bufs=1))
            know_p = ep(tc.tile_pool(name="know", bufs=1))    # [128,4096] know FM
            kfm_p = ep(tc.tile_pool(name="kfm", bufs=4))      # [128,2048] SA K fp16
            kca_p = ep(tc.tile_pool(name="kca", bufs=4))      # [128,1024] CA K fp16
            vp_p = ep(tc.tile_pool(name="vp", bufs=12))       # resident V' pairs
            hch_p = ep(tc.tile_pool(name="hch", bufs=2))      # [128,2048] H_fm chunk
            qfm_p = ep(tc.tile_pool(name="qfm", bufs=8))
            attn_p = ep(tc.tile_pool(name="attn", bufs=8))
            pt_p = ep(tc.tile_pool(name="pt", bufs=4))        # exp out [128,1024] fp16
            gel_p = ep(tc.tile_pool(name="gel", bufs=3))      # [128,1024] fp16
            stm_p = ep(tc.tile_pool(name="stm", bufs=12))     # hid/inter/co TM fp16
            xfm_p = ep(tc.tile_pool(name="xfm", bufs=2))      # inter_fm / co_fm
            ofm_p = ep(tc.tile_pool(name="ofm", bufs=2))      # own_fm
            out32_p = ep(tc.tile_pool(name="out32", bufs=2))  # final layer fp32 out
            wbig_p = ep(tc.tile_pool(name="wbig", bufs=1))    # [128,2048] weights
            wsm_p = ep(tc.tile_pool(name="wsm", bufs=1))      # [128,520] wv weights
            st_p = ep(tc.tile_pool(name="st", bufs=8))        # small stats
            rr_p = ep(tc.tile_pool(name="rr", bufs=4))        # recip rows / bcast
            ps_p = ep(tc.tile_pool(name="ps", bufs=8, space="PSUM"))
            dram_p = ep(tc.tile_pool(name="dram", bufs=2, space="DRAM"))

            def big_ps():
                return ps_p.tile([128, 1024], FP32, tag="big", name="big",
                                 bufs=2 if USE_DMA_TRANSPOSE else 1)

            def small_ps():
                return ps_p.tile([128, 512], FP32, tag="small", name="small", bufs=4)

            def fm_from_tm(out_fm, tm_tile, t):
                """FM[:, e*512 + t*128 + b] = TM[b, e*128 + p]: one batched
                DMA transpose per TM tile (out is a 3D strided AP whose
                (partition, e) dims cover the 512 transposed rows)."""
                if USE_DMA_TRANSPOSE:
                    out3 = out_fm[:].rearrange("p (e c) -> p e c", e=ET)
                    nc.sync.dma_start_transpose(
                        out3[:, :, t * 128:(t + 1) * 128], tm_tile[:])
                else:
                    for e in range(ET):
                        tp = ps_p.tile([128, 128], FP16, tag="tp", name="tp",
                                       bufs=2)
                        nc.tensor.transpose(tp[:], tm_tile[:, e * 128:(e + 1) * 128],
                                            identt[:])
                        nc.vector.tensor_copy(
                            out_fm[:, e * 512 + t * 128:e * 512 + (t + 1) * 128],
                            tp[:])

            identt = const_p.tile([128, 128], FP16, tag="ident", name="ident")
            nc.sync.dma_start(identt[:], ident_d[:])
            ident32t = const_p.tile([128, 128], FP16, tag="ident32",
                                    name="ident32")
            nc.sync.dma_start(ident32t[:], ident32_d[:])
            # ones-mask for the V' drain: 1.0 at each head's denominator
            # column (rel. cols 64,129,194,259 in each 260-wide half)
            vmask = const_p.tile([128, HH], FP16, tag="vmask", name="vmask")
            nc.vector.memset(vmask[:], 0.0)
            for hh in range(4):
                nc.vector.memset(vmask[:, hh * HW + D:hh * HW + D + 1], 1.0)

            knowfm = know_p.tile([128, ET * SK], FP16, tag="know", name="know")
            for e in range(ET):
                nc.sync.dma_start(knowfm[:, e * SK:(e + 1) * SK],
                                  know_fm_d[e * 128:(e + 1) * 128, :])

            hid = []
            for t in range(TT):
                h = stm_p.tile([128, E], FP16, tag="stm", name="stm")
                nc.sync.dma_start(h[:], own_tm0[t * 128:(t + 1) * 128, :])
                hid.append(h)
            ownfm = ofm_p.tile([128, ET * CH], FP16, tag="ofm", name="ofm")
            for e in range(ET):
                nc.sync.dma_start(ownfm[:, e * CH:(e + 1) * CH],
                                  own_fm0[e * 128:(e + 1) * 128, :])

            def load_w(dram, l, cols, tag, bufs=1):
                """One [128, ET*cols] tile; slice (ei, c) = [:, ei*cols+c]."""
                wt = wbig_p.tile([128, ET * cols], FP16, tag=tag, name=tag, bufs=bufs)
                for ei in range(ET):
                    nc.sync.dma_start(wt[:, ei * cols:(ei + 1) * cols], dram[l, ei])
                return wt

            def load_wv(dram, l, tag):
                """Four [128, H*HW] tiles, one per input-feature block ei."""
                wts = []
                for ei in range(ET):
                    wt = wsm_p.tile([128, H * HW], FP16, tag=tag, name=tag, bufs=4)
                    nc.sync.dma_start(wt[:], dram[l, ei])
                    wts.append(wt)
                return wts

            def q_proj(wq_t, src_fm):
                """Q_fm tiles [128, 512] from a single [128, ET*512] FM tile."""
                qs = []
                for e in range(ET):
                    ps = small_ps()
                    for ei in range(ET):
                        nc.tensor.matmul(
                            ps[:],
                            wq_t[:, ei * E + e * 128:ei * E + (e + 1) * 128],
                            src_fm[:, ei * 512:(ei + 1) * 512],
                            start=(ei == 0), stop=(ei == ET - 1))
                    qt = qfm_p.tile([128, 512], FP16, tag="qfm", name="qfm")
                    nc.vector.tensor_copy(qt[:], ps[:])
                    qs.append(qt)
                return qs

            def kv_chunk(kdst, col0, hch, wk_t):
                """K_fm columns [col0:col0+512) from one FM chunk tile."""
                bps = [big_ps(), big_ps()]
                for e in range(ET):
                    ps = bps[e // 2][:, (e % 2) * 512:(e % 2 + 1) * 512]
                    for ei in range(ET):
                        nc.tensor.matmul(
                            ps, wk_t[:, ei * E + e * 128:ei * E + (e + 1) * 128],
                            hch[:, ei * 512:(ei + 1) * 512],
                            start=(ei == 0), stop=(ei == ET - 1))
                    nc.vector.tensor_copy(kdst[e][:, col0:col0 + 512], ps)

            def v_chunk(vdst, kt0, hch, wv_ts):
                """V' token tiles kt0..kt0+3 into kt-pair tiles [*, 2*H*HW]."""
                for ktl in range(4):
                    psA = small_ps()
                    psB = small_ps()
                    for ei in range(ET):
                        lhs = hch[:, ei * 512 + ktl * 128:ei * 512 + (ktl + 1) * 128]
                        nc.tensor.matmul(psA[:, 0:HH], lhs, wv_ts[ei][:, 0:HH],
                                         start=(ei == 0), stop=(ei == ET - 1))
                        nc.tensor.matmul(psB[:, 0:HH], lhs, wv_ts[ei][:, HH:2 * HH],
                                         start=(ei == 0), stop=(ei == ET - 1))
                    kt = kt0 + ktl
                    vt = vdst[kt // 2]
                    j = kt % 2
                    vt4 = vt[:].rearrange("p (h two c) -> p h two c", h=H, two=2)
                    if j == 0 and FP8_AV:
                        nc.vector.memset(vt4[:, :, :, HW:HWP], 0.0)
                    m3 = vmask[:].rearrange("p (h c) -> p h c", h=4)
                    for half, psX in ((0, psA), (1, psB)):
                        p3 = psX[:, 0:HH].rearrange("p (h c) -> p h c", h=4)
                        nc.vector.tensor_add(
                            vt4[:, half * 4:half * 4 + 4, j, 0:HW], p3, m3)

            def attention(qfm, kfm, vp, nkt, attn_tiles):
                """vp: with FP8_AV, kt-PAIR tiles [128, 2*H*HW] fp8 (one per
                2 key tiles); AV runs one fp8 DoubleRow matmul per pair.
                Without FP8_AV, per-kt fp16 tiles as before."""
                nkp = nkt // 2
                for hs in range(2):
                    attps = [small_ps() for _ in range(4)]
                    if FP8_AV:
                        pts = {}
                        for kp in range(nkp + 1):
                            for h4 in range(4):
                                h = hs * 4 + h4
                                e, r = h // 2, (h % 2) * 64
                                if kp < nkp:
                                    sp2 = big_ps()
                                    for j in range(2):
                                        kt = kp * 2 + j
                                        nc.tensor.matmul(
                                            sp2[:, j * 512:(j + 1) * 512],
                                            kfm[e][r:r + 64,
                                                   kt * 128:(kt + 1) * 128],
                                            qfm[e][r:r + 64, :],
                                            start=True, stop=True)
                                    pt = pt_p.tile([128, 1024], FP8,
                                                   tag="pt", name="pt", bufs=8)
                                    for j in range(2):
                                        nc.scalar.activation(
                                            pt[:, j * 512:(j + 1) * 512],
                                            sp2[:, j * 512:(j + 1) * 512],
                                            AF.Exp, scale=0.125)
                                    pts[kp, h4] = pt
                                if kp >= 1:
                                    pt = pts.pop((kp - 1, h4))
                                    lhs3 = vp[kp - 1][:, h * 2 * HS:
                                                      (h + 1) * 2 * HS] \
                                        .rearrange("p (two c) -> p two c",
                                                   two=2)
                                    rhs3 = pt[:].rearrange(
                                        "p (two c) -> p two c", two=2)
                                    nc.tensor.matmul(
                                        attps[h4][0:HWP, :], lhs3, rhs3,
                                        start=(kp == 1), stop=(kp == nkp),
                                        perf_mode=PM.DoubleRow)
                    else:
                        pts = {}
                        for kt in range(nkt + 1):
                            for hp in range(2):
                                if kt < nkt:
                                    sp2 = big_ps()
                                    for j in range(2):
                                        h = hs * 4 + hp * 2 + j
                                        e, r = h // 2, (h % 2) * 64
                                        nc.tensor.matmul(
                                            sp2[:, j * 512:(j + 1) * 512],
                                            kfm[e][r:r + 64,
                                                   kt * 128:(kt + 1) * 128],
                                            qfm[e][r:r + 64, :],
                                            start=True, stop=True)
                                    pt = pt_p.tile([128, 1024], FP16,
                                                   tag="pt", name="pt", bufs=4)
                                    for j in range(2):
                                        nc.scalar.activation(
                                            pt[:, j * 512:(j + 1) * 512],
                                            sp2[:, j * 512:(j + 1) * 512],
                                            AF.Exp, scale=0.125)
                                    pts[kt, hp] = pt
                                if kt >= 1:
                                    pt = pts.pop((kt - 1, hp))
                                    for j in range(2):
                                        h = hs * 4 + hp * 2 + j
                                        c0 = h * 2 * HS + ((kt - 1) % 2) * HS
                                        nc.tensor.matmul(
                                            attps[hp * 2 + j][0:HW, :],
                                            vp[(kt - 1) // 2][:, c0:c0 + HW],
                                            pt[:, j * 512:(j + 1) * 512],
                                            start=(kt == 1), stop=(kt == nkt))
                    for h4 in range(4):
                        h = hs * 4 + h4
                        e, r = h // 2, (h % 2) * 64
                        # den must be copied to a partition-0 SBUF tile first:
                        # custom-DVE ops mishandle partition-offset PSUM reads
                        den = rr_p.tile([1, 512], FP32, tag="den", name="den",
                                        bufs=2)
                        nc.vector.tensor_copy(den[:], attps[h4][D:D + 1, :])
                        rec = rr_p.tile([1, 512], FP32, tag="rec", name="rec", bufs=2)
                        nc.vector.reciprocal_approx_fast(rec[:], den[:])
                        rb = rr_p.tile([64, 512], FP32, tag="rb", name="rb", bufs=2)
                        nc.gpsimd.partition_broadcast(rb[:], rec[:])
                        nc.vector.tensor_mul(attn_tiles[e][r:r + 64, :],
                                             attps[h4][0:64, :], rb[:])

            def ln_tile(ps, out_t):
                """out = (ps - mean)/(bessel_std + eps), LN gamma=1 beta=0.

                sqrt runs as exp(0.5*ln(v)) so the Act engine stays in the
                ln+exp table; the final scale/shift rides on Act (Copy with
                per-partition scale/bias) to keep the serial DVE chain short.
                """
                stt = st_p.tile([128, 6], FP32, tag="bnst", name="bnst")
                nc.vector.bn_stats(out=stt[:], in_=ps)
                mv = st_p.tile([128, 2], FP32, tag="bnmv", name="bnmv")
                nc.vector.bn_aggr(out=mv[:], in_=stt[:])
                lnv = st_p.tile([128, 1], FP32, tag="lnv", name="lnv")
                nc.scalar.activation(lnv[:], mv[:, 1:2], AF.Ln,
                                     scale=float(E) / (E - 1))
                sd = st_p.tile([128, 1], FP32, tag="sd", name="sd")
                nc.scalar.activation(sd[:], lnv[:], AF.Exp, scale=0.5)
                nc.vector.tensor_scalar_add(sd[:], sd[:], 1e-6)
                inv = st_p.tile([128, 1], FP32, tag="inv", name="inv")
                nc.vector.reciprocal_approx_fast(inv[:], sd[:])
                minv = st_p.tile([128, 1], FP32, tag="minv", name="minv")
                nc.vector.tensor_mul(minv[:], mv[:, 0:1], inv[:])
                nc.vector.tensor_scalar(out_t, in0=ps, scalar1=inv[:],
                                        scalar2=minv[:], op0=OP.mult,
                                        op1=OP.subtract)

            def out_ln(attn_tiles, wo_t, res_tiles, out_tm, out_fm):
                """out-proj + residual (ident matmul) + LN + DMA transpose."""
                bps = [big_ps(), big_ps()]
                pss = []
                for t in range(TT):
                    ps = bps[t // 2][:, (t % 2) * 512:(t % 2 + 1) * 512]
                    for ei in range(ET):
                        nc.tensor.matmul(
                            ps, attn_tiles[ei][:, t * 128:(t + 1) * 128],
                            wo_t[:, ei * E:(ei + 1) * E],
                            start=(ei == 0), stop=False)
                    nc.tensor.matmul(ps, identt[:], res_tiles[t][:],
                                     start=False, stop=True)
                    pss.append(ps)
                for t in range(TT):
                    ln_tile(pss[t], out_tm[t][:])
                    if out_fm is not None:
                        fm_from_tm(out_fm, out_tm[t], t)

            # --- CA K/V (uses knowfm, which is a 2-chunk FM source) ---
            def ca_kv(l, wk_t, wv_ts):
                kca = [kca_p.tile([128, SK], FP16, tag="kca", name="kca")
                       for _ in range(ET)]
                for c2 in range(2):
                    bps = [big_ps(), big_ps()]
                    for e in range(ET):
                        ps = bps[e // 2][:, (e % 2) * 512:(e % 2 + 1) * 512]
                        for ei in range(ET):
                            nc.tensor.matmul(
                                ps, wk_t[:, ei * E + e * 128:ei * E + (e + 1) * 128],
                                knowfm[:, ei * SK + c2 * 512:ei * SK + (c2 + 1) * 512],
                                start=(ei == 0), stop=(ei == ET - 1))
                        nc.vector.tensor_copy(kca[e][:, c2 * 512:(c2 + 1) * 512], ps)
                vp_ca = [vp_p.tile([128, 2 * H * HS], FP8 if FP8_AV else FP16,
                                   tag="vp", name="vp")
                         for _ in range(KT_CA // 2)]
                for kt in range(KT_CA):
                    psA = small_ps()
                    psB = small_ps()
                    for ei in range(ET):
                        lhs = knowfm[:, ei * SK + kt * 128:ei * SK + (kt + 1) * 128]
                        nc.tensor.matmul(psA[:, 0:HH], lhs, wv_ts[ei][:, 0:HH],
                                         start=(ei == 0), stop=(ei == ET - 1))
                        nc.tensor.matmul(psB[:, 0:HH], lhs,
                                         wv_ts[ei][:, HH:2 * HH],
                                         start=(ei == 0), stop=(ei == ET - 1))
                    vt = vp_ca[kt // 2]
                    j = kt % 2
                    vt4 = vt[:].rearrange("p (h two c) -> p h two c", h=H, two=2)
                    if j == 0 and FP8_AV:
                        nc.vector.memset(vt4[:, :, :, HW:HWP], 0.0)
                    m3 = vmask[:].rearrange("p (h c) -> p h c", h=4)
                    for half, psX in ((0, psA), (1, psB)):
                        p3 = psX[:, 0:HH].rearrange("p (h c) -> p h c", h=4)
                        nc.vector.tensor_add(
                            vt4[:, half * 4:half * 4 + 4, j, 0:HW], p3, m3)
                return kca, vp_ca

            # --- layer 0 prologue: weights + CA KV + own Q ---
            wq_sa_t = load_w(wq_sa, 0, E, "wq_sa")
            wk_sa_t = load_w(wk_sa, 0, E, "wk_sa")
            wv_sa_t = load_wv(wv_sa, 0, "wv_sa")
            wo_sa_t = load_w(wo_sa, 0, E, "wo_sa")
            wq_ca_t = load_w(wq_ca, 0, E, "wq_ca")
            wk_ca_t = load_w(wk_ca, 0, E, "wk_ca")
            wv_ca_t = load_wv(wv_ca, 0, "wv_ca")
            wo_ca_t = load_w(wo_ca, 0, E, "wo_ca")

            ca_state = ca_kv(0, wk_ca_t, wv_ca_t)
            qsa = q_proj(wq_sa_t, ownfm)

            ag_out_prev = None
            for l in range(L):
                with nc.named_scope(f"L{l}"):
                    kca, vp_ca = ca_state
                    # ---- SA K/V from the gathered hidden state ----
                    ksa = [kfm_p.tile([128, S], FP16, tag="kfm", name="kfm")
                           for _ in range(ET)]
                    vp_sa = [vp_p.tile([128, 2 * H * HS],
                                       FP8 if FP8_AV else FP16,
                                       tag="vp", name="vp")
                             for _ in range(KT_SA // 2)]
                    for ch in range(4):
                        hch = hch_p.tile([128, ET * 512], FP16, tag="hch",
                                         name="hch")
                        for ei in range(ET):
                            if l == 0:
                                nc.sync.dma_start(
                                    hch[:, ei * 512:(ei + 1) * 512],
                                    sen_fm[ei * 128:(ei + 1) * 128,
                                           ch * 512:(ch + 1) * 512])
                            else:
                                nc.sync.dma_start(
                                    hch[:, ei * 512:(ei + 1) * 512],
                                    ag_out_prev[ch * 512 + ei * 128:
                                                ch * 512 + (ei + 1) * 128, :])
                        kv_chunk(ksa, ch * 512, hch, wk_sa_t)
                        v_chunk(vp_sa, ch * 4, hch, wv_sa_t)

                    # prefetch next layer K/V/Q weights (rings just freed)
                    if l < L - 1:
                        wk_sa_t = load_w(wk_sa, l + 1, E, "wk_sa")
                        wv_sa_t = load_wv(wv_sa, l + 1, "wv_sa")
                        wq_sa_t = load_w(wq_sa, l + 1, E, "wq_sa")
                        wk_ca_t = load_w(wk_ca, l + 1, E, "wk_ca")
                        wv_ca_t = load_wv(wv_ca, l + 1, "wv_ca")

                    # ---- SA attention + out-proj + LN1 ----
                    attn = [attn_p.tile([128, 512], FP16, tag="attn", name="attn")
                            for _ in range(ET)]
                    attention(qsa, ksa, vp_sa, KT_SA, attn)
                    inter = [stm_p.tile([128, E], FP16, tag="stm", name="stm")
                             for _ in range(TT)]
                    interfm = xfm_p.tile([128, ET * CH], FP16, tag="xfm",
                                         name="xfm")
                    out_ln(attn, wo_sa_t, hid, inter, interfm)
                    if DEBUG_DUMPS and l == 0:
                        nc.sync.dma_start(dbg["dbg_q"][:], qsa[0][:])
                        nc.sync.dma_start(dbg["dbg_k"][:], ksa[0][:])
                        nc.sync.dma_start(dbg["dbg_v"][:], vp_sa[0][:])
                        nc.sync.dma_start(dbg["dbg_attn"][:], attn[0][:])
                        nc.sync.dma_start(dbg["dbg_inter"][:], inter[0][:])
                    if l < L - 1:
                        wo_sa_t = load_w(wo_sa, l + 1, E, "wo_sa")

                    # ---- CA Q + attention + out-proj + LN2 ----
                    qca = q_proj(wq_ca_t, interfm)
                    if l < L - 1:
                        wq_ca_t = load_w(wq_ca, l + 1, E, "wq_ca")
                    attn2 = [attn_p.tile([128, 512], FP16, tag="attn", name="attn")
                             for _ in range(ET)]
                    attention(qca, kca, vp_ca, KT_CA, attn2)
                    co = [stm_p.tile([128, E], FP16, tag="stm", name="stm")
                          for _ in range(TT)]
                    cofm = xfm_p.tile([128, ET * CH], FP16, tag="xfm",
                                      name="xfm")
                    out_ln(attn2, wo_ca_t, inter, co, cofm)
                    if DEBUG_DUMPS and l == 0:
                        nc.sync.dma_start(dbg["dbg_co"][:], co[0][:])
                    if l < L - 1:
                        wo_ca_t = load_w(wo_ca, l + 1, E, "wo_ca")

                    # ---- FFN: software-pipelined h1 -> gelu -> h2 ----
                    w1_ts = []
                    for ei in range(ET):
                        wt = wbig_p.tile([128, F], FP16, tag="w1", name="w1",
                                         bufs=4)
                        nc.sync.dma_start(wt[:], w1_d[l, ei])
                        w1_ts.append(wt)
                    w2_t = wbig_p.tile([128, FT * E], FP8 if FP8_H2 else FP16,
                                       tag="w2", name="w2", bufs=1)
                    for ft in range(FT):
                        nc.sync.dma_start(w2_t[:, ft * E:(ft + 1) * E],
                                          w2_d[l, ft])
                    h2ps = [small_ps() for _ in range(TT)]
                    gts = {}
                    for fp in range(9):
                        if fp < 8:
                            sp2 = big_ps()
                            for j in range(2):
                                ft = fp * 2 + j
                                for ei in range(ET):
                                    nc.tensor.matmul(
                                        sp2[:, j * 512:(j + 1) * 512],
                                        w1_ts[ei][:, ft * 128:(ft + 1) * 128],
                                        cofm[:, ei * 512:(ei + 1) * 512],
                                        start=(ei == 0), stop=(ei == ET - 1))
                            gt = gel_p.tile([128, 1024],
                                            FP8 if FP8_H2 else FP16,
                                            tag="gel", name="gel")
                            if FP8_H2:
                                gt4 = gt[:].rearrange(
                                    "p (t two c) -> p t two c", t=TT, two=2)
                                for j in range(2):
                                    nc.scalar.activation(
                                        gt4[:, :, j, :],
                                        sp2[:, j * 512:(j + 1) * 512]
                                        .rearrange("p (t c) -> p t c", t=TT),
                                        AF.Gelu)
                            else:
                                for j in range(2):
                                    nc.scalar.activation(
                                        gt[:, j * 512:(j + 1) * 512],
                                        sp2[:, j * 512:(j + 1) * 512], AF.Gelu)
                            gts[fp] = gt
                        if fp >= 1:
                            gt = gts.pop(fp - 1)
                            if FP8_H2:
                                w23 = w2_t[:, (fp - 1) * 2 * E:fp * 2 * E] \
                                    .rearrange("p (two c) -> p two c", two=2)
                                for t in range(TT):
                                    nc.tensor.matmul(
                                        h2ps[t][:],
                                        gt[:, t * 256:(t + 1) * 256]
                                        .rearrange("p (two c) -> p two c",
                                                   two=2),
                                        w23, start=(fp == 1), stop=False,
                                        perf_mode=PM.DoubleRow)
                            else:
                                for j in range(2):
                                    ft = (fp - 1) * 2 + j
                                    for t in range(TT):
                                        nc.tensor.matmul(
                                            h2ps[t][:],
                                            gt[:, j * 512 + t * 128:
                                               j * 512 + (t + 1) * 128],
                                            w2_t[:, ft * E:(ft + 1) * E],
                                            start=(ft == 0), stop=False)
                    for t in range(TT):
                        nc.tensor.matmul(h2ps[t][:],
                                         ident32t[:] if FP8_H2 else identt[:],
                                         co[t][:], start=False, stop=True)
                    if l == L - 1:
                        for t in range(TT):
                            o32 = out32_p.tile([128, E], FP32, tag="out32",
                                               name="out32")
                            ln_tile(h2ps[t][:], o32[:])
                            nc.sync.dma_start(out_d[t * 128:(t + 1) * 128, :],
                                              o32[:])
                    else:
                        hidn = [stm_p.tile([128, E], FP16, tag="stm", name="stm")
                                for _ in range(TT)]
                        ownfm_n = ofm_p.tile([128, ET * CH], FP16, tag="ofm",
                                             name="ofm")
                        for t in range(TT):
                            ln_tile(h2ps[t][:], hidn[t][:])
                            fm_from_tm(ownfm_n, hidn[t], t)
                        if DEBUG_DUMPS and l == 0:
                            nc.sync.dma_start(dbg["dbg_hid1"][:], hidn[0][:])
                        ag_in = dram_p.tile([CH, E], FP16, tag="agin", name="agin")
                        for e in range(ET):
                            nc.sync.dma_start(ag_in[e * 128:(e + 1) * 128, :],
                                              ownfm_n[:, e * CH:(e + 1) * CH])
                        ag_out = dram_p.tile([S, E], FP16, tag="agout",
                                             name="agout")
                        nc.gpsimd.collective_compute(
                            "AllGather", OP.bypass, replica_groups=GROUPS,
                            ins=[ag_in.opt()], outs=[ag_out.opt()])
                        # AG-independent work fills the collective latency
                        ca_state = ca_kv(l + 1, wk_ca_t, wv_ca_t)
                        qsa = q_proj(wq_sa_t, ownfm_n)
                        ag_out_prev = ag_out
                        hid = hidn

    nc.compile()
    return nc


def _prep_inputs_fast(sen, know, sa_qkv_w, sa_qkv_b, sa_out_w, sa_out_b,
                      ca_qkv_w, ca_qkv_b, ca_out_w, ca_out_b,
                      ff_w1, ff_b1, ff_w2, ff_b2, ln_g, ln_b):
    f16 = np.float16

    def rowtile(w):  # [L,E,cols] -> [L,ET,128,cols]
        return np.ascontiguousarray(w.reshape(L, ET, 128, -1).astype(f16))

    def padv(w):  # [L,E,E] -> [L,ET,128,H*HW], no bias/ones (mask adds ones)
        wp = np.zeros((L, E, H, HW), np.float32)
        wp[:, :, :, :D] = w.reshape(L, E, H, D)
        return np.ascontiguousarray(wp.reshape(L, ET, 128, H * HW).astype(f16))

    f8 = mybir.dt.np(FP8)
    common = {
        "ident": np.eye(128, dtype=f16),
        "ident32": (np.eye(128) * (W2_SCALE if FP8_H2 else 1.0)).astype(f16),
        "wq_sa": rowtile(sa_qkv_w[:, 0]), "wk_sa": rowtile(sa_qkv_w[:, 1]),
        "wv_sa": padv(sa_qkv_w[:, 2]),
        "wo_sa": rowtile(sa_out_w),
        "wq_ca": rowtile(ca_qkv_w[:, 0]), "wk_ca": rowtile(ca_qkv_w[:, 1]),
        "wv_ca": padv(ca_qkv_w[:, 2]),
        "wo_ca": rowtile(ca_out_w),
        "w1": rowtile(ff_w1),
        "w2": np.ascontiguousarray(
            (ff_w2 * W2_SCALE).reshape(L, FT, 128, E).astype(f8))
        if FP8_H2 else
        np.ascontiguousarray(ff_w2.reshape(L, FT, 128, E).astype(f16)),
    }
    in_maps = []
    for core in range(NCORES):
        g, c = core // 4, core % 4
        m = dict(common)
        m["sen_fm"] = np.ascontiguousarray(sen[g].T.astype(f16))
        m["own_fm0"] = np.ascontiguousarray(sen[g, c * CH:(c + 1) * CH].T.astype(f16))
        m["own_tm0"] = np.ascontiguousarray(sen[g, c * CH:(c + 1) * CH].astype(f16))
        m["know_fm"] = np.ascontiguousarray(know[g].T.astype(f16))
        in_maps.append(m)
    return in_maps


def _build_general():
    """Fallback for inputs with non-zero biases / non-unit LN gamma."""
    nc = bacc.Bacc("TRN2", target_bir_lowering=False, debug=False, num_devices=NCORES)

    def din(name, shape, dt=FP16):
        return nc.dram_tensor(name, shape, dt, kind="ExternalInput").ap()

    sen_fm = din("sen_fm", [E, S])
    own_fm0 = din("own_fm0", [E, CH])
    own_tm0 = din("own_tm0", [CH, E])
    know_fm_d = din("know_fm", [E, SK])
    ident_d = din("ident", [128, 128])
    ones_d = din("ones", [1, 128])

    wq_sa = din("wq_sa", [L, ET, ET, 128, 128])
    wk_sa = din("wk_sa", [L, ET, ET, 128, 128])
    wv_sa = din("wv_sa", [L, ET, 128, H * HW])
    wo_sa = din("wo_sa", [L, ET, 128, E])
    wq_ca = din("wq_ca", [L, ET, ET, 128, 128])
    wk_ca = din("wk_ca", [L, ET, ET, 128, 128])
    wv_ca = din("wv_ca", [L, ET, 128, H * HW])
    wo_ca = din("wo_ca", [L, ET, 128, E])
    w1_d = din("w1", [L, ET, FT, 128, 128])
    w2_d = din("w2", [L, FT, 128, E])

    bq_sa = din("bq_sa", [L, 128, ET], FP32)
    bk_sa = din("bk_sa", [L, 128, ET], FP32)
    bq_ca = din("bq_ca", [L, 128, ET], FP32)
    bk_ca = din("bk_ca", [L, 128, ET], FP32)
    b1_d = din("b1", [L, 128, FT], FP32)
    rbv_sa = din("rbv_sa", [L, 1, H * HW])
    rbo_sa = din("rbo_sa", [L, 1, E])
    rbv_ca = din("rbv_ca", [L, 1, H * HW])
    rbo_ca = din("rbo_ca", [L, 1, E])
    rb2_d = din("rb2", [L, 1, E])
    lng_d = din("lng", [L, 1, E], FP32)
    lnb_d = din("lnb", [L, 1, E], FP32)

    out_d = nc.dram_tensor("out_tm", [CH, E], FP32, kind="ExternalOutput").ap()

    with tile.TileContext(nc) as tc:
        from contextlib import ExitStack
        with ExitStack() as ctx:
            ep = ctx.enter_context
            const_p = ep(tc.tile_pool(name="const", bufs=1))
            know_p = ep(tc.tile_pool(name="know", bufs=4))
            kfm_p = ep(tc.tile_pool(name="kfm", bufs=4))
            kca_p = ep(tc.tile_pool(name="kca", bufs=4))
            vp_p = ep(tc.tile_pool(name="vp", bufs=27))
            hch_p = ep(tc.tile_pool(name="hch", bufs=6))
            qfm_p = ep(tc.tile_pool(name="qfm", bufs=8))
            attn_p = ep(tc.tile_pool(name="attn", bufs=4))
            ofm_p = ep(tc.tile_pool(name="ofm", bufs=8))
            xfm_p = ep(tc.tile_pool(name="xfm", bufs=5))
            stm_p = ep(tc.tile_pool(name="stm", bufs=8))
            out32_p = ep(tc.tile_pool(name="out32", bufs=2))
            pt_p = ep(tc.tile_pool(name="pt", bufs=6))
            gel_p = ep(tc.tile_pool(name="gel", bufs=17))
            wl_p = ep(tc.tile_pool(name="wl", bufs=16))
            wr_p = ep(tc.tile_pool(name="wr", bufs=6))
            row_p = ep(tc.tile_pool(name="row", bufs=4))
            gb_p = ep(tc.tile_pool(name="gb", bufs=2))
            sc_p = ep(tc.tile_pool(name="sc", bufs=3))
            s1_p = ep(tc.tile_pool(name="s1", bufs=2))
            st_p = ep(tc.tile_pool(name="st", bufs=8))
            ps_p = ep(tc.tile_pool(name="ps", bufs=8, space="PSUM"))
            dram_p = ep(tc.tile_pool(name="dram", bufs=2, space="DRAM"))

            identt = const_p.tile([128, 128], FP16, tag="ident", name="ident")
            nc.sync.dma_start(identt[:], ident_d[:])
            onest = const_p.tile([1, 128], FP16, tag="ones", name="ones")
            nc.sync.dma_start(onest[:], ones_d[:])
            knowfm = []
            for e in range(ET):
                t = know_p.tile([128, SK], FP16, tag="know", name="know")
                nc.sync.dma_start(t[:], know_fm_d[e * 128:(e + 1) * 128, :])
                knowfm.append(t)

            hid = []
            for t in range(TT):
                h = stm_p.tile([128, E], FP16, tag="stm", name="stm")
                nc.sync.dma_start(h[:], own_tm0[t * 128:(t + 1) * 128, :])
                hid.append(h)
            ownfm = []
            for e in range(ET):
                t = ofm_p.tile([128, CH], FP16, tag="ofm", name="ofm")
                nc.sync.dma_start(t[:], own_fm0[e * 128:(e + 1) * 128, :])
                ownfm.append(t)

            def ln_norm(xres, G, Bt, out):
                stt = st_p.tile([128, 6], FP32, tag="bnst", name="bnst")
                nc.vector.bn_stats(out=stt[:], in_=xres[:])
                mv = st_p.tile([128, 2], FP32, tag="bnmv", name="bnmv")
                nc.vector.bn_aggr(out=mv[:], in_=stt[:])
                sd = st_p.tile([128, 1], FP32, tag="sd", name="sd")
                nc.scalar.activation(sd[:], mv[:, 1:2], AF.Sqrt,
                                     scale=float(E) / (E - 1))
                nc.vector.tensor_scalar_add(sd[:], sd[:], 1e-6)
                inv = st_p.tile([128, 1], FP32, tag="inv", name="inv")
                nc.vector.reciprocal_approx_fast(inv[:], sd[:])
                minv = st_p.tile([128, 1], FP32, tag="minv", name="minv")
                nc.vector.tensor_mul(minv[:], mv[:, 0:1], inv[:])
                tmp = sc_p.tile([128, E], FP32, tag="lntmp", name="lntmp")
                nc.vector.tensor_scalar(tmp[:], in0=xres[:], scalar1=inv[:],
                                        scalar2=minv[:], op0=OP.mult,
                                        op1=OP.subtract)
                nc.vector.tensor_mul(tmp[:], tmp[:], G[:])
                nc.vector.tensor_add(out[:], tmp[:], Bt[:])

            def transpose_to(dst_tiles, src_tile, t):
                for e in range(ET):
                    tp = ps_p.tile([128, 128], FP16, tag="ps", name="ps")
                    nc.tensor.transpose(tp[:], src_tile[:, e * 128:(e + 1) * 128],
                                        identt[:])
                    nc.vector.tensor_copy(dst_tiles[e][:, t * 128:(t + 1) * 128],
                                          tp[:])

            def load_w16(wdram, l):
                ts = {}
                for ei in range(ET):
                    for e in range(ET):
                        wt = wl_p.tile([128, 128], FP16, tag="wl", name="wl")
                        nc.sync.dma_start(wt[:], wdram[l, ei, e])
                        ts[ei, e] = wt
                return ts

            def load_bias(bdram, l, n):
                bt = st_p.tile([128, n], FP32, tag="bias", name="bias", bufs=6)
                nc.sync.dma_start(bt[:], bdram[l])
                return bt

            def kv_proj(kdst, n_tok, src_tiles, src_col0, wk_tiles, bkt):
                nch = n_tok // 512
                for e in range(ET):
                    for c2 in range(nch):
                        pst = ps_p.tile([128, 512], FP32, tag="ps", name="ps")
                        for ei in range(ET):
                            nc.tensor.matmul(pst[:], wk_tiles[ei, e][:],
                                             src_tiles[ei][:, c2 * 512:(c2 + 1) * 512],
                                             start=(ei == 0), stop=(ei == ET - 1))
                        nc.vector.tensor_scalar_add(
                            kdst[e][:, src_col0 + c2 * 512:src_col0 + (c2 + 1) * 512],
                            pst[:], bkt[:, e:e + 1])

            def v_proj(vdst, kt0, nkt, src_tiles, wv_tiles, rbv):
                for ktl in range(nkt):
                    vt = vdst[kt0 + ktl]
                    for half in range(2):
                        pst = ps_p.tile([128, H * HW // 2], FP32, tag="ps",
                                        name="ps")
                        cs = half * (H * HW // 2)
                        for ei in range(ET):
                            nc.tensor.matmul(
                                pst[:], src_tiles[ei][:, ktl * 128:(ktl + 1) * 128],
                                wv_tiles[ei][:, cs:cs + H * HW // 2],
                                start=(ei == 0), stop=False)
                        nc.tensor.matmul(pst[:], onest[:],
                                         rbv[:, cs:cs + H * HW // 2],
                                         start=False, stop=True)
                        nc.vector.tensor_copy(vt[:, cs:cs + H * HW // 2], pst[:])

            def attention(qfm, kfm, vp, nkt, attn_tiles):
                for hs in range(2):
                    attps = [ps_p.tile([HW, 512], FP32, tag="ps", name="ps")
                             for _ in range(4)]
                    for kt in range(nkt):
                        for h4 in range(4):
                            h = hs * 4 + h4
                            e, r = h // 2, (h % 2) * 64
                            spt = ps_p.tile([128, 512], FP32, tag="ps", name="ps")
                            nc.tensor.matmul(
                                spt[:], kfm[e][r:r + 64, kt * 128:(kt + 1) * 128],
                                qfm[e][r:r + 64, :], start=True, stop=True)
                            pt = pt_p.tile([128, 512], FP16, tag="pt", name="pt")
                            nc.scalar.activation(pt[:], spt[:], AF.Exp, scale=0.125)
                            nc.tensor.matmul(attps[h4][:],
                                             vp[kt][:, h * HW:(h + 1) * HW],
                                             pt[:], start=(kt == 0),
                                             stop=(kt == nkt - 1))
                    for h4 in range(4):
                        h = hs * 4 + h4
                        e, r = h // 2, (h % 2) * 64
                        ats = sc_p.tile([64, 512], FP32, tag="ats", name="ats",
                                        bufs=4)
                        nc.scalar.activation(ats[:], attps[h4][0:64, :], AF.Copy)
                        den = s1_p.tile([1, 512], FP32, tag="den", name="den")
                        nc.vector.tensor_copy(den[:], attps[h4][64:65, :])
                        rec = s1_p.tile([1, 512], FP32, tag="rec", name="rec")
                        nc.vector.reciprocal_approx_fast(rec[:], den[:])
                        rb = sc_p.tile([64, 512], FP32, tag="rb", name="rb")
                        nc.gpsimd.partition_broadcast(rb[:], rec[:])
                        nc.vector.tensor_mul(attn_tiles[e][r:r + 64, :],
                                             ats[:], rb[:])

            def out_proj_ln(attn_tiles, wo_tiles, rbo, res_tiles, G, Bt, out_tiles):
                for t in range(TT):
                    pst = ps_p.tile([128, E], FP32, tag="ps", name="ps")
                    for ei in range(ET):
                        nc.tensor.matmul(pst[:],
                                         attn_tiles[ei][:, t * 128:(t + 1) * 128],
                                         wo_tiles[ei][:], start=(ei == 0),
                                         stop=False)
                    nc.tensor.matmul(pst[:], onest[:], rbo[:], start=False,
                                     stop=True)
                    xres = sc_p.tile([128, E], FP32, tag="xres", name="xres")
                    nc.vector.tensor_add(xres[:], pst[:], res_tiles[t][:])
                    ln_norm(xres, G, Bt, out_tiles[t])

            def make_ca_kv(l):
                kca = [kca_p.tile([128, SK], FP16, tag="kca", name="kca")
                       for _ in range(ET)]
                wkt_ca = load_w16(wk_ca, l)
                bkt_ca = load_bias(bk_ca, l, ET)
                kv_proj(kca, SK, knowfm, 0, wkt_ca, bkt_ca)
                vp_ca = [vp_p.tile([128, H * HW], FP16, tag="vp", name="vp")
                         for _ in range(KT_CA)]
                wvt_ca = []
                for ei in range(ET):
                    wt = wr_p.tile([128, H * HW], FP16, tag="wr", name="wr")
                    nc.sync.dma_start(wt[:], wv_ca[l, ei])
                    wvt_ca.append(wt)
                rbv = row_p.tile([1, H * HW], FP16, tag="row", name="row")
                nc.sync.dma_start(rbv[:], rbv_ca[l])
                v_proj(vp_ca, 0, KT_CA, knowfm, wvt_ca, rbv)
                return kca, vp_ca

            ag_out_prev = None
            ca_kv_next = None
            for l in range(L):
                with nc.named_scope(f"L{l}"):
                    if l == 0:
                        kca, vp_ca = make_ca_kv(0)
                    else:
                        kca, vp_ca = ca_kv_next
                    lr = s1_p.tile([1, E], FP32, tag="lnrow", name="lnrow")
                    nc.sync.dma_start(lr[:], lng_d[l])
                    G = gb_p.tile([128, E], FP32, tag="G", name="G")
                    nc.gpsimd.partition_broadcast(G[:], lr[:])
                    lr2 = s1_p.tile([1, E], FP32, tag="lnrow", name="lnrow")
                    nc.sync.dma_start(lr2[:], lnb_d[l])
                    Bt = gb_p.tile([128, E], FP32, tag="B", name="B")
                    nc.gpsimd.partition_broadcast(Bt[:], lr2[:])

                    ksa = [kfm_p.tile([128, S], FP16, tag="kfm", name="kfm")
                           for _ in range(ET)]
                    vp_sa = [vp_p.tile([128, H * HW], FP16, tag="vp", name="vp")
                             for _ in range(KT_SA)]
                    wkt_sa = load_w16(wk_sa, l)
                    wvt_sa = []
                    for ei in range(ET):
                        wt = wr_p.tile([128, H * HW], FP16, tag="wr", name="wr")
                        nc.sync.dma_start(wt[:], wv_sa[l, ei])
                        wvt_sa.append(wt)
                    rbvs = row_p.tile([1, H * HW], FP16, tag="row", name="row")
                    nc.sync.dma_start(rbvs[:], rbv_sa[l])
                    bkt_sa = load_bias(bk_sa, l, ET)
                    for ch in range(4):
                        hch = []
                        for ei in range(ET):
                            ht = hch_p.tile([128, 512], FP16, tag="hch", name="hch")
                            if l == 0:
                                nc.sync.dma_start(
                                    ht[:], sen_fm[ei * 128:(ei + 1) * 128,
                                                  ch * 512:(ch + 1) * 512])
                            else:
                                nc.sync.dma_start(
                                    ht[:], ag_out_prev[ch * 512 + ei * 128:
                                                       ch * 512 + (ei + 1) * 128, :])
                            hch.append(ht)
                        kv_proj(ksa, 512, hch, ch * 512, wkt_sa, bkt_sa)
                        v_proj(vp_sa, ch * 4, 4, hch, wvt_sa, rbvs)

                    if l == 0:
                        qsa = [qfm_p.tile([128, 512], FP16, tag="qfm", name="qfm")
                               for _ in range(ET)]
                        wqt_sa = load_w16(wq_sa, l)
                        bqt = load_bias(bq_sa, l, ET)
                        for e in range(ET):
                            pst = ps_p.tile([128, 512], FP32, tag="ps", name="ps")
                            for ei in range(ET):
                                nc.tensor.matmul(pst[:], wqt_sa[ei, e][:],
                                                 ownfm[ei][:],
                                                 start=(ei == 0),
                                                 stop=(ei == ET - 1))
                            nc.vector.tensor_scalar_add(qsa[e][:], pst[:],
                                                        bqt[:, e:e + 1])
                    else:
                        qsa = qsa_next

                    attn = [attn_p.tile([128, 512], FP16, tag="attn", name="attn")
                            for _ in range(ET)]
                    attention(qsa, ksa, vp_sa, KT_SA, attn)
                    wot = []
                    for ei in range(ET):
                        wt = wr_p.tile([128, E], FP16, tag="wr", name="wr")
                        nc.sync.dma_start(wt[:], wo_sa[l, ei])
                        wot.append(wt)
                    rbo = row_p.tile([1, E], FP16, tag="row", name="row")
                    nc.sync.dma_start(rbo[:], rbo_sa[l])
                    inter = [stm_p.tile([128, E], FP16, tag="stm", name="stm")
                             for _ in range(TT)]
                    out_proj_ln(attn, wot, rbo, hid, G, Bt, inter)

                    interfm = [xfm_p.tile([128, CH], FP16, tag="xfm", name="xfm")
                               for _ in range(ET)]
                    for t in range(TT):
                        transpose_to(interfm, inter[t], t)

                    qca = [qfm_p.tile([128, 512], FP16, tag="qfm", name="qfm")
                           for _ in range(ET)]
                    wqt_ca = load_w16(wq_ca, l)
                    bqt_ca = load_bias(bq_ca, l, ET)
                    for e in range(ET):
                        pst = ps_p.tile([128, 512], FP32, tag="ps", name="ps")
                        for ei in range(ET):
                            nc.tensor.matmul(pst[:], wqt_ca[ei, e][:],
                                             interfm[ei][:],
                                             start=(ei == 0), stop=(ei == ET - 1))
                        nc.vector.tensor_scalar_add(qca[e][:], pst[:],
                                                    bqt_ca[:, e:e + 1])

                    attn2 = [attn_p.tile([128, 512], FP16, tag="attn", name="attn")
                             for _ in range(ET)]
                    attention(qca, kca, vp_ca, KT_CA, attn2)
                    wot2 = []
                    for ei in range(ET):
                        wt = wr_p.tile([128, E], FP16, tag="wr", name="wr")
                        nc.sync.dma_start(wt[:], wo_ca[l, ei])
                        wot2.append(wt)
                    rbo2 = row_p.tile([1, E], FP16, tag="row", name="row")
                    nc.sync.dma_start(rbo2[:], rbo_ca[l])
                    co = [stm_p.tile([128, E], FP16, tag="stm", name="stm")
                          for _ in range(TT)]
                    out_proj_ln(attn2, wot2, rbo2, inter, G, Bt, co)

                    cofm = [xfm_p.tile([128, CH], FP16, tag="xfm", name="xfm")
                            for _ in range(ET)]
                    for t in range(TT):
                        transpose_to(cofm, co[t], t)

                    rb2 = row_p.tile([1, E], FP16, tag="row", name="row")
                    nc.sync.dma_start(rb2[:], rb2_d[l])
                    b1t = load_bias(b1_d, l, FT)
                    gel = []
                    for ft in range(FT):
                        pst = ps_p.tile([128, 512], FP32, tag="ps", name="ps")
                        for ei in range(ET):
                            wt = wl_p.tile([128, 128], FP16, tag="wl", name="wl")
                            nc.sync.dma_start(wt[:], w1_d[l, ei, ft])
                            nc.tensor.matmul(pst[:], wt[:], cofm[ei][:],
                                             start=(ei == 0), stop=(ei == ET - 1))
                        gt = gel_p.tile([128, 512], FP16, tag="gel", name="gel")
                        nc.scalar.activation(gt[:], pst[:], AF.Gelu,
                                             bias=b1t[:, ft:ft + 1])
                        gel.append(gt)
                    w2ts = []
                    for ft in range(FT):
                        w2t = wr_p.tile([128, E], FP16, tag="w2r", name="w2r",
                                        bufs=17)
                        nc.sync.dma_start(w2t[:], w2_d[l, ft])
                        w2ts.append(w2t)
                    h2ps = [ps_p.tile([128, E], FP32, tag="ps", name="ps")
                            for _ in range(TT)]
                    for t in range(TT):
                        for ft in range(FT):
                            nc.tensor.matmul(h2ps[t][:],
                                             gel[ft][:, t * 128:(t + 1) * 128],
                                             w2ts[ft][:], start=(ft == 0),
                                             stop=False)
                    if l == L - 1:
                        hidn = [out32_p.tile([128, E], FP32, tag="out32",
                                             name="out32") for _ in range(TT)]
                    else:
                        hidn = [stm_p.tile([128, E], FP16, tag="stm", name="stm")
                                for _ in range(TT)]
                    for t in range(TT):
                        nc.tensor.matmul(h2ps[t][:], onest[:], rb2[:],
                                         start=False, stop=True)
                        xres = sc_p.tile([128, E], FP32, tag="xres", name="xres")
                        nc.vector.tensor_add(xres[:], h2ps[t][:], co[t][:])
                        ln_norm(xres, G, Bt, hidn[t])
                        if l == L - 1:
                            nc.sync.dma_start(out_d[t * 128:(t + 1) * 128, :],
                                              hidn[t][:])

                    if l < L - 1:
                        ownfm_n = [ofm_p.tile([128, CH], FP16, tag="ofm",
                                              name="ofm") for _ in range(ET)]
                        for t in range(TT):
                            transpose_to(ownfm_n, hidn[t], t)
                        ag_in = dram_p.tile([CH, E], FP16, tag="agin", name="agin")
                        for e in range(ET):
                            nc.sync.dma_start(ag_in[e * 128:(e + 1) * 128, :],
                                              ownfm_n[e][:])
                        ag_out = dram_p.tile([S, E], FP16, tag="agout",
                                             name="agout")
                        nc.gpsimd.collective_compute(
                            "AllGather", OP.bypass, replica_groups=GROUPS,
                            ins=[ag_in.opt()], outs=[ag_out.opt()])
                        ca_kv_next = make_ca_kv(l + 1)
                        qsa_next = [qfm_p.tile([128, 512], FP16, tag="qfm",
                                               name="qfm") for _ in range(ET)]
                        wqt_n = load_w16(wq_sa, l + 1)
                        bqt_n = load_bias(bq_sa, l + 1, ET)
                        for e in range(ET):
                            pst = ps_p.tile([128, 512], FP32, tag="ps", name="ps")
                            for ei in range(ET):
                                nc.tensor.matmul(pst[:], wqt_n[ei, e][:],
                                                 ownfm_n[ei][:],
                                                 start=(ei == 0),
                                                 stop=(ei == ET - 1))
                            nc.vector.tensor_scalar_add(qsa_next[e][:], pst[:],
                                                        bqt_n[:, e:e + 1])
                        ag_out_prev = ag_out
                        ownfm = ownfm_n
                        hid = hidn

    nc.compile()
    return nc


def _prep_inputs(sen, know, sa_qkv_w, sa_qkv_b, sa_out_w, sa_out_b,
                 ca_qkv_w, ca_qkv_b, ca_out_w, ca_out_b,
                 ff_w1, ff_b1, ff_w2, ff_b2, ln_g, ln_b):
    """Host-side weight packing for the general fallback kernel."""
    f16, f32 = np.float16, np.float32

    def tile4(w):
        return np.ascontiguousarray(
            w.reshape(L, ET, 128, ET, 128).transpose(0, 1, 3, 2, 4).astype(f16))

    def padv(w, b):
        wp = np.zeros((L, E, H, HW), f32)
        wp[:, :, :, :D] = w.reshape(L, E, H, D)
        bp = np.zeros((L, H, HW), f32)
        bp[:, :, :D] = b.reshape(L, H, D)
        bp[:, :, D] = 1.0
        return (np.ascontiguousarray(wp.reshape(L, ET, 128, H * HW).astype(f16)),
                np.ascontiguousarray(bp.reshape(L, 1, H * HW).astype(f16)))

    wv_sa_p, rbv_sa_h = padv(sa_qkv_w[:, 2], sa_qkv_b[:, 2])
    wv_ca_p, rbv_ca_h = padv(ca_qkv_w[:, 2], ca_qkv_b[:, 2])

    common = {
        "ident": np.eye(128, dtype=f16),
        "ones": np.ones((1, 128), f16),
        "wq_sa": tile4(sa_qkv_w[:, 0]), "wk_sa": tile4(sa_qkv_w[:, 1]),
        "wv_sa": wv_sa_p,
        "wo_sa": np.ascontiguousarray(sa_out_w.reshape(L, ET, 128, E).astype(f16)),
        "wq_ca": tile4(ca_qkv_w[:, 0]), "wk_ca": tile4(ca_qkv_w[:, 1]),
        "wv_ca": wv_ca_p,
        "wo_ca": np.ascontiguousarray(ca_out_w.reshape(L, ET, 128, E).astype(f16)),
        "w1": np.ascontiguousarray(
            ff_w1.reshape(L, ET, 128, FT, 128).transpose(0, 1, 3, 2, 4).astype(f16)),
        "w2": np.ascontiguousarray(ff_w2.reshape(L, FT, 128, E).astype(f16)),
        "bq_sa": np.ascontiguousarray(
            sa_qkv_b[:, 0].reshape(L, ET, 128).transpose(0, 2, 1)),
        "bk_sa": np.ascontiguousarray(
            sa_qkv_b[:, 1].reshape(L, ET, 128).transpose(0, 2, 1)),
        "bq_ca": np.ascontiguousarray(
            ca_qkv_b[:, 0].reshape(L, ET, 128).transpose(0, 2, 1)),
        "bk_ca": np.ascontiguousarray(
            ca_qkv_b[:, 1].reshape(L, ET, 128).transpose(0, 2, 1)),
        "b1": np.ascontiguousarray(
            ff_b1.reshape(L, FT, 128).transpose(0, 2, 1)),
        "rbv_sa": rbv_sa_h, "rbv_ca": rbv_ca_h,
        "rbo_sa": np.ascontiguousarray(sa_out_b[:, None, :].astype(f16)),
        "rbo_ca": np.ascontiguousarray(ca_out_b[:, None, :].astype(f16)),
        "rb2": np.ascontiguousarray(ff_b2[:, None, :].astype(f16)),
        "lng": np.ascontiguousarray(ln_g[:, None, :]),
        "lnb": np.ascontiguousarray(ln_b[:, None, :]),
    }
    in_maps = []
    for core in range(NCORES):
        g, c = core // 4, core % 4
        m = dict(common)
        m["sen_fm"] = np.ascontiguousarray(sen[g].T.astype(f16))
        m["own_fm0"] = np.ascontiguousarray(sen[g, c * CH:(c + 1) * CH].T.astype(f16))
        m["own_tm0"] = np.ascontiguousarray(sen[g, c * CH:(c + 1) * CH].astype(f16))
        m["know_fm"] = np.ascontiguousarray(know[g].T.astype(f16))
        in_maps.append(m)
    return in_maps


def _inputs_are_fast(sa_qkv_b, sa_out_b, ca_qkv_b, ca_out_b,
                     ff_b1, ff_b2, ln_g, ln_b, **_):
    zeros = [sa_qkv_b, sa_out_b, ca_qkv_b, ca_out_b, ff_b1, ff_b2, ln_b]
    return all(not np.any(z) for z in zeros) and np.all(ln_g == 1.0)


def kernel(**inputs):
    inputs = {k: np.asarray(v, dtype=np.float32) for k, v in inputs.items()}
    if _inputs_are_fast(**inputs):
        if "nc" not in _CACHE:
            _CACHE["nc"] = _build_fast()
        nc = _CACHE["nc"]
        in_maps = _prep_inputs_fast(**inputs)
    else:
        if "nc_gen" not in _CACHE:
            _CACHE["nc_gen"] = _build_general()
        nc = _CACHE["nc_gen"]
        in_maps = _prep_inputs(**inputs)
        _CACHE["nc"] = nc
    res = run_bass_kernel_spmd(nc, in_maps, list(range(NCORES)))
    out = np.empty((B, S, E), np.float32)
    for core in range(NCORES):
        g, c = core // 4, core % 4
        out[g, c * CH:(c + 1) * CH] = res.results[core]["out_tm"]
    return out


# revision 57
# speedup vs baseline: 1.1090x; 1.0085x over previous
"""Trainium2 Bass kernel for a 4-layer hierarchical-attention encoder.

Sharding: 8 cores = 2 batch groups x 4 sequence chunks of 512 query tokens.
Each core runs the full layer stack for its 512 tokens; the hidden state is
all-gathered (per batch group) at each layer boundary so every core can
compute full-sequence self-attention K/V locally.

Fast path (inputs with zero biases, unit LN gamma, zero LN beta — which is
what setup_inputs() produces):
 - no bias matmul rows; K-bias is dropped (exactly free under softmax)
 - residual adds ride on the PE via identity matmuls into the out-proj PSUM
 - LN sqrt computed as exp(0.5*ln(v)) so the Act engine never leaves the
   exp table on the critical path
 - exp/gelu processed on 2-bank (1024-wide) PSUM regions
 - all TM->FM transposes on the DMA transpose engine (PE/DVE freed)
 - attention denominator via a ones-mask added during the V' PSUM drain
 - batched weight DMA layouts ([128, E]-row tiles)
A general fallback (the previous kernel) handles arbitrary bias/gamma.
"""
import os
import sys

for _p in ("/root/.axon_site/_ro/trn_rl_repo", "/opt/trn_rl_repo", "/opt/pypackages",
           "/root/.axon_site/_ro/pypackages"):
    if os.path.isdir(_p) and _p not in sys.path:
        sys.path.append(_p)

import numpy as np

import concourse.bass as bass
import concourse.mybir as mybir
import concourse.tile as tile
from concourse import bacc
from concourse.bass_utils import run_bass_kernel_spmd

L, E, H, D, F = 4, 512, 8, 64, 2048
B, S, SK = 2, 2048, 1024
NCORES = 8
GROUPS = [[0, 1, 2, 3], [4, 5, 6, 7]]
CH = 512          # tokens per core
ET = E // 128     # 4 feature tiles
TT = CH // 128    # 4 token tiles in own chunk
FT = F // 128     # 16 ffn tiles
KT_SA = S // 128  # 16 key tiles (self)
KT_CA = SK // 128  # 8 key tiles (cross)
HW = 65           # head width incl. denominator column
HWP = 80          # fp8 DoubleRow head stride (M%16==0 requirement)

FP32 = mybir.dt.float32
FP16 = mybir.dt.float16
FP8 = mybir.dt.float8e4
AF = mybir.ActivationFunctionType
OP = mybir.AluOpType
PM = mybir.MatmulPerfMode
W2_SCALE = 32.0  # fp8 w2 pre-scale; exact under LN's scale invariance

_CACHE = {}

# Feature toggles for HW bring-up bisection.
# EXP_2BANK stays off: a single Act instruction reading a PSUM access
# pattern that crosses a bank boundary wedges the exec unit on HW.
USE_DMA_TRANSPOSE = os.environ.get("K_DMA_T", "1") == "1"
EXP_2BANK = os.environ.get("K_EXP2", "0") == "1"
DEBUG_DUMPS = os.environ.get("K_DEBUG", "0") == "1"
FP8_AV = os.environ.get("K_FP8AV", "0") == "1"
FP8_H2 = os.environ.get("K_FP8H2", "0") == "1"
HS = HWP if FP8_AV else HW  # per-(head,pair-slot) stride in V tiles


def _patch_act_tables():
    """Steer the act-table-load pass away from the exp-less `natural_log`
    table so Ln resolves to `natural_log_exp_and_others` and the LN
    ln/exp pair never swaps tables against the attention exps.

    Only the bass-side selector sees the emptied entry; table ids and the
    hardware table contents (walrus reads act_info.json directly) are
    unchanged, so every emitted load remains valid.
    """
    import concourse.bacc as bacc_mod
    if getattr(bacc_mod, "_ln_exp_patched", False):
        return
    orig = bacc_mod.get_activation_tables

    def patched(arch):
        tables = dict(orig(arch))
        shared = "natural_log_exp_and_others"
        if shared in tables and {mybir.ActivationFunctionType.Exp,
                                 mybir.ActivationFunctionType.Ln} <= tables[shared]:
            for name, fns in tables.items():
                if name != shared:
                    tables[name] = fns - {mybir.ActivationFunctionType.Exp,
                                          mybir.ActivationFunctionType.Ln}
        return tables

    bacc_mod.get_activation_tables = patched
    bacc_mod._ln_exp_patched = True


def _build_fast():
    _patch_act_tables()
    nc = bacc.Bacc("TRN2", target_bir_lowering=False, debug=False, num_devices=NCORES)

    def din(name, shape, dt=FP16):
        return nc.dram_tensor(name, shape, dt, kind="ExternalInput").ap()

    sen_fm = din("sen_fm", [E, S])            # full batch sequence, feature-major
    own_fm0 = din("own_fm0", [E, CH])         # own chunk, feature-major
    own_tm0 = din("own_tm0", [CH, E])         # own chunk, token-major
    know_fm_d = din("know_fm", [E, SK])
    ident_d = din("ident", [128, 128])
    ident32_d = din("ident32", [128, 128])

    wq_sa = din("wq_sa", [L, ET, 128, E])
    wk_sa = din("wk_sa", [L, ET, 128, E])
    wv_sa = din("wv_sa", [L, ET, 128, H * HW])
    wo_sa = din("wo_sa", [L, ET, 128, E])
    wq_ca = din("wq_ca", [L, ET, 128, E])
    wk_ca = din("wk_ca", [L, ET, 128, E])
    wv_ca = din("wv_ca", [L, ET, 128, H * HW])
    wo_ca = din("wo_ca", [L, ET, 128, E])
    w1_d = din("w1", [L, ET, 128, F])
    w2_d = din("w2", [L, FT, 128, E], FP8 if FP8_H2 else FP16)

    out_d = nc.dram_tensor("out_tm", [CH, E], FP32, kind="ExternalOutput").ap()
    dbg = {}
    if DEBUG_DUMPS:
        for nm, shape in [("dbg_q", [128, 512]), ("dbg_k", [128, S]),
                          ("dbg_v", [128, H * HW]), ("dbg_attn", [128, 512]),
                          ("dbg_inter", [128, E]), ("dbg_co", [128, E]),
                          ("dbg_hid1", [128, E])]:
            dbg[nm] = nc.dram_tensor(nm, shape, FP16,
                                     kind="ExternalOutput").ap()

    HH = H * HW // 2  # 260, half of the padded V width

    with tile.TileContext(nc) as tc:
        from contextlib import ExitStack
        with ExitStack() as ctx:
            ep = ctx.enter_context
            const_p = ep(tc.tile_pool(name="const", bufs=1))
            know_p = ep(tc.tile_pool(name="know", bufs=1))    # [128,4096] know FM
            kfm_p = ep(tc.tile_pool(name="kfm", bufs=4))      # [128,2048] SA K fp16
            kca_p = ep(tc.tile_pool(name="kca", bufs=4))      # [128,1024] CA K fp16
            vp_p = ep(tc.tile_pool(name="vp", bufs=12))       # resident V' pairs
            hch_p = ep(tc.tile_pool(name="hch", bufs=2))      # [128,2048] H_fm chunk
            qfm_p = ep(tc.tile_pool(name="qfm", bufs=8))
            attn_p = ep(tc.tile_pool(name="attn", bufs=8))
            pt_p = ep(tc.tile_pool(name="pt", bufs=4))        # exp out [128,1024] fp16
            gel_p = ep(tc.tile_pool(name="gel", bufs=3))      # [128,1024] fp16
            stm_p = ep(tc.tile_pool(name="stm", bufs=12))     # hid/inter/co TM fp16
            xfm_p = ep(tc.tile_pool(name="xfm", bufs=2))      # inter_fm / co_fm
            ofm_p = ep(tc.tile_pool(name="ofm", bufs=2))      # own_fm
            out32_p = ep(tc.tile_pool(name="out32", bufs=2))  # final layer fp32 out
            wbig_p = ep(tc.tile_pool(name="wbig", bufs=1))    # [128,2048] weights
            wsm_p = ep(tc.tile_pool(name="wsm", bufs=1))      # [128,520] wv weights
            st_p = ep(tc.tile_pool(name="st", bufs=8))        # small stats
            rr_p = ep(tc.tile_pool(name="rr", bufs=4))        # recip rows / bcast
            ps_p = ep(tc.tile_pool(name="ps", bufs=8, space="PSUM"))
            dram_p = ep(tc.tile_pool(name="dram", bufs=2, space="DRAM"))

            def big_ps():
                return ps_p.tile([128, 1024], FP32, tag="big", name="big",
                                 bufs=2 if USE_DMA_TRANSPOSE else 1)

            def small_ps():
                return ps_p.tile([128, 512], FP32, tag="small", name="small", bufs=4)

            def fm_from_tm(out_fm, tm_tile, t):
                """FM[:, e*512 + t*128 + b] = TM[b, e*128 + p]: one batched
                DMA transpose per TM tile (out is a 3D strided AP whose
                (partition, e) dims cover the 512 transposed rows)."""
                if USE_DMA_TRANSPOSE:
                    out3 = out_fm[:].rearrange("p (e c) -> p e c", e=ET)
                    nc.sync.dma_start_transpose(
                        out3[:, :, t * 128:(t + 1) * 128], tm_tile[:])
                else:
                    for e in range(ET):
                        tp = ps_p.tile([128, 128], FP16, tag="tp", name="tp",
                                       bufs=2)
                        nc.tensor.transpose(tp[:], tm_tile[:, e * 128:(e + 1) * 128],
                                            identt[:])
                        nc.vector.tensor_copy(
                            out_fm[:, e * 512 + t * 128:e * 512 + (t + 1) * 128],
                            tp[:])

            identt = const_p.tile([128, 128], FP16, tag="ident", name="ident")
            nc.sync.dma_start(identt[:], ident_d[:])
            ident32t = const_p.tile([128, 128], FP16, tag="ident32",
                                    name="ident32")
            nc.sync.dma_start(ident32t[:], ident32_d[:])
            # ones-mask for the V' drain: 1.0 at each head's denominator
            # column (rel. cols 64,129,194,259 in each 260-wide half)
            vmask = const_p.tile([128, HH], FP16, tag="vmask", name="vmask")
            nc.vector.memset(vmask[:], 0.0)
            for hh in range(4):
                nc.vector.memset(vmask[:, hh * HW + D:hh * HW + D + 1], 1.0)

            knowfm = know_p.tile([128, ET * SK], FP16, tag="know", name="know")
            for e in range(ET):
                nc.sync.dma_start(knowfm[:, e * SK:(e + 1) * SK],
                                  know_fm_d[e * 128:(e + 1) * 128, :])

            hid = []
            for t in range(TT):
                h = stm_p.tile([128, E], FP16, tag="stm", name="stm")
                nc.sync.dma_start(h[:], own_tm0[t * 128:(t + 1) * 128, :])
                hid.append(h)
            ownfm = ofm_p.tile([128, ET * CH], FP16, tag="ofm", name="ofm")
            for e in range(ET):
                nc.sync.dma_start(ownfm[:, e * CH:(e + 1) * CH],
                                  own_fm0[e * 128:(e + 1) * 128, :])

            def load_w(dram, l, cols, tag, bufs=1):
                """One [128, ET*cols] tile; slice (ei, c) = [:, ei*cols+c]."""
                wt = wbig_p.tile([128, ET * cols], FP16, tag=tag, name=tag, bufs=bufs)
                for ei in range(ET):
                    nc.sync.dma_start(wt[:, ei * cols:(ei + 1) * cols], dram[l, ei])
                return wt

            def load_wv(dram, l, tag):
                """Four [128, H*HW] tiles, one per input-feature block ei."""
                wts = []
                for ei in range(ET):
                    wt = wsm_p.tile([128, H * HW], FP16, tag=tag, name=tag, bufs=4)
                    nc.sync.dma_start(wt[:], dram[l, ei])
                    wts.append(wt)
                return wts

            def q_proj(wq_t, src_fm):
                """Q_fm tiles [128, 512] from a single [128, ET*512] FM tile."""
                qs = []
                for e in range(ET):
                    ps = small_ps()
                    for ei in range(ET):
                        nc.tensor.matmul(
                            ps[:],
                            wq_t[:, ei * E + e * 128:ei * E + (e + 1) * 128],
                            src_fm[:, ei * 512:(ei + 1) * 512],
                            start=(ei == 0), stop=(ei == ET - 1))
                    qt = qfm_p.tile([128, 512], FP16, tag="qfm", name="qfm")
                    nc.vector.tensor_copy(qt[:], ps[:])
                    qs.append(qt)
                return qs

            def kv_chunk(kdst, col0, hch, wk_t):
                """K_fm columns [col0:col0+512) from one FM chunk tile."""
                bps = [big_ps(), big_ps()]
                for e in range(ET):
                    ps = bps[e // 2][:, (e % 2) * 512:(e % 2 + 1) * 512]
                    for ei in range(ET):
                        nc.tensor.matmul(
                            ps, wk_t[:, ei * E + e * 128:ei * E + (e + 1) * 128],
                            hch[:, ei * 512:(ei + 1) * 512],
                            start=(ei == 0), stop=(ei == ET - 1))
                    nc.vector.tensor_copy(kdst[e][:, col0:col0 + 512], ps)

            def v_chunk(vdst, kt0, hch, wv_ts):
                """V' token tiles kt0..kt0+3 into kt-pair tiles [*, 2*H*HW]."""
                for ktl in range(4):
                    psA = small_ps()
                    psB = small_ps()
                    for ei in range(ET):
                        lhs = hch[:, ei * 512 + ktl * 128:ei * 512 + (ktl + 1) * 128]
                        nc.tensor.matmul(psA[:, 0:HH], lhs, wv_ts[ei][:, 0:HH],
                                         start=(ei == 0), stop=(ei == ET - 1))
                        nc.tensor.matmul(psB[:, 0:HH], lhs, wv_ts[ei][:, HH:2 * HH],
                                         start=(ei == 0), stop=(ei == ET - 1))
                    kt = kt0 + ktl
                    vt = vdst[kt // 2]
                    j = kt % 2
                    vt4 = vt[:].rearrange("p (h two c) -> p h two c", h=H, two=2)
                    if j == 0 and FP8_AV:
                        nc.vector.memset(vt4[:, :, :, HW:HWP], 0.0)
                    m3 = vmask[:].rearrange("p (h c) -> p h c", h=4)
                    for half, psX in ((0, psA), (1, psB)):
                        p3 = psX[:, 0:HH].rearrange("p (h c) -> p h c", h=4)
                        nc.vector.tensor_add(
                            vt4[:, half * 4:half * 4 + 4, j, 0:HW], p3, m3)

            def attention(qfm, kfm, vp, nkt, attn_tiles):
                """vp: with FP8_AV, kt-PAIR tiles [128, 2*H*HW] fp8 (one per
                2 key tiles); AV runs one fp8 DoubleRow matmul per pair.
                Without FP8_AV, per-kt fp16 tiles as before."""
                nkp = nkt // 2
                for hs in range(2):
                    attps = [small_ps() for _ in range(4)]
                    if FP8_AV:
                        pts = {}
                        for kp in range(nkp + 1):
                            for h4 in range(4):
                                h = hs * 4 + h4
                                e, r = h // 2, (h % 2) * 64
                                if kp < nkp:
                                    sp2 = big_ps()
                                    for j in range(2):
                                        kt = kp * 2 + j
                                        nc.tensor.matmul(
                                            sp2[:, j * 512:(j + 1) * 512],
                                            kfm[e][r:r + 64,
                                                   kt * 128:(kt + 1) * 128],
                                            qfm[e][r:r + 64, :],
                                            start=True, stop=True)
                                    pt = pt_p.tile([128, 1024], FP8,
                                                   tag="pt", name="pt", bufs=8)
                                    for j in range(2):
                                        nc.scalar.activation(
                                            pt[:, j * 512:(j + 1) * 512],
                                            sp2[:, j * 512:(j + 1) * 512],
                                            AF.Exp, scale=0.125)
                                    pts[kp, h4] = pt
                                if kp >= 1:
                                    pt = pts.pop((kp - 1, h4))
                                    lhs3 = vp[kp - 1][:, h * 2 * HS:
                                                      (h + 1) * 2 * HS] \
                                        .rearrange("p (two c) -> p two c",
                                                   two=2)
                                    rhs3 = pt[:].rearrange(
                                        "p (two c) -> p two c", two=2)
                                    nc.tensor.matmul(
                                        attps[h4][0:HWP, :], lhs3, rhs3,
                                        start=(kp == 1), stop=(kp == nkp),
                                        perf_mode=PM.DoubleRow)
                    else:
                        pts = {}
                        for kt in range(nkt + 1):
                            for hp in range(2):
                                if kt < nkt:
                                    sp2 = big_ps()
                                    for j in range(2):
                                        h = hs * 4 + hp * 2 + j
                                        e, r = h // 2, (h % 2) * 64
                                        nc.tensor.matmul(
                                            sp2[:, j * 512:(j + 1) * 512],
                                            kfm[e][r:r + 64,
                                                   kt * 128:(kt + 1) * 128],
                                            qfm[e][r:r + 64, :],
                                            start=True, stop=True)
                                    pt = pt_p.tile([128, 1024], FP16,
                                                   tag="pt", name="pt", bufs=4)
                                    for j in range(2):
                                        nc.scalar.activation(
                                            pt[:, j * 512:(j + 1) * 512],
                                            sp2[:, j * 512:(j + 1) * 512],
                                            AF.Exp, scale=0.125)
                                    pts[kt, hp] = pt
                                if kt >= 1:
                                    pt = pts.pop((kt - 1, hp))
                                    for j in range(2):
                                        h = hs * 4 + hp * 2 + j
                                        c0 = h * 2 * HS + ((kt - 1) % 2) * HS
                                        nc.tensor.matmul(
                                            attps[hp * 2 + j][0:HW, :],
                                            vp[(kt - 1) // 2][:, c0:c0 + HW],
                                            pt[:, j * 512:(j + 1) * 512],
                                            start=(kt == 1), stop=(kt == nkt))
                    for h4 in range(4):
                        h = hs * 4 + h4
                        e, r = h // 2, (h % 2) * 64
                        # den must be copied to a partition-0 SBUF tile first:
                        # custom-DVE ops mishandle partition-offset PSUM reads
                        den = rr_p.tile([1, 512], FP32, tag="den", name="den",
                                        bufs=2)
                        nc.vector.tensor_copy(den[:], attps[h4][D:D + 1, :])
                        rec = rr_p.tile([1, 512], FP32, tag="rec", name="rec", bufs=2)
                        nc.vector.reciprocal_approx_fast(rec[:], den[:])
                        rb = rr_p.tile([64, 512], FP32, tag="rb", name="rb", bufs=2)
                        nc.gpsimd.partition_broadcast(rb[:], rec[:])
                        nc.vector.tensor_mul(attn_tiles[e][r:r + 64, :],
                                             attps[h4][0:64, :], rb[:])

            def ln_tile(ps, out_t):
                """out = (ps - mean)/(bessel_std + eps), LN gamma=1 beta=0.

                sqrt runs as exp(0.5*ln(v)) so the Act engine stays in the
                ln+exp table; the final scale/shift rides on Act (Copy with
                per-partition scale/bias) to keep the serial DVE chain short.
                """
                stt = st_p.tile([128, 6], FP32, tag="bnst", name="bnst")
                nc.vector.bn_stats(out=stt[:], in_=ps)
                mv = st_p.tile([128, 2], FP32, tag="bnmv", name="bnmv")
                nc.vector.bn_aggr(out=mv[:], in_=stt[:])
                lnv = st_p.tile([128, 1], FP32, tag="lnv", name="lnv")
                nc.scalar.activation(lnv[:], mv[:, 1:2], AF.Ln,
                                     scale=float(E) / (E - 1))
                sd = st_p.tile([128, 1], FP32, tag="sd", name="sd")
                nc.scalar.activation(sd[:], lnv[:], AF.Exp, scale=0.5)
                nc.vector.tensor_scalar_add(sd[:], sd[:], 1e-6)
                inv = st_p.tile([128, 1], FP32, tag="inv", name="inv")
                nc.vector.reciprocal_approx_fast(inv[:], sd[:])
                minv = st_p.tile([128, 1], FP32, tag="minv", name="minv")
                nc.vector.tensor_mul(minv[:], mv[:, 0:1], inv[:])
                nc.vector.tensor_scalar(out_t, in0=ps, scalar1=inv[:],
                                        scalar2=minv[:], op0=OP.mult,
                                        op1=OP.subtract)

            def out_ln(attn_tiles, wo_t, res_tiles, out_tm, out_fm):
                """out-proj + residual (ident matmul) + LN + DMA transpose."""
                bps = [big_ps(), big_ps()]
                pss = []
                for t in range(TT):
                    ps = bps[t // 2][:, (t % 2) * 512:(t % 2 + 1) * 512]
                    for ei in range(ET):
                        nc.tensor.matmul(
                            ps, attn_tiles[ei][:, t * 128:(t + 1) * 128],
                            wo_t[:, ei * E:(ei + 1) * E],
                            start=(ei == 0), stop=False)
                    nc.tensor.matmul(ps, identt[:], res_tiles[t][:],
                                     start=False, stop=True)
                    pss.append(ps)
                for t in range(TT):
                    ln_tile(pss[t], out_tm[t][:])
                    if out_fm is not None:
                        fm_from_tm(out_fm, out_tm[t], t)

            # --- CA K/V (uses knowfm, which is a 2-chunk FM source) ---
            def ca_kv(l, wk_t, wv_ts):
                kca = [kca_p.tile([128, SK], FP16, tag="kca", name="kca")
                       for _ in range(ET)]
                for c2 in range(2):
                    bps = [big_ps(), big_ps()]
                    for e in range(ET):
                        ps = bps[e // 2][:, (e % 2) * 512:(e % 2 + 1) * 512]
                        for ei in range(ET):
                            nc.tensor.matmul(
                                ps, wk_t[:, ei * E + e * 128:ei * E + (e + 1) * 128],
                                knowfm[:, ei * SK + c2 * 512:ei * SK + (c2 + 1) * 512],
                                start=(ei == 0), stop=(ei == ET - 1))
                        nc.vector.tensor_copy(kca[e][:, c2 * 512:(c2 + 1) * 512], ps)
                vp_ca = [vp_p.tile([128, 2 * H * HS], FP8 if FP8_AV else FP16,
                                   tag="vp", name="vp")
                         for _ in range(KT_CA // 2)]
                for kt in range(KT_CA):
                    psA = small_ps()
                    psB = small_ps()
                    for ei in range(ET):
                        lhs = knowfm[:, ei * SK + kt * 128:ei * SK + (kt + 1) * 128]
                        nc.tensor.matmul(psA[:, 0:HH], lhs, wv_ts[ei][:, 0:HH],
                                         start=(ei == 0), stop=(ei == ET - 1))
                        nc.tensor.matmul(psB[:, 0:HH], lhs,
                                         wv_ts[ei][:, HH:2 * HH],
                                         start=(ei == 0), stop=(ei == ET - 1))
                    vt = vp_ca[kt // 2]
                    j = kt % 2
                    vt4 = vt[:].rearrange("p (h two c) -> p h two c", h=H, two=2)
                    if j == 0 and FP8_AV:
                        nc.vector.memset(vt4[:, :, :, HW:HWP], 0.0)
                    m3 = vmask[:].rearrange("p (h c) -> p h c", h=4)
                    for half, psX in ((0, psA), (1, psB)):
                        p3 = psX[:, 0:HH].rearrange("p (h c) -> p h c", h=4)
                        nc.vector.tensor_add(
                            vt4[:, half * 4:half * 4 + 4, j, 0:HW], p3, m3)
                return kca, vp_ca

            # --- layer 0 prologue: weights + CA KV + own Q ---
            wq_sa_t = load_w(wq_sa, 0, E, "wq_sa")
            wk_sa_t = load_w(wk_sa, 0, E, "wk_sa")
            wv_sa_t = load_wv(wv_sa, 0, "wv_sa")
            wo_sa_t = load_w(wo_sa, 0, E, "wo_sa")
            wq_ca_t = load_w(wq_ca, 0, E, "wq_ca")
            wk_ca_t = load_w(wk_ca, 0, E, "wk_ca")
            wv_ca_t = load_wv(wv_ca, 0, "wv_ca")
            wo_ca_t = load_w(wo_ca, 0, E, "wo_ca")

            ca_state = ca_kv(0, wk_ca_t, wv_ca_t)
            qsa = q_proj(wq_sa_t, ownfm)

            ag_out_prev = None
            for l in range(L):
                with nc.named_scope(f"L{l}"):
                    kca, vp_ca = ca_state
                    # ---- SA K/V from the gathered hidden state ----
                    ksa = [kfm_p.tile([128, S], FP16, tag="kfm", name="kfm")
                           for _ in range(ET)]
                    vp_sa = [vp_p.tile([128, 2 * H * HS],
                                       FP8 if FP8_AV else FP16,
                                       tag="vp", name="vp")
                             for _ in range(KT_SA // 2)]
                    for ch in range(4):
                        hch = hch_p.tile([128, ET * 512], FP16, tag="hch",
                                         name="hch")
                        for ei in range(ET):
                            if l == 0:
                                nc.sync.dma_start(
                                    hch[:, ei * 512:(ei + 1) * 512],
                                    sen_fm[ei * 128:(ei + 1) * 128,
                                           ch * 512:(ch + 1) * 512])
                            else:
                                nc.sync.dma_start(
                                    hch[:, ei * 512:(ei + 1) * 512],
                                    ag_out_prev[ch * 512 + ei * 128:
                                                ch * 512 + (ei + 1) * 128, :])
                        kv_chunk(ksa, ch * 512, hch, wk_sa_t)
                        v_chunk(vp_sa, ch * 4, hch, wv_sa_t)

                    # this layer's FFN weights: load now while SP is idle
                    # (issuing them at the FFN stalls h1 behind the LN2-site
                    # transposes on the in-order SP queue)
                    w1_ts = []
                    for ei in range(ET):
                        wt = wbig_p.tile([128, F], FP16, tag="w1", name="w1",
                                         bufs=4)
                        nc.sync.dma_start(wt[:], w1_d[l, ei])
                        w1_ts.append(wt)
                    w2_t = wbig_p.tile([128, FT * E], FP8 if FP8_H2 else FP16,
                                       tag="w2", name="w2", bufs=1)
                    for ft in range(FT):
                        nc.sync.dma_start(w2_t[:, ft * E:(ft + 1) * E],
                                          w2_d[l, ft])
                    # prefetch next layer K/V/Q weights (rings just freed)
                    if l < L - 1:
                        wk_sa_t = load_w(wk_sa, l + 1, E, "wk_sa")
                        wv_sa_t = load_wv(wv_sa, l + 1, "wv_sa")
                        wq_sa_t = load_w(wq_sa, l + 1, E, "wq_sa")
                        wk_ca_t = load_w(wk_ca, l + 1, E, "wk_ca")
                        wv_ca_t = load_wv(wv_ca, l + 1, "wv_ca")

                    # ---- SA attention + out-proj + LN1 ----
                    attn = [attn_p.tile([128, 512], FP16, tag="attn", name="attn")
                            for _ in range(ET)]
                    attention(qsa, ksa, vp_sa, KT_SA, attn)
                    inter = [stm_p.tile([128, E], FP16, tag="stm", name="stm")
                             for _ in range(TT)]
                    interfm = xfm_p.tile([128, ET * CH], FP16, tag="xfm",
                                         name="xfm")
                    out_ln(attn, wo_sa_t, hid, inter, interfm)
                    if DEBUG_DUMPS and l == 0:
                        nc.sync.dma_start(dbg["dbg_q"][:], qsa[0][:])
                        nc.sync.dma_start(dbg["dbg_k"][:], ksa[0][:])
                        nc.sync.dma_start(dbg["dbg_v"][:], vp_sa[0][:])
                        nc.sync.dma_start(dbg["dbg_attn"][:], attn[0][:])
                        nc.sync.dma_start(dbg["dbg_inter"][:], inter[0][:])
                    if l < L - 1:
                        wo_sa_t = load_w(wo_sa, l + 1, E, "wo_sa")

                    # ---- CA Q + attention + out-proj + LN2 ----
                    qca = q_proj(wq_ca_t, interfm)
                    if l < L - 1:
                        wq_ca_t = load_w(wq_ca, l + 1, E, "wq_ca")
                    attn2 = [attn_p.tile([128, 512], FP16, tag="attn", name="attn")
                             for _ in range(ET)]
                    attention(qca, kca, vp_ca, KT_CA, attn2)
                    co = [stm_p.tile([128, E], FP16, tag="stm", name="stm")
                          for _ in range(TT)]
                    cofm = xfm_p.tile([128, ET * CH], FP16, tag="xfm",
                                      name="xfm")
                    out_ln(attn2, wo_ca_t, inter, co, cofm)
                    if DEBUG_DUMPS and l == 0:
                        nc.sync.dma_start(dbg["dbg_co"][:], co[0][:])
                    if l < L - 1:
                        wo_ca_t = load_w(wo_ca, l + 1, E, "wo_ca")

                    # ---- FFN: software-pipelined h1 -> gelu -> h2 ----
                    h2ps = [small_ps() for _ in range(TT)]
                    gts = {}
                    for fp in range(9):
                        if fp < 8:
                            sp2 = big_ps()
                            for j in range(2):
                                ft = fp * 2 + j
                                for ei in range(ET):
                                    nc.tensor.matmul(
                                        sp2[:, j * 512:(j + 1) * 512],
                                        w1_ts[ei][:, ft * 128:(ft + 1) * 128],
                                        cofm[:, ei * 512:(ei + 1) * 512],
                                        start=(ei == 0), stop=(ei == ET - 1))
                            gt = gel_p.tile([128, 1024],
                                            FP8 if FP8_H2 else FP16,
                                            tag="gel", name="gel")
                            if FP8_H2:
                                gt4 = gt[:].rearrange(
                                    "p (t two c) -> p t two c", t=TT, two=2)
                                for j in range(2):
                                    nc.scalar.activation(
                                        gt4[:, :, j, :],
                                        sp2[:, j * 512:(j + 1) * 512]
                                        .rearrange("p (t c) -> p t c", t=TT),
                                        AF.Gelu)
                            else:
                                for j in range(2):
                                    nc.scalar.activation(
                                        gt[:, j * 512:(j + 1) * 512],
                                        sp2[:, j * 512:(j + 1) * 512], AF.Gelu)
                            gts[fp] = gt
                        if fp >= 1:
                            gt = gts.pop(fp - 1)
                            if FP8_H2:
                                w23 = w2_t[:, (fp - 1) * 2 * E:fp * 2 * E] \
                                    .rearrange("p (two c) -> p two c", two=2)
                                for t in range(TT):
                                    nc.tensor.matmul(
                                        h2ps[t][:],
                                        gt[:, t * 256:(t + 1) * 256]
                                        .rearrange("p (two c) -> p two c",
                                                   two=2),
                                        w23, start=(fp == 1), stop=False,
                                        perf_mode=PM.DoubleRow)
                            else:
                                for j in range(2):
                                    ft = (fp - 1) * 2 + j
                                    for t in range(TT):
                                        nc.tensor.matmul(
                                            h2ps[t][:],
                                            gt[:, j * 512 + t * 128:
                                               j * 512 + (t + 1) * 128],
                                            w2_t[:, ft * E:(ft + 1) * E],
                                            start=(ft == 0), stop=False)
                    for t in range(TT):
                        nc.tensor.matmul(h2ps[t][:],
                                         ident32t[:] if FP8_H2 else identt[:],
                                         co[t][:], start=False, stop=True)
                    if l == L - 1:
                        for t in range(TT):
                            o32 = out32_p.tile([128, E], FP32, tag="out32",
                                               name="out32")
                            ln_tile(h2ps[t][:], o32[:])
                            nc.sync.dma_start(out_d[t * 128:(t + 1) * 128, :],
                                              o32[:])
                    else:
                        hidn = [stm_p.tile([128, E], FP16, tag="stm", name="stm")
                                for _ in range(TT)]
                        ownfm_n = ofm_p.tile([128, ET * CH], FP16, tag="ofm",
                                             name="ofm")
                        for t in range(TT):
                            ln_tile(h2ps[t][:], hidn[t][:])
                            fm_from_tm(ownfm_n, hidn[t], t)
                        if DEBUG_DUMPS and l == 0:
                            nc.sync.dma_start(dbg["dbg_hid1"][:], hidn[0][:])
                        ag_in = dram_p.tile([CH, E], FP16, tag="agin", name="agin")
                        for e in range(ET):
                            nc.sync.dma_start(ag_in[e * 128:(e + 1) * 128, :],
                                              ownfm_n[:, e * CH:(e + 1) * CH])
                        ag_out = dram_p.tile([S, E], FP16, tag="agout",
                                             name="agout")
                        nc.gpsimd.collective_compute(
                            "AllGather", OP.bypass, replica_groups=GROUPS,
                            ins=[ag_in.opt()], outs=[ag_out.opt()])
                        # AG-independent work fills the collective latency
                        ca_state = ca_kv(l + 1, wk_ca_t, wv_ca_t)
                        qsa = q_proj(wq_sa_t, ownfm_n)
                        ag_out_prev = ag_out
                        hid = hidn

    nc.compile()
    return nc


def _prep_inputs_fast(sen, know, sa_qkv_w, sa_qkv_b, sa_out_w, sa_out_b,
                      ca_qkv_w, ca_qkv_b, ca_out_w, ca_out_b,
                      ff_w1, ff_b1, ff_w2, ff_b2, ln_g, ln_b):
    f16 = np.float16

    def rowtile(w):  # [L,E,cols] -> [L,ET,128,cols]
        return np.ascontiguousarray(w.reshape(L, ET, 128, -1).astype(f16))

    def padv(w):  # [L,E,E] -> [L,ET,128,H*HW], no bias/ones (mask adds ones)
        wp = np.zeros((L, E, H, HW), np.float32)
        wp[:, :, :, :D] = w.reshape(L, E, H, D)
        return np.ascontiguousarray(wp.reshape(L, ET, 128, H * HW).astype(f16))

    f8 = mybir.dt.np(FP8)
    common = {
        "ident": np.eye(128, dtype=f16),
        "ident32": (np.eye(128) * (W2_SCALE if FP8_H2 else 1.0)).astype(f16),
        "wq_sa": rowtile(sa_qkv_w[:, 0]), "wk_sa": rowtile(sa_qkv_w[:, 1]),
        "wv_sa": padv(sa_qkv_w[:, 2]),
        "wo_sa": rowtile(sa_out_w),
        "wq_ca": rowtile(ca_qkv_w[:, 0]), "wk_ca": rowtile(ca_qkv_w[:, 1]),
        "wv_ca": padv(ca_qkv_w[:, 2]),
        "wo_ca": rowtile(ca_out_w),
        "w1": rowtile(ff_w1),
        "w2": np.ascontiguousarray(
            (ff_w2 * W2_SCALE).reshape(L, FT, 128, E).astype(f8))
        if FP8_H2 else
        np.ascontiguousarray(ff_w2.reshape(L, FT, 128, E).astype(f16)),
    }
    in_maps = []
    for core in range(NCORES):
        g, c = core // 4, core % 4
        m = dict(common)
        m["sen_fm"] = np.ascontiguousarray(sen[g].T.astype(f16))
        m["own_fm0"] = np.ascontiguousarray(sen[g, c * CH:(c + 1) * CH].T.astype(f16))
        m["own_tm0"] = np.ascontiguousarray(sen[g, c * CH:(c + 1) * CH].astype(f16))
        m["know_fm"] = np.ascontiguousarray(know[g].T.astype(f16))
        in_maps.append(m)
    return in_maps


def _build_general():
    """Fallback for inputs with non-zero biases / non-unit LN gamma."""
    nc = bacc.Bacc("TRN2", target_bir_lowering=False, debug=False, num_devices=NCORES)

    def din(name, shape, dt=FP16):
        return nc.dram_tensor(name, shape, dt, kind="ExternalInput").ap()

    sen_fm = din("sen_fm", [E, S])
    own_fm0 = din("own_fm0", [E, CH])
    own_tm0 = din("own_tm0", [CH, E])
    know_fm_d = din("know_fm", [E, SK])
    ident_d = din("ident", [128, 128])
    ones_d = din("ones", [1, 128])

    wq_sa = din("wq_sa", [L, ET, ET, 128, 128])
    wk_sa = din("wk_sa", [L, ET, ET, 128, 128])
    wv_sa = din("wv_sa", [L, ET, 128, H * HW])
    wo_sa = din("wo_sa", [L, ET, 128, E])
    wq_ca = din("wq_ca", [L, ET, ET, 128, 128])
    wk_ca = din("wk_ca", [L, ET, ET, 128, 128])
    wv_ca = din("wv_ca", [L, ET, 128, H * HW])
    wo_ca = din("wo_ca", [L, ET, 128, E])
    w1_d = din("w1", [L, ET, FT, 128, 128])
    w2_d = din("w2", [L, FT, 128, E])

    bq_sa = din("bq_sa", [L, 128, ET], FP32)
    bk_sa = din("bk_sa", [L, 128, ET], FP32)
    bq_ca = din("bq_ca", [L, 128, ET], FP32)
    bk_ca = din("bk_ca", [L, 128, ET], FP32)
    b1_d = din("b1", [L, 128, FT], FP32)
    rbv_sa = din("rbv_sa", [L, 1, H * HW])
    rbo_sa = din("rbo_sa", [L, 1, E])
    rbv_ca = din("rbv_ca", [L, 1, H * HW])
    rbo_ca = din("rbo_ca", [L, 1, E])
    rb2_d = din("rb2", [L, 1, E])
    lng_d = din("lng", [L, 1, E], FP32)
    lnb_d = din("lnb", [L, 1, E], FP32)

    out_d = nc.dram_tensor("out_tm", [CH, E], FP32, kind="ExternalOutput").ap()

    with tile.TileContext(nc) as tc:
        from contextlib import ExitStack
        with ExitStack() as ctx:
            ep = ctx.enter_context
            const_p = ep(tc.tile_pool(name="const", bufs=1))
            know_p = ep(tc.tile_pool(name="know", bufs=4))
            kfm_p = ep(tc.tile_pool(name="kfm", bufs=4))
            kca_p = ep(tc.tile_pool(name="kca", bufs=4))
            vp_p = ep(tc.tile_pool(name="vp", bufs=27))
            hch_p = ep(tc.tile_pool(name="hch", bufs=6))
            qfm_p = ep(tc.tile_pool(name="qfm", bufs=8))
            attn_p = ep(tc.tile_pool(name="attn", bufs=4))
            ofm_p = ep(tc.tile_pool(name="ofm", bufs=8))
            xfm_p = ep(tc.tile_pool(name="xfm", bufs=5))
            stm_p = ep(tc.tile_pool(name="stm", bufs=8))
            out32_p = ep(tc.tile_pool(name="out32", bufs=2))
            pt_p = ep(tc.tile_pool(name="pt", bufs=6))
            gel_p = ep(tc.tile_pool(name="gel", bufs=17))
            wl_p = ep(tc.tile_pool(name="wl", bufs=16))
            wr_p = ep(tc.tile_pool(name="wr", bufs=6))
            row_p = ep(tc.tile_pool(name="row", bufs=4))
            gb_p = ep(tc.tile_pool(name="gb", bufs=2))
            sc_p = ep(tc.tile_pool(name="sc", bufs=3))
            s1_p = ep(tc.tile_pool(name="s1", bufs=2))
            st_p = ep(tc.tile_pool(name="st", bufs=8))
            ps_p = ep(tc.tile_pool(name="ps", bufs=8, space="PSUM"))
            dram_p = ep(tc.tile_pool(name="dram", bufs=2, space="DRAM"))

            identt = const_p.tile([128, 128], FP16, tag="ident", name="ident")
            nc.sync.dma_start(identt[:], ident_d[:])
            onest = const_p.tile([1, 128], FP16, tag="ones", name="ones")
            nc.sync.dma_start(onest[:], ones_d[:])
            knowfm = []
            for e in range(ET):
                t = know_p.tile([128, SK], FP16, tag="know", name="know")
                nc.sync.dma_start(t[:], know_fm_d[e * 128:(e + 1) * 128, :])
                knowfm.append(t)

            hid = []
            for t in range(TT):
                h = stm_p.tile([128, E], FP16, tag="stm", name="stm")
                nc.sync.dma_start(h[:], own_tm0[t * 128:(t + 1) * 128, :])
                hid.append(h)
            ownfm = []
            for e in range(ET):
                t = ofm_p.tile([128, CH], FP16, tag="ofm", name="ofm")
                nc.sync.dma_start(t[:], own_fm0[e * 128:(e + 1) * 128, :])
                ownfm.append(t)

            def ln_norm(xres, G, Bt, out):
                stt = st_p.tile([128, 6], FP32, tag="bnst", name="bnst")
                nc.vector.bn_stats(out=stt[:], in_=xres[:])
                mv = st_p.tile([128, 2], FP32, tag="bnmv", name="bnmv")
                nc.vector.bn_aggr(out=mv[:], in_=stt[:])
                sd = st_p.tile([128, 1], FP32, tag="sd", name="sd")
                nc.scalar.activation(sd[:], mv[:, 1:2], AF.Sqrt,
                                     scale=float(E) / (E - 1))
                nc.vector.tensor_scalar_add(sd[:], sd[:], 1e-6)
                inv = st_p.tile([128, 1], FP32, tag="inv", name="inv")
                nc.vector.reciprocal_approx_fast(inv[:], sd[:])
                minv = st_p.tile([128, 1], FP32, tag="minv", name="minv")
                nc.vector.tensor_mul(minv[:], mv[:, 0:1], inv[:])
                tmp = sc_p.tile([128, E], FP32, tag="lntmp", name="lntmp")
                nc.vector.tensor_scalar(tmp[:], in0=xres[:], scalar1=inv[:],
                                        scalar2=minv[:], op0=OP.mult,
                                        op1=OP.subtract)
                nc.vector.tensor_mul(tmp[:], tmp[:], G[:])
                nc.vector.tensor_add(out[:], tmp[:], Bt[:])

            def transpose_to(dst_tiles, src_tile, t):
                for e in range(ET):
                    tp = ps_p.tile([128, 128], FP16, tag="ps", name="ps")
                    nc.tensor.transpose(tp[:], src_tile[:, e * 128:(e + 1) * 128],
                                        identt[:])
                    nc.vector.tensor_copy(dst_tiles[e][:, t * 128:(t + 1) * 128],
                                          tp[:])

            def load_w16(wdram, l):
                ts = {}
                for ei in range(ET):
                    for e in range(ET):
                        wt = wl_p.tile([128, 128], FP16, tag="wl", name="wl")
                        nc.sync.dma_start(wt[:], wdram[l, ei, e])
                        ts[ei, e] = wt
                return ts

            def load_bias(bdram, l, n):
                bt = st_p.tile([128, n], FP32, tag="bias", name="bias", bufs=6)
                nc.sync.dma_start(bt[:], bdram[l])
                return bt

            def kv_proj(kdst, n_tok, src_tiles, src_col0, wk_tiles, bkt):
                nch = n_tok // 512
                for e in range(ET):
                    for c2 in range(nch):
                        pst = ps_p.tile([128, 512], FP32, tag="ps", name="ps")
                        for ei in range(ET):
                            nc.tensor.matmul(pst[:], wk_tiles[ei, e][:],
                                             src_tiles[ei][:, c2 * 512:(c2 + 1) * 512],
                                             start=(ei == 0), stop=(ei == ET - 1))
                        nc.vector.tensor_scalar_add(
                            kdst[e][:, src_col0 + c2 * 512:src_col0 + (c2 + 1) * 512],
                            pst[:], bkt[:, e:e + 1])

            def v_proj(vdst, kt0, nkt, src_tiles, wv_tiles, rbv):
                for ktl in range(nkt):
                    vt = vdst[kt0 + ktl]
                    for half in range(2):
                        pst = ps_p.tile([128, H * HW // 2], FP32, tag="ps",
                                        name="ps")
                        cs = half * (H * HW // 2)
                        for ei in range(ET):
                            nc.tensor.matmul(
                                pst[:], src_tiles[ei][:, ktl * 128:(ktl + 1) * 128],
                                wv_tiles[ei][:, cs:cs + H * HW // 2],
                                start=(ei == 0), stop=False)
                        nc.tensor.matmul(pst[:], onest[:],
                                         rbv[:, cs:cs + H * HW // 2],
                                         start=False, stop=True)
                        nc.vector.tensor_copy(vt[:, cs:cs + H * HW // 2], pst[:])

            def attention(qfm, kfm, vp, nkt, attn_tiles):
                for hs in range(2):
                    attps = [ps_p.tile([HW, 512], FP32, tag="ps", name="ps")
                             for _ in range(4)]
                    for kt in range(nkt):
                        for h4 in range(4):
                            h = hs * 4 + h4
                            e, r = h // 2, (h % 2) * 64
                            spt = ps_p.tile([128, 512], FP32, tag="ps", name="ps")
                            nc.tensor.matmul(
                                spt[:], kfm[e][r:r + 64, kt * 128:(kt + 1) * 128],
                                qfm[e][r:r + 64, :], start=True, stop=True)
                            pt = pt_p.tile([128, 512], FP16, tag="pt", name="pt")
                            nc.scalar.activation(pt[:], spt[:], AF.Exp, scale=0.125)
                            nc.tensor.matmul(attps[h4][:],
                                             vp[kt][:, h * HW:(h + 1) * HW],
                                             pt[:], start=(kt == 0),
                                             stop=(kt == nkt - 1))
                    for h4 in range(4):
                        h = hs * 4 + h4
                        e, r = h // 2, (h % 2) * 64
                        ats = sc_p.tile([64, 512], FP32, tag="ats", name="ats",
                                        bufs=4)
                        nc.scalar.activation(ats[:], attps[h4][0:64, :], AF.Copy)
                        den = s1_p.tile([1, 512], FP32, tag="den", name="den")
                        nc.vector.tensor_copy(den[:], attps[h4][64:65, :])
                        rec = s1_p.tile([1, 512], FP32, tag="rec", name="rec")
                        nc.vector.reciprocal_approx_fast(rec[:], den[:])
                        rb = sc_p.tile([64, 512], FP32, tag="rb", name="rb")
                        nc.gpsimd.partition_broadcast(rb[:], rec[:])
                        nc.vector.tensor_mul(attn_tiles[e][r:r + 64, :],
                                             ats[:], rb[:])

            def out_proj_ln(attn_tiles, wo_tiles, rbo, res_tiles, G, Bt, out_tiles):
                for t in range(TT):
                    pst = ps_p.tile([128, E], FP32, tag="ps", name="ps")
                    for ei in range(ET):
                        nc.tensor.matmul(pst[:],
                                         attn_tiles[ei][:, t * 128:(t + 1) * 128],
                                         wo_tiles[ei][:], start=(ei == 0),
                                         stop=False)
                    nc.tensor.matmul(pst[:], onest[:], rbo[:], start=False,
                                     stop=True)
                    xres = sc_p.tile([128, E], FP32, tag="xres", name="xres")
                    nc.vector.tensor_add(xres[:], pst[:], res_tiles[t][:])
                    ln_norm(xres, G, Bt, out_tiles[t])

            def make_ca_kv(l):
                kca = [kca_p.tile([128, SK], FP16, tag="kca", name="kca")
                       for _ in range(ET)]
                wkt_ca = load_w16(wk_ca, l)
                bkt_ca = load_bias(bk_ca, l, ET)
                kv_proj(kca, SK, knowfm, 0, wkt_ca, bkt_ca)
                vp_ca = [vp_p.tile([128, H * HW], FP16, tag="vp", name="vp")
                         for _ in range(KT_CA)]
                wvt_ca = []
                for ei in range(ET):
                    wt = wr_p.tile([128, H * HW], FP16, tag="wr", name="wr")
                    nc.sync.dma_start(wt[:], wv_ca[l, ei])
                    wvt_ca.append(wt)
                rbv = row_p.tile([1, H * HW], FP16, tag="row", name="row")
                nc.sync.dma_start(rbv[:], rbv_ca[l])
                v_proj(vp_ca, 0, KT_CA, knowfm, wvt_ca, rbv)
                return kca, vp_ca

            ag_out_prev = None
            ca_kv_next = None
            for l in range(L):
                with nc.named_scope(f"L{l}"):
                    if l == 0:
                        kca, vp_ca = make_ca_kv(0)
                    else:
                        kca, vp_ca = ca_kv_next
                    lr = s1_p.tile([1, E], FP32, tag="lnrow", name="lnrow")
                    nc.sync.dma_start(lr[:], lng_d[l])
                    G = gb_p.tile([128, E], FP32, tag="G", name="G")
                    nc.gpsimd.partition_broadcast(G[:], lr[:])
                    lr2 = s1_p.tile([1, E], FP32, tag="lnrow", name="lnrow")
                    nc.sync.dma_start(lr2[:], lnb_d[l])
                    Bt = gb_p.tile([128, E], FP32, tag="B", name="B")
                    nc.gpsimd.partition_broadcast(Bt[:], lr2[:])

                    ksa = [kfm_p.tile([128, S], FP16, tag="kfm", name="kfm")
                           for _ in range(ET)]
                    vp_sa = [vp_p.tile([128, H * HW], FP16, tag="vp", name="vp")
                             for _ in range(KT_SA)]
                    wkt_sa = load_w16(wk_sa, l)
                    wvt_sa = []
                    for ei in range(ET):
                        wt = wr_p.tile([128, H * HW], FP16, tag="wr", name="wr")
                        nc.sync.dma_start(wt[:], wv_sa[l, ei])
                        wvt_sa.append(wt)
                    rbvs = row_p.tile([1, H * HW], FP16, tag="row", name="row")
                    nc.sync.dma_start(rbvs[:], rbv_sa[l])
                    bkt_sa = load_bias(bk_sa, l, ET)
                    for ch in range(4):
                        hch = []
                        for ei in range(ET):
                            ht = hch_p.tile([128, 512], FP16, tag="hch", name="hch")
                            if l == 0:
                                nc.sync.dma_start(
                                    ht[:], sen_fm[ei * 128:(ei + 1) * 128,
                                                  ch * 512:(ch + 1) * 512])
                            else:
                                nc.sync.dma_start(
                                    ht[:], ag_out_prev[ch * 512 + ei * 128:
                                                       ch * 512 + (ei + 1) * 128, :])
                            hch.append(ht)
                        kv_proj(ksa, 512, hch, ch * 512, wkt_sa, bkt_sa)
                        v_proj(vp_sa, ch * 4, 4, hch, wvt_sa, rbvs)

                    if l == 0:
                        qsa = [qfm_p.tile([128, 512], FP16, tag="qfm", name="qfm")
                               for _ in range(ET)]
                        wqt_sa = load_w16(wq_sa, l)
                        bqt = load_bias(bq_sa, l, ET)
                        for e in range(ET):
                            pst = ps_p.tile([128, 512], FP32, tag="ps", name="ps")
                            for ei in range(ET):
                                nc.tensor.matmul(pst[:], wqt_sa[ei, e][:],
                                                 ownfm[ei][:],
                                                 start=(ei == 0),
                                                 stop=(ei == ET - 1))
                            nc.vector.tensor_scalar_add(qsa[e][:], pst[:],
                                                        bqt[:, e:e + 1])
                    else:
                        qsa = qsa_next

                    attn = [attn_p.tile([128, 512], FP16, tag="attn", name="attn")
                            for _ in range(ET)]
                    attention(qsa, ksa, vp_sa, KT_SA, attn)
                    wot = []
                    for ei in range(ET):
                        wt = wr_p.tile([128, E], FP16, tag="wr", name="wr")
                        nc.sync.dma_start(wt[:], wo_sa[l, ei])
                        wot.append(wt)
                    rbo = row_p.tile([1, E], FP16, tag="row", name="row")
                    nc.sync.dma_start(rbo[:], rbo_sa[l])
                    inter = [stm_p.tile([128, E], FP16, tag="stm", name="stm")
                             for _ in range(TT)]
                    out_proj_ln(attn, wot, rbo, hid, G, Bt, inter)

                    interfm = [xfm_p.tile([128, CH], FP16, tag="xfm", name="xfm")
                               for _ in range(ET)]
                    for t in range(TT):
                        transpose_to(interfm, inter[t], t)

                    qca = [qfm_p.tile([128, 512], FP16, tag="qfm", name="qfm")
                           for _ in range(ET)]
                    wqt_ca = load_w16(wq_ca, l)
                    bqt_ca = load_bias(bq_ca, l, ET)
                    for e in range(ET):
                        pst = ps_p.tile([128, 512], FP32, tag="ps", name="ps")
                        for ei in range(ET):
                            nc.tensor.matmul(pst[:], wqt_ca[ei, e][:],
                                             interfm[ei][:],
                                             start=(ei == 0), stop=(ei == ET - 1))
                        nc.vector.tensor_scalar_add(qca[e][:], pst[:],
                                                    bqt_ca[:, e:e + 1])

                    attn2 = [attn_p.tile([128, 512], FP16, tag="attn", name="attn")
                             for _ in range(ET)]
                    attention(qca, kca, vp_ca, KT_CA, attn2)
                    wot2 = []
                    for ei in range(ET):
                        wt = wr_p.tile([128, E], FP16, tag="wr", name="wr")
                        nc.sync.dma_start(wt[:], wo_ca[l, ei])
                        wot2.append(wt)
                    rbo2 = row_p.tile([1, E], FP16, tag="row", name="row")
                    nc.sync.dma_start(rbo2[:], rbo_ca[l])
                    co = [stm_p.tile([128, E], FP16, tag="stm", name="stm")
                          for _ in range(TT)]
                    out_proj_ln(attn2, wot2, rbo2, inter, G, Bt, co)

                    cofm = [xfm_p.tile([128, CH], FP16, tag="xfm", name="xfm")
                            for _ in range(ET)]
                    for t in range(TT):
                        transpose_to(cofm, co[t], t)

                    rb2 = row_p.tile([1, E], FP16, tag="row", name="row")
                    nc.sync.dma_start(rb2[:], rb2_d[l])
                    b1t = load_bias(b1_d, l, FT)
                    gel = []
                    for ft in range(FT):
                        pst = ps_p.tile([128, 512], FP32, tag="ps", name="ps")
                        for ei in range(ET):
                            wt = wl_p.tile([128, 128], FP16, tag="wl", name="wl")
                            nc.sync.dma_start(wt[:], w1_d[l, ei, ft])
                            nc.tensor.matmul(pst[:], wt[:], cofm[ei][:],
                                             start=(ei == 0), stop=(ei == ET - 1))
                        gt = gel_p.tile([128, 512], FP16, tag="gel", name="gel")
                        nc.scalar.activation(gt[:], pst[:], AF.Gelu,
                                             bias=b1t[:, ft:ft + 1])
                        gel.append(gt)
                    w2ts = []
                    for ft in range(FT):
                        w2t = wr_p.tile([128, E], FP16, tag="w2r", name="w2r",
                                        bufs=17)
                        nc.sync.dma_start(w2t[:], w2_d[l, ft])
                        w2ts.append(w2t)
                    h2ps = [ps_p.tile([128, E], FP32, tag="ps", name="ps")
                            for _ in range(TT)]
                    for t in range(TT):
                        for ft in range(FT):
                            nc.tensor.matmul(h2ps[t][:],
                                             gel[ft][:, t * 128:(t + 1) * 128],
                                             w2ts[ft][:], start=(ft == 0),
                                             stop=False)
                    if l == L - 1:
                        hidn = [out32_p.tile([128, E], FP32, tag="out32",
                                             name="out32") for _ in range(TT)]
                    else:
                        hidn = [stm_p.tile([128, E], FP16, tag="stm", name="stm")
                                for _ in range(TT)]
                    for t in range(TT):
                        nc.tensor.matmul(h2ps[t][:], onest[:], rb2[:],
                                         start=False, stop=True)
                        xres = sc_p.tile([128, E], FP32, tag="xres", name="xres")
                        nc.vector.tensor_add(xres[:], h2ps[t][:], co[t][:])
                        ln_norm(xres, G, Bt, hidn[t])
                        if l == L - 1:
                            nc.sync.dma_start(out_d[t * 128:(t + 1) * 128, :],
                                              hidn[t][:])

                    if l < L - 1:
                        ownfm_n = [ofm_p.tile([128, CH], FP16, tag="ofm",
                                              name="ofm") for _ in range(ET)]
                        for t in range(TT):
                            transpose_to(ownfm_n, hidn[t], t)
                        ag_in = dram_p.tile([CH, E], FP16, tag="agin", name="agin")
                        for e in range(ET):
                            nc.sync.dma_start(ag_in[e * 128:(e + 1) * 128, :],
                                              ownfm_n[e][:])
                        ag_out = dram_p.tile([S, E], FP16, tag="agout",
                                             name="agout")
                        nc.gpsimd.collective_compute(
                            "AllGather", OP.bypass, replica_groups=GROUPS,
                            ins=[ag_in.opt()], outs=[ag_out.opt()])
                        ca_kv_next = make_ca_kv(l + 1)
                        qsa_next = [qfm_p.tile([128, 512], FP16, tag="qfm",
                                               name="qfm") for _ in range(ET)]
                        wqt_n = load_w16(wq_sa, l + 1)
                        bqt_n = load_bias(bq_sa, l + 1, ET)
                        for e in range(ET):
                            pst = ps_p.tile([128, 512], FP32, tag="ps", name="ps")
                            for ei in range(ET):
                                nc.tensor.matmul(pst[:], wqt_n[ei, e][:],
                                                 ownfm_n[ei][:],
                                                 start=(ei == 0),
                                                 stop=(ei == ET - 1))
                            nc.vector.tensor_scalar_add(qsa_next[e][:], pst[:],
                                                        bqt_n[:, e:e + 1])
                        ag_out_prev = ag_out
                        ownfm = ownfm_n
                        hid = hidn

    nc.compile()
    return nc


def _prep_inputs(sen, know, sa_qkv_w, sa_qkv_b, sa_out_w, sa_out_b,
                 ca_qkv_w, ca_qkv_b, ca_out_w, ca_out_b,
                 ff_w1, ff_b1, ff_w2, ff_b2, ln_g, ln_b):
    """Host-side weight packing for the general fallback kernel."""
    f16, f32 = np.float16, np.float32

    def tile4(w):
        return np.ascontiguousarray(
            w.reshape(L, ET, 128, ET, 128).transpose(0, 1, 3, 2, 4).astype(f16))

    def padv(w, b):
        wp = np.zeros((L, E, H, HW), f32)
        wp[:, :, :, :D] = w.reshape(L, E, H, D)
        bp = np.zeros((L, H, HW), f32)
        bp[:, :, :D] = b.reshape(L, H, D)
        bp[:, :, D] = 1.0
        return (np.ascontiguousarray(wp.reshape(L, ET, 128, H * HW).astype(f16)),
                np.ascontiguousarray(bp.reshape(L, 1, H * HW).astype(f16)))

    wv_sa_p, rbv_sa_h = padv(sa_qkv_w[:, 2], sa_qkv_b[:, 2])
    wv_ca_p, rbv_ca_h = padv(ca_qkv_w[:, 2], ca_qkv_b[:, 2])

    common = {
        "ident": np.eye(128, dtype=f16),
        "ones": np.ones((1, 128), f16),
        "wq_sa": tile4(sa_qkv_w[:, 0]), "wk_sa": tile4(sa_qkv_w[:, 1]),
        "wv_sa": wv_sa_p,
        "wo_sa": np.ascontiguousarray(sa_out_w.reshape(L, ET, 128, E).astype(f16)),
        "wq_ca": tile4(ca_qkv_w[:, 0]), "wk_ca": tile4(ca_qkv_w[:, 1]),
        "wv_ca": wv_ca_p,
        "wo_ca": np.ascontiguousarray(ca_out_w.reshape(L, ET, 128, E).astype(f16)),
        "w1": np.ascontiguousarray(
            ff_w1.reshape(L, ET, 128, FT, 128).transpose(0, 1, 3, 2, 4).astype(f16)),
        "w2": np.ascontiguousarray(ff_w2.reshape(L, FT, 128, E).astype(f16)),
        "bq_sa": np.ascontiguousarray(
            sa_qkv_b[:, 0].reshape(L, ET, 128).transpose(0, 2, 1)),
        "bk_sa": np.ascontiguousarray(
            sa_qkv_b[:, 1].reshape(L, ET, 128).transpose(0, 2, 1)),
        "bq_ca": np.ascontiguousarray(
            ca_qkv_b[:, 0].reshape(L, ET, 128).transpose(0, 2, 1)),
        "bk_ca": np.ascontiguousarray(
            ca_qkv_b[:, 1].reshape(L, ET, 128).transpose(0, 2, 1)),
        "b1": np.ascontiguousarray(
            ff_b1.reshape(L, FT, 128).transpose(0, 2, 1)),
        "rbv_sa": rbv_sa_h, "rbv_ca": rbv_ca_h,
        "rbo_sa": np.ascontiguousarray(sa_out_b[:, None, :].astype(f16)),
        "rbo_ca": np.ascontiguousarray(ca_out_b[:, None, :].astype(f16)),
        "rb2": np.ascontiguousarray(ff_b2[:, None, :].astype(f16)),
        "lng": np.ascontiguousarray(ln_g[:, None, :]),
        "lnb": np.ascontiguousarray(ln_b[:, None, :]),
    }
    in_maps = []
    for core in range(NCORES):
        g, c = core // 4, core % 4
        m = dict(common)
        m["sen_fm"] = np.ascontiguousarray(sen[g].T.astype(f16))
        m["own_fm0"] = np.ascontiguousarray(sen[g, c * CH:(c + 1) * CH].T.astype(f16))
        m["own_tm0"] = np.ascontiguousarray(sen[g, c * CH:(c + 1) * CH].astype(f16))
        m["know_fm"] = np.ascontiguousarray(know[g].T.astype(f16))
        in_maps.append(m)
    return in_maps


def _inputs_are_fast(sa_qkv_b, sa_out_b, ca_qkv_b, ca_out_b,
                     ff_b1, ff_b2, ln_g, ln_b, **_):
    zeros = [sa_qkv_b, sa_out_b, ca_qkv_b, ca_out_b, ff_b1, ff_b2, ln_b]
    return all(not np.any(z) for z in zeros) and np.all(ln_g == 1.0)


def kernel(**inputs):
    inputs = {k: np.asarray(v, dtype=np.float32) for k, v in inputs.items()}
    if _inputs_are_fast(**inputs):
        if "nc" not in _CACHE:
            _CACHE["nc"] = _build_fast()
        nc = _CACHE["nc"]
        in_maps = _prep_inputs_fast(**inputs)
    else:
        if "nc_gen" not in _CACHE:
            _CACHE["nc_gen"] = _build_general()
        nc = _CACHE["nc_gen"]
        in_maps = _prep_inputs(**inputs)
        _CACHE["nc"] = nc
    res = run_bass_kernel_spmd(nc, in_maps, list(range(NCORES)))
    out = np.empty((B, S, E), np.float32)
    for core in range(NCORES):
        g, c = core // 4, core % 4
        out[g, c * CH:(c + 1) * CH] = res.results[core]["out_tm"]
    return out
